# revision 7
# baseline (speedup 1.0000x reference)
"""DetectionLoss Bass/Tile kernel for TRN2, v2 (one core = one image; SPMD x8).

fp16 data path (coords pre-scaled by 1/64 on host), per-j scalar ops from a
broadcast gt table, DVE 2x/4x perf modes, Pool runs the argmax chain,
ACT runs relu + focal activations, PE does the one-hot gather matmuls.

Per core (image b), layout: anchor a <-> (partition p = a // COLS, col a % COLS).
Inputs (per core, planar, host-packed):
  anch [4, A] f16 (x1,y1,x2,y2 scaled), clsp [8, A] f16, regp [4, A] f16,
  gtaux [1, 320] f32 = gx1 gy1 gx2 gy2 aB xg yg lwg lhg label (each [32])
Output: out [1, 4] f32 = [npos, sl1_sum, nsum, corr] partial sums; host finishes.
"""
import dataclasses
import numpy as np

import concourse.bass as bass
import concourse.mybir as mybir
from concourse import tile

AL = mybir.AluOpType
AF = mybir.ActivationFunctionType
f32 = mybir.dt.float32
f16 = mybir.dt.float16

P = 128
G = 32
C = 8
BETA = 1.0 / 9.0
THIRD = 1.0 / 3.0
LN_THIRD = float(np.log(np.float32(1.0) / np.float32(3.0)))
CSCALE = 1.0 / 64.0


def patch_tile_drain(maxw: int = 1):
    """Split the TileContext exit drain's sem waits across NOPs (walrus
    setupSyncWait rejects >1 wait on a CTRL instruction in this build)."""
    import concourse.tile as tile_mod
    from concourse.vector_clock import ScopedClock

    def _drain_and_barrier(self, tick_clock, wait_clock):
        drain_inst = self.nc.sync.drain()
        wait_clock.add_sem_waits(
            drain_inst.ins, ScopedClock({None: tick_clock.global_clock})
        )
        si = drain_inst.ins.sync_info
        waits = list(si.on_wait)
        if len(waits) > maxw:
            si.on_wait = waits[:maxw]
            rest = waits[maxw:]
            for i in range(0, len(rest), maxw):
                nop = self.nc.sync.nop(nofuse=True, hint="drain_split")
                nop.ins.sync_info = mybir.SyncInfo(
                    on_wait=rest[i:i + maxw], on_update=[]
                )
        self.nc.all_engine_barrier()
        assert self.sems is not None
        popped = self.nc._tile_sem_poison_stack.pop()
        assert popped is self._sem_poison
        self.nc.clear_and_free_semaphores(list(self.sems.allocated().values()))
        self.nc.all_engine_barrier()

    tile_mod.TileContext._drain_and_barrier = _drain_and_barrier


def split_sync_waits(nc, maxw: int = 1):
    """Walrus rejects >2 sem waits on one instruction (and >1 on CTRL-type).
    Hoist excess waits onto same-engine NOPs inserted immediately before."""
    ctr = [0]

    def mknop(engine, waits):
        ctr[0] += 1
        nop = mybir.InstNoOp(name=f"I-wsplit-{ctr[0]}", ins=[], outs=[])
        nop.engine = engine
        nop.sync_info = mybir.SyncInfo(on_wait=waits, on_update=[])
        return nop

    for blk in nc.bb_map.values():
        bb = blk.bb
        il = bb.instructions
        i = 0
        while i < len(il):
            inst = il[i]
            si = inst.sync_info
            mw = 1 if isinstance(inst, mybir.InstTensorScalarPtr) else maxw
            if si is not None and len(si.on_wait) > mw:
                waits = list(si.on_wait)
                si.on_wait = waits[:mw]
                rest = waits[mw:]
                for k in range(0, len(rest), 1):
                    il.insert(i, mknop(inst.engine, rest[k:k + 1]))
                    i += 1
            i += 1


def _expand32(ap):
    """[P, n] AP -> [P, n, 32] with step-0 inner dim (broadcast)."""
    return dataclasses.replace(ap, ap=ap.ap + [[0, G]])


def build(A: int, ohd: int = 3):
    """Emit the per-core program. A must be divisible by 128.
    ohd: number of one-hot superquads handled by DVE (rest on Pool)."""
    assert A % P == 0
    COLS = A // P                    # 1250
    NSQ = (COLS + 15) // 16          # superquads (16 cols each)
    if NSQ % 2:
        NSQ += 1                     # two equal halves
    W = NSQ * 16                     # 1280
    HS = NSQ // 2                    # sq per half
    WH = W // 2                      # 640

    nc = bass.Bass()
    anch = nc.declare_dram_parameter("anch", [4, A], f16, isOutput=False)
    clsp = nc.declare_dram_parameter("clsp", [C, A], f16, isOutput=False)
    regp = nc.declare_dram_parameter("regp", [4, A], f16, isOutput=False)
    gtaux = nc.declare_dram_parameter("gtaux", [1, 11 * G], f32, isOutput=False)
    out = nc.declare_dram_parameter("out", [1, 4], f32, isOutput=True)

    def plane(t, c):
        return t[c].rearrange("(p w) -> p w", p=P)

    def ts(eng, o, i0, s1, op0, s2=None, op1=None, acc=None):
        kw = {}
        if op1 is not None:
            kw["op1"] = op1
        if acc is not None:
            kw["accum_out"] = acc
        eng.tensor_scalar(out=o, in0=i0, scalar1=s1, scalar2=s2, op0=op0, **kw)

    with tile.TileContext(nc) as tc:
        from contextlib import ExitStack
        with ExitStack() as ctx:
            const = ctx.enter_context(tc.tile_pool(name="const", bufs=1))
            persist = ctx.enter_context(tc.tile_pool(name="persist", bufs=1))

            # ---------- constants ----------
            iotarep = const.tile([P, 512], f16, name="iotarep")
            nc.gpsimd.iota(iotarep[:], pattern=[[0, 16], [1, G]], base=1,
                           channel_multiplier=0,
                           allow_small_or_imprecise_dtypes=True)
            irow = const.tile([P, P], f32, name="irow")
            nc.gpsimd.iota(irow[:], pattern=[[1, P]], base=0,
                           channel_multiplier=0,
                           allow_small_or_imprecise_dtypes=True)
            icol = const.tile([P, 1], f32, name="icol")
            nc.gpsimd.iota(icol[:], pattern=[[0, 1]], base=0,
                           channel_multiplier=1,
                           allow_small_or_imprecise_dtypes=True)
            ident = const.tile([P, P], f16, name="ident")
            ts(nc.vector, ident[:], irow[:], icol[:], AL.is_equal)
            ones1 = const.tile([P, 1], f32, name="ones1")
            nc.gpsimd.memset(ones1[:], 1.0)
            lnb = const.tile([P, 1], f32, name="lnb")
            nc.gpsimd.memset(lnb[:], 1e-7)

            # gt broadcast [P, 320] f32
            gtb = const.tile([P, 11 * G], f32, name="gtb")
            gsrc = gtaux[:]
            gsrc_b = dataclasses.replace(gsrc, ap=[[0, P]] + gsrc.ap[1:])
            nc.sync.dma_start(gtb[:], gsrc_b)

            def gsc(k, j):
                # [P,1] f32 scalar ptr for gt field k, gt j
                return gtb[:, k * G + j:k * G + j + 1]

            # gather table tt16 [P, 20] f16, block-diag:
            # rows 32fs..+32, cols 5fs..+5 = [xg yg lwg lhg labelf]
            NF = 5
            tt16 = const.tile([P, 4 * NF], f16, name="tt16")
            nc.gpsimd.memset(tt16[:], 0.0)
            traw = const.tile([G, 10], f32, name="traw")
            gsrc2 = dataclasses.replace(gsrc, ap=[[1, G], [G, 10]])
            nc.sync.dma_start(traw[:], gsrc2)
            tblk = const.tile([G, NF], f16, name="tblk")
            nc.scalar.copy(tblk[:, 0:NF], traw[:, 5:10])
            for fs in range(4):
                nc.sync.dma_start(tt16[32 * fs:32 * fs + 32,
                                       NF * fs:NF * fs + NF], tblk[:])

            # ---------- anchors + per-anchor prep (all f16) ----------
            pA_stack = ExitStack()
            pA = pA_stack.enter_context(tc.tile_pool(name="pA", bufs=1))
            ax1 = pA.tile([P, COLS], f16, name="ax1")
            ay1 = pA.tile([P, COLS], f16, name="ay1")
            ax2 = pA.tile([P, COLS], f16, name="ax2")
            ay2 = pA.tile([P, COLS], f16, name="ay2")
            for t, c in ((ax1, 0), (ay1, 1), (ax2, 2), (ay2, 3)):
                nc.sync.dma_start(t[:], plane(anch, c))
            # cls/reg planes (f16); DMAs staggered into the j-loop
            xcp = [persist.tile([P, COLS], f16, name=f"xcp{c}") for c in range(C)]
            rpp = [persist.tile([P, COLS], f16, name=f"rpp{k}") for k in range(4)]

            area_a = pA.tile([P, COLS], f16, name="area_a")
            with tc.tile_pool(name="areap", bufs=1) as areap:
                wa0 = areap.tile([P, COLS], f16, name="wa0")
                ha0 = areap.tile([P, COLS], f16, name="ha0")
                nc.vector.tensor_tensor(out=wa0[:], in0=ax2[:], in1=ax1[:],
                                        op=AL.subtract)
                nc.gpsimd.tensor_tensor(out=ha0[:], in0=ay2[:], in1=ay1[:],
                                        op=AL.subtract)
                nc.vector.tensor_tensor(out=area_a[:], in0=wa0[:],
                                        in1=ha0[:], op=AL.mult)
            # xa/ya/iwa/iha/La/Ha are computed after the j-loop (phase B prep)
            xa = persist.tile([P, COLS], f16, name="xa")
            ya = persist.tile([P, COLS], f16, name="ya")
            iwa = persist.tile([P, COLS], f16, name="iwa")
            iha = persist.tile([P, COLS], f16, name="iha")
            La = persist.tile([P, COLS], f16, name="La")
            Ha = persist.tile([P, COLS], f16, name="Ha")

            m2 = [pA.tile([P, COLS], f16, name=f"m{i}") for i in range(2)]
            bestp = pA.tile([P, COLS], f16, name="bestp")
            nc.gpsimd.memset(m2[1][:], -60000.0)
            nc.gpsimd.memset(bestp[:], 0.0)

            # accumulators
            nposA = persist.tile([P, 1], f32, name="nposA")
            sl1A = persist.tile([P, 1], f32, name="sl1A")
            nsumA = persist.tile([P, 1], f32, name="nsumA")
            corrA = persist.tile([P, 1], f32, name="corrA")
            tacc = persist.tile([P, 1], f32, name="tacc")
            for t in (nposA, sl1A, nsumA, corrA):
                nc.vector.memset(t[:], 0.0)

            # focal result planes (retained through phase B)
            Rp = [persist.tile([P, COLS], f16, name=f"Rp{c}") for c in range(C)]

            # ---------- phase A: j-loop + interleaved focal ----------
            with tc.tile_pool(name="jt", bufs=1) as jt:
                t_ltx = [jt.tile([P, COLS], f16, name=f"ltx{i}") for i in range(2)]
                t_mnx = [jt.tile([P, COLS], f16, name=f"mnx{i}") for i in range(2)]
                t_wxr = [jt.tile([P, COLS], f16, name=f"wxr{i}") for i in range(2)]
                t_lty = [jt.tile([P, COLS], f16, name=f"lty{i}") for i in range(2)]
                t_mny = [jt.tile([P, COLS], f16, name=f"mny{i}") for i in range(2)]
                t_wyr = [jt.tile([P, COLS], f16, name=f"wyr{i}") for i in range(2)]
                t_wxp = [jt.tile([P, COLS], f16, name=f"wxp{i}") for i in range(3)]
                wyp_t = [jt.tile([P, COLS], f16, name=f"wyp{i}") for i in range(3)]
                t_li = [jt.tile([P, COLS], f16, name=f"li{i}") for i in range(3)]
                t_den = [jt.tile([P, COLS], f16, name=f"den{i}") for i in range(2)]
                t_int = [jt.tile([P, COLS], f16, name=f"int{i}") for i in range(3)]
                t_t = [jt.tile([P, COLS], f16, name=f"tt{i}") for i in range(3)]
                t_upd = [jt.tile([P, COLS], f16, name=f"upd{i}") for i in range(2)]
                # focal temps
                f_sp = [jt.tile([P, COLS], f16, name=f"fsp{i}") for i in range(2)]
                f_spn = [jt.tile([P, COLS], f16, name=f"fspn{i}") for i in range(2)]
                f_sig = [jt.tile([P, COLS], f16, name=f"fsig{i}") for i in range(2)]
                f_sgn = [jt.tile([P, COLS], f16, name=f"fsgn{i}") for i in range(2)]
                f_s2 = [jt.tile([P, COLS], f16, name=f"fs2{i}") for i in range(2)]
                f_nt = [jt.tile([P, COLS], f16, name=f"fnt{i}") for i in range(2)]
                f_sq = [jt.tile([P, COLS], f16, name=f"fsq{i}") for i in range(2)]
                nacc = [persist.tile([P, 1], f32, name=f"nacc{i}") for i in range(2)]

                def stA(j):
                    v = nc.vector
                    nc.scalar.activation(t_ltx[j % 2][:], ax1[:], AF.Relu,
                                         bias=gsc(10, j))
                    ts(v, t_mnx[j % 2][:], ax2[:], gsc(2, j), AL.min,
                       gsc(0, j), AL.subtract)
                    ts(v, t_lty[j % 2][:], ay1[:], gsc(1, j), AL.max)
                    ts(v, t_mny[j % 2][:], ay2[:], gsc(3, j), AL.min)

                def stB(j):
                    v = nc.vector
                    v.tensor_tensor(out=t_wxr[j % 2][:], in0=t_mnx[j % 2][:],
                                    in1=t_ltx[j % 2][:], op=AL.subtract)
                    v.tensor_tensor(out=t_wyr[j % 2][:], in0=t_mny[j % 2][:],
                                    in1=t_lty[j % 2][:], op=AL.subtract)
                    ts(v, wyp_t[j % 3][:], t_wyr[j % 2][:], 0.0, AL.max)

                def stC(j):
                    nc.scalar.activation(t_wxp[j % 3][:], t_wxr[j % 2][:],
                                         AF.Relu)

                def stD(j):
                    nc.vector.tensor_tensor(out=t_int[j % 3][:],
                                            in0=t_wxp[j % 3][:],
                                            in1=wyp_t[j % 3][:], op=AL.mult)

                def stE(j):
                    nc.scalar.activation(t_li[j % 3][:], t_int[j % 3][:],
                                         AF.Ln, bias=lnb[:])
                    nc.scalar.activation(t_den[j % 2][:], area_a[:], AF.Ln,
                                         bias=gsc(4, j))

                def stF(j):
                    v = nc.vector
                    tj = t_t[j % 3]
                    mprev = m2[(j + 1) % 2]
                    mcur = m2[j % 2]
                    v.tensor_tensor(out=tj[:], in0=t_li[j % 3][:],
                                    in1=t_den[j % 2][:], op=AL.subtract)
                    v.tensor_tensor(out=mcur[:], in0=mprev[:], in1=tj[:],
                                    op=AL.max)
                    nc.gpsimd.tensor_tensor(out=t_upd[j % 2][:], in0=tj[:],
                                            in1=mcur[:], op=AL.subtract)

                def stG(j):
                    v = nc.vector
                    upd = t_upd[j % 2]
                    ts(v, upd[:], upd[:], 0.0, AL.is_ge, float(j + 1), AL.mult)
                    v.tensor_tensor(out=bestp[:], in0=bestp[:], in1=upd[:],
                                    op=AL.max)

                stages = [stA, stB, stC, stD, stE, stF, stG]

                # focal for class c, split into 6 emission slices.
                # exp/ln formulation (single ACT table set):
                #   E = e^-x, u = 1+E, spn = ln(u) = softplus(-x),
                #   sp = x + spn = softplus(x), sgn = E/u = sigmoid(-x),
                #   sig = 1-sgn, N = sig^2*sp, P = sgn^2*spn, Rp = P/3 - N
                def focal_slice(c, s):
                    if c >= C:
                        return
                    v = nc.vector
                    xc = xcp[c]
                    E = f_sig[c % 2]; u = f_sp[c % 2]; spn = f_spn[c % 2]
                    sgn = f_sgn[c % 2]; s2n = f_s2[c % 2]; nt = f_nt[c % 2]
                    sp = u      # overwrites u after spn is computed
                    sig = E     # overwrites E (E dead after u)
                    s2 = f_sq[c % 2]
                    if s == 0:
                        nc.scalar.activation(E[:], xc[:], AF.Exp, scale=-1.0)
                    elif s == 1:
                        ts(v, u[:], E[:], 1.0, AL.add)
                        nc.scalar.activation(spn[:], u[:], AF.Ln)
                    elif s == 2:
                        v.tensor_tensor(out=sp[:], in0=xc[:], in1=spn[:],
                                        op=AL.add)
                        # sgn = sigmoid(-x) = exp(-softplus(x))
                        nc.scalar.activation(sgn[:], sp[:], AF.Exp, scale=-1.0)
                    elif s == 3:
                        nc.scalar.activation(s2n[:], sgn[:], AF.Square)
                        ts(v, sig[:], sgn[:], -1.0, AL.mult, 1.0, AL.add)
                    elif s == 4:
                        nc.scalar.activation(s2[:], sig[:], AF.Square)
                        v.scalar_tensor_tensor(
                            out=nt[:], in0=s2[:], scalar=0.0, in1=sp[:],
                            op0=AL.add, op1=AL.mult,
                            accum_out=nacc[c % 2][:])
                        nc.vector.tensor_tensor(out=nsumA[:], in0=nsumA[:],
                                                in1=nacc[c % 2][:], op=AL.add)
                    else:
                        nc.gpsimd.tensor_tensor(out=s2n[:], in0=s2n[:],
                                                in1=spn[:], op=AL.mult)
                        ts(nc.gpsimd, s2n[:], s2n[:], 1.0 / 3.0, AL.mult)
                        nc.gpsimd.tensor_tensor(out=Rp[c][:], in0=s2n[:],
                                                in1=nt[:], op=AL.subtract)

                # class c slices at j = 4c .. 4c+5 (overlap ok: c%2 buffers)
                sched = {}
                for c in range(C):
                    for s in range(6):
                        sched.setdefault(4 * c + s, []).append((c, s))
                NS = len(stages)
                for k in range(G + NS - 1):
                    if k % 4 == 0 and k // 4 < C:
                        nc.sync.dma_start(xcp[k // 4][:], plane(clsp, k // 4))
                    if k >= 24 and k % 2 == 0 and (k - 24) // 2 < 4:
                        k4 = (k - 24) // 2
                        nc.sync.dma_start(rpp[k4][:], plane(regp, k4))
                    for si, st in enumerate(stages):
                        j = k - si
                        if 0 <= j < G:
                            st(j)
                    for (c, s) in sched.get(k, []):
                        focal_slice(c, s)

            # ---------- pos, bpm, deferred anchor prep ----------
            pos = persist.tile([P, COLS], f16, name="pos")
            ts(nc.vector, pos[:], m2[1][:], LN_THIRD, AL.is_ge, None, AL.add,
               acc=tacc[:])
            nc.vector.tensor_tensor(out=nposA[:], in0=nposA[:], in1=tacc[:],
                                    op=AL.add)
            bpm = persist.tile([P, W], f16, name="bpm")
            nc.gpsimd.memset(bpm[:], 0.0)
            nc.vector.tensor_tensor(out=bpm[:, 0:COLS], in0=pos[:],
                                    in1=bestp[:], op=AL.mult)
            # xa/ya/iwa/iha/La/Ha (anchors still alive)
            with tc.tile_pool(name="prepp", bufs=1) as prepp:
                wa = prepp.tile([P, COLS], f16, name="wa")
                ha = prepp.tile([P, COLS], f16, name="ha")
                nc.vector.tensor_tensor(out=wa[:], in0=ax2[:], in1=ax1[:],
                                        op=AL.subtract)
                nc.gpsimd.tensor_tensor(out=ha[:], in0=ay2[:], in1=ay1[:],
                                        op=AL.subtract)
                nc.gpsimd.tensor_tensor(out=xa[:], in0=ax1[:], in1=ax2[:],
                                        op=AL.add)
                ts(nc.gpsimd, xa[:], xa[:], 0.5, AL.mult)
                nc.vector.tensor_tensor(out=ya[:], in0=ay1[:], in1=ay2[:],
                                        op=AL.add)
                ts(nc.vector, ya[:], ya[:], 0.5, AL.mult)
                with nc.allow_low_precision(reason="f16 reg-target recips"):
                    nc.vector.reciprocal(iwa[:], wa[:])
                    nc.vector.reciprocal(iha[:], ha[:])
                nc.scalar.activation(La[:], wa[:], AF.Ln)
                nc.scalar.activation(Ha[:], ha[:], AF.Ln)
            pA_stack.close()

            # ---------- phase B ----------
            with ExitStack() as bctx:
                ohp = bctx.enter_context(tc.tile_pool(name="ohp", bufs=3))
                psum_t = bctx.enter_context(
                    tc.tile_pool(name="psum_t", bufs=3, space="PSUM"))
                psum_g = bctx.enter_context(
                    tc.tile_pool(name="psum_g", bufs=3, space="PSUM"))
                gath_p = bctx.enter_context(tc.tile_pool(name="gath", bufs=2))
                scr = bctx.enter_context(tc.tile_pool(name="scr", bufs=1))

                sc = [scr.tile([P, WH], f16, name=f"sc{i}") for i in range(8)]
                accp = [persist.tile([P, 1], f32, name=f"accp{i}")
                        for i in range(2)]

                for half in range(2):
                    base = half * WH
                    rw = min(COLS - base, WH)
                    if rw <= 0:
                        break
                    gath = gath_p.tile([P, NF * WH], f16, name="gath")

                    def gpl(mm):
                        return gath[:, mm * WH:mm * WH + rw]

                    for s in range(HS):
                        sq = half * HS + s
                        oh = ohp.tile([P, 512], f16, name="oh")
                        srcx = _expand32(bpm[:, 16 * sq:16 * sq + 16])
                        if (sq % 10) < ohd:
                            nc.vector.tensor_tensor(
                                out=oh[:].rearrange("p (f j) -> p f j", j=G),
                                in0=srcx,
                                in1=iotarep[:].rearrange("p (f j) -> p f j",
                                                         j=G),
                                op=AL.is_equal)
                        else:
                            # Pool: e = bpm - iota; DVE: oh = (e == 0)
                            nc.gpsimd.tensor_tensor(
                                out=oh[:].rearrange("p (f j) -> p f j", j=G),
                                in0=srcx,
                                in1=iotarep[:].rearrange("p (f j) -> p f j",
                                                         j=G),
                                op=AL.subtract)
                            ts(nc.vector, oh[:], oh[:], 0.0, AL.is_equal)
                        pt = psum_t.tile([P, 512], f16, name="pt")
                        for t4 in range(4):
                            nc.tensor.transpose(pt[:, 128 * t4:128 * t4 + 128],
                                                oh[:, 128 * t4:128 * t4 + 128],
                                                ident[:])
                        ohT = ohp.tile([P, 512], f16, name="ohT")
                        if s % 4 == 0:
                            nc.vector.tensor_copy(ohT[:], pt[:])
                        else:
                            nc.scalar.copy(ohT[:], pt[:])
                        gp = psum_g.tile([P, 4 * NF * 4], f32, name="gp")
                        for t4 in range(4):
                            nc.tensor.matmul(
                                out=gp[:, 4 * NF * t4:4 * NF * t4 + 4 * NF],
                                lhsT=ohT[:, 128 * t4:128 * t4 + 128],
                                rhs=tt16[:], start=True, stop=True)
                        src_g = gp[:].rearrange("p (t f mm) -> p t f mm",
                                                t=4, f=4)
                        dst = gath[:]
                        dst_ap = dataclasses.replace(
                            dst, offset=dst.offset + 16 * s,
                            ap=[dst.ap[0], [4, 4], [1, 4], [WH, NF]])
                        if s % 8 < 3:
                            nc.vector.tensor_copy(dst_ap, src_g)
                        else:
                            nc.scalar.copy(dst_ap, src_g)

                    for q0, q1 in ((0, rw // 2), (rw // 2, rw)):
                      qw = q1 - q0
                      posh = pos[:, base + q0:base + q1]

                      def gplq(mm):
                          return gath[:, mm * WH + q0:mm * WH + q1]

                      # ---- reg: targets + smooth-L1 ----
                      for k, (ctr_t, inv_t, lg_t) in enumerate(
                              ((xa, iwa, None), (ya, iha, None),
                               (None, None, La), (None, None, Ha))):
                        s1, s2_, s3, s4 = sc[4 * (k % 2):4 * (k % 2) + 4]
                        rt = s1
                        if lg_t is None:
                            nc.vector.tensor_tensor(
                                out=s2_[:, :qw], in0=gplq(k),
                                in1=ctr_t[:, base + q0:base + q1],
                                op=AL.subtract)
                            nc.vector.tensor_tensor(
                                out=rt[:, :qw], in0=s2_[:, :qw],
                                in1=inv_t[:, base + q0:base + q1], op=AL.mult)
                        else:
                            nc.vector.tensor_tensor(
                                out=rt[:, :qw], in0=gplq(k),
                                in1=lg_t[:, base + q0:base + q1],
                                op=AL.subtract)
                        e = s2_
                        nc.vector.tensor_tensor(
                            out=e[:, :qw],
                            in0=rpp[k][:, base + q0:base + q1],
                            in1=rt[:, :qw], op=AL.subtract)
                        q = s3
                        nc.scalar.activation(q[:, :qw], e[:, :qw], AF.Abs)
                        qm = s4
                        nc.vector.tensor_tensor(out=qm[:, :qw], in0=q[:, :qw],
                                                in1=posh, op=AL.mult)
                        cm = s1  # reuse rt
                        ts(nc.vector, cm[:, :qw], qm[:, :qw], BETA, AL.min)
                        q2 = s3  # reuse q: 2*qm - cm
                        nc.vector.tensor_tensor(out=q2[:, :qw], in0=qm[:, :qw],
                                                in1=qm[:, :qw], op=AL.add)
                        nc.vector.tensor_tensor(out=q2[:, :qw], in0=q2[:, :qw],
                                                in1=cm[:, :qw], op=AL.subtract)
                        nc.vector.scalar_tensor_tensor(
                            out=s4[:, :qw], in0=cm[:, :qw], scalar=0.0,
                            in1=q2[:, :qw], op0=AL.add, op1=AL.mult,
                            accum_out=accp[k % 2][:])
                        nc.vector.tensor_tensor(out=sl1A[:], in0=sl1A[:],
                                                in1=accp[k % 2][:], op=AL.add)

                      # ---- corr dots: label-select over Rp planes ----
                      for c in range(C):
                        eqc = sc[4 + (c % 2)]
                        # table holds label+1, so background (0) matches none
                        ts(nc.vector, eqc[:, :qw], gplq(4), float(c + 1),
                           AL.is_equal)
                        cc = sc[6 + (c % 2)]
                        nc.vector.scalar_tensor_tensor(
                            out=cc[:, :qw], in0=eqc[:, :qw], scalar=0.0,
                            in1=Rp[c][:, base + q0:base + q1],
                            op0=AL.add, op1=AL.mult,
                            accum_out=accp[c % 2][:])
                        nc.vector.tensor_tensor(out=corrA[:], in0=corrA[:],
                                                in1=accp[c % 2][:], op=AL.add)

            # ---------- final cross-partition reduce ----------
            acc4 = persist.tile([P, 4], f32, name="acc4")
            nc.scalar.copy(acc4[:, 0:1], nposA[:])
            nc.scalar.copy(acc4[:, 1:2], sl1A[:])
            nc.scalar.copy(acc4[:, 2:3], nsumA[:])
            nc.scalar.copy(acc4[:, 3:4], corrA[:])
            with tc.tile_pool(name="psum_f", bufs=1, space="PSUM") as pf:
                fps = pf.tile([1, 4], f32, name="fps")
                nc.tensor.matmul(out=fps[:], lhsT=ones1[:], rhs=acc4[:],
                                 start=True, stop=True)
                osb = persist.tile([1, 4], f32, name="osb")
                nc.scalar.copy(osb[:], fps[:])
                nc.sync.dma_start(out[:], osb[:])

    return nc


def build_for_timing():
    patch_tile_drain(1)
    nc = build(160000)
    split_sync_waits(nc)
    return nc


# ---------------- host side ----------------

def pack_inputs(cls_preds, reg_preds, anchors, gt_boxes, gt_labels):
    """Full inputs -> list of 8 per-core input maps (planar f16 layouts)."""
    B, A, _ = cls_preds.shape
    anch = np.ascontiguousarray(
        (anchors.astype(np.float32).T * np.float32(CSCALE)).astype(np.float16))
    maps = []
    for b in range(B):
        clsp = np.ascontiguousarray(
            cls_preds[b].astype(np.float32).T.astype(np.float16))
        regp = np.ascontiguousarray(
            reg_preds[b].astype(np.float32).T.astype(np.float16))
        gb = gt_boxes[b].astype(np.float32) * np.float32(CSCALE)
        gx1, gy1, gx2, gy2 = gb[:, 0], gb[:, 1], gb[:, 2], gb[:, 3]
        wg = gx2 - gx1
        hg = gy2 - gy1
        aB = wg * hg
        xg = (gx1 + gx2) * np.float32(0.5)
        yg = (gy1 + gy2) * np.float32(0.5)
        lwg = np.log(wg)
        lhg = np.log(hg)
        lab1 = gt_labels[b].astype(np.float32) + np.float32(1.0)
        gtaux = np.concatenate(
            [gx1, gy1, gx2, gy2, aB, xg, yg, lwg, lhg, lab1, -gx1]
        ).astype(np.float32)[None, :]
        maps.append({"anch": anch, "clsp": clsp, "regp": regp, "gtaux": gtaux})
    return maps


def finish(partials):
    """partials: list of [1,4] arrays per core -> (cls_loss, reg_loss)."""
    f = np.float32
    npos = f(0); sl1 = f(0); nsum = f(0); corr = f(0)
    for p in partials:
        p = p.reshape(4)
        npos += f(p[0]); sl1 += f(p[1]); nsum += f(p[2]); corr += f(p[3])
    denom = max(float(npos), 1.0)
    if npos > 0:
        cls_loss = f(0.75) * (nsum + corr) / f(denom)
        reg_loss = sl1 / f(2 * BETA) / f(denom)
    else:
        cls_loss = f(0.0); reg_loss = f(0.0)
    return np.float32(cls_loss), np.float32(reg_loss)


# ---------------- self-contained kernel entry ----------------

_CACHE = {}


def _get_fn(n_cores=8):
    if "fn" in _CACHE:
        return _CACHE["fn"]
    import jax
    from jax.sharding import Mesh, PartitionSpec, NamedSharding
    from jax.experimental.shard_map import shard_map
    from concourse.bass2jax import (_bass_exec_p, install_neuronx_cc_hook,
                                    partition_id_tensor)
    nc = build_for_timing()
    install_neuronx_cc_hook()
    in_names, out_names, out_avals, zero_shapes = [], [], [], []
    partition_name = (nc.partition_id_tensor.name
                      if nc.partition_id_tensor else None)
    for alloc in nc.m.functions[0].allocations:
        if not isinstance(alloc, mybir.MemoryLocationSet):
            continue
        name = alloc.memorylocations[0].name
        if alloc.kind == "ExternalInput":
            if name != partition_name:
                in_names.append(name)
        elif alloc.kind == "ExternalOutput":
            out_names.append(name)
            shape = tuple(alloc.tensor_shape)
            dtype = mybir.dt.np(alloc.dtype)
            out_avals.append(jax.core.ShapedArray(shape, dtype))
            zero_shapes.append((shape, dtype))
    n_params = len(in_names)
    n_outs = len(out_avals)
    all_in_names = in_names + out_names + ([partition_name]
                                           if partition_name else [])
    donate = tuple(range(n_params, n_params + n_outs))

    def _body(*args):
        operands = list(args)
        if partition_name is not None:
            operands.append(partition_id_tensor())
        outs = _bass_exec_p.bind(
            *operands, out_avals=tuple(out_avals),
            in_names=tuple(all_in_names), out_names=tuple(out_names),
            lowering_input_output_aliases=(),
            sim_require_finite=True, sim_require_nnan=True, nc=nc)
        return tuple(outs)

    devices = jax.devices()[:n_cores]
    mesh = Mesh(np.asarray(devices), ("core",))
    in_specs = (PartitionSpec("core"),) * (n_params + n_outs)
    out_specs = (PartitionSpec("core"),) * len(out_names)
    fn = jax.jit(shard_map(_body, mesh=mesh, in_specs=in_specs,
                           out_specs=out_specs, check_rep=False),
                 donate_argnums=donate, keep_unused=True)
    sh = NamedSharding(mesh, PartitionSpec("core"))
    _CACHE["fn"] = (fn, in_names, out_names, out_avals, zero_shapes, sh,
                    n_cores)
    return _CACHE["fn"]


def kernel(cls_preds, reg_preds, anchors, gt_boxes, gt_labels):
    """Full-input DetectionLoss on 8 NeuronCores (data-parallel over batch).

    Returns (cls_loss, reg_loss) as float32 scalars, matching reference()."""
    import jax
    cls_preds = np.asarray(cls_preds)
    reg_preds = np.asarray(reg_preds)
    anchors = np.asarray(anchors)
    gt_boxes = np.asarray(gt_boxes)
    gt_labels = np.asarray(gt_labels)
    B, A, _ = cls_preds.shape
    assert (B, A) == (8, 160000), (B, A)
    maps = pack_inputs(cls_preds, reg_preds, anchors, gt_boxes, gt_labels)
    fn, in_names, out_names, out_avals, zero_shapes, sh, n_cores = _get_fn()
    concat_in = [jax.device_put(
        np.concatenate([np.asarray(maps[c][nm]) for c in range(n_cores)],
                       axis=0), sh) for nm in in_names]
    zeros = [jax.device_put(
        np.zeros((n_cores * s[0], *s[1:]), d), sh) for s, d in zero_shapes]
    out_arrs = fn(*concat_in, *zeros)
    res = np.asarray(out_arrs[out_names.index("out")]).reshape(n_cores, 1, 4)
    partials = [res[c] for c in range(n_cores)]
    cls_loss, reg_loss = finish(partials)
    return cls_loss, reg_loss


# revision 8
# speedup vs baseline: 1.0579x; 1.0579x over previous
"""DetectionLoss Bass/Tile kernel for TRN2, v2 (one core = one image; SPMD x8).

fp16 data path (coords pre-scaled by 1/64 on host), per-j scalar ops from a
broadcast gt table, DVE 2x/4x perf modes, Pool runs the argmax chain,
ACT runs relu + focal activations, PE does the one-hot gather matmuls.

Per core (image b), layout: anchor a <-> (partition p = a // COLS, col a % COLS).
Inputs (per core, planar, host-packed):
  anch [4, A] f16 (x1,y1,x2,y2 scaled), clsp [8, A] f16, regp [4, A] f16,
  gtaux [1, 320] f32 = gx1 gy1 gx2 gy2 aB xg yg lwg lhg label (each [32])
Output: out [1, 4] f32 = [npos, sl1_sum, nsum, corr] partial sums; host finishes.
"""
import dataclasses
import numpy as np

import concourse.bass as bass
import concourse.mybir as mybir
from concourse import tile

AL = mybir.AluOpType
AF = mybir.ActivationFunctionType
f32 = mybir.dt.float32
f16 = mybir.dt.float16

P = 128
G = 32
C = 8
BETA = 1.0 / 9.0
THIRD = 1.0 / 3.0
LN_THIRD = float(np.log(np.float32(1.0) / np.float32(3.0)))
CSCALE = 1.0 / 64.0


def patch_tile_drain(maxw: int = 1):
    """Split the TileContext exit drain's sem waits across NOPs (walrus
    setupSyncWait rejects >1 wait on a CTRL instruction in this build)."""
    import concourse.tile as tile_mod
    from concourse.vector_clock import ScopedClock

    def _drain_and_barrier(self, tick_clock, wait_clock):
        drain_inst = self.nc.sync.drain()
        wait_clock.add_sem_waits(
            drain_inst.ins, ScopedClock({None: tick_clock.global_clock})
        )
        si = drain_inst.ins.sync_info
        waits = list(si.on_wait)
        if len(waits) > maxw:
            si.on_wait = waits[:maxw]
            rest = waits[maxw:]
            for i in range(0, len(rest), maxw):
                nop = self.nc.sync.nop(nofuse=True, hint="drain_split")
                nop.ins.sync_info = mybir.SyncInfo(
                    on_wait=rest[i:i + maxw], on_update=[]
                )
        self.nc.all_engine_barrier()
        assert self.sems is not None
        popped = self.nc._tile_sem_poison_stack.pop()
        assert popped is self._sem_poison
        self.nc.clear_and_free_semaphores(list(self.sems.allocated().values()))
        self.nc.all_engine_barrier()

    tile_mod.TileContext._drain_and_barrier = _drain_and_barrier


def split_sync_waits(nc, maxw: int = 1):
    """Walrus rejects >2 sem waits on one instruction (and >1 on CTRL-type).
    Hoist excess waits onto same-engine NOPs inserted immediately before."""
    ctr = [0]

    def mknop(engine, waits):
        ctr[0] += 1
        nop = mybir.InstNoOp(name=f"I-wsplit-{ctr[0]}", ins=[], outs=[])
        nop.engine = engine
        nop.sync_info = mybir.SyncInfo(on_wait=waits, on_update=[])
        return nop

    for blk in nc.bb_map.values():
        bb = blk.bb
        il = bb.instructions
        i = 0
        while i < len(il):
            inst = il[i]
            si = inst.sync_info
            mw = 1 if isinstance(inst, mybir.InstTensorScalarPtr) else maxw
            if si is not None and len(si.on_wait) > mw:
                waits = list(si.on_wait)
                si.on_wait = waits[:mw]
                rest = waits[mw:]
                for k in range(0, len(rest), 1):
                    il.insert(i, mknop(inst.engine, rest[k:k + 1]))
                    i += 1
            i += 1


def _expand32(ap):
    """[P, n] AP -> [P, n, 32] with step-0 inner dim (broadcast)."""
    return dataclasses.replace(ap, ap=ap.ap + [[0, G]])


def build(A: int, ohd: int = 7):
    """Emit the per-core program. A must be divisible by 128.
    ohd: number of one-hot superquads handled by DVE (rest on Pool)."""
    assert A % P == 0
    COLS = A // P                    # 1250
    NSQ = (COLS + 15) // 16          # superquads (16 cols each)
    if NSQ % 2:
        NSQ += 1                     # two equal halves
    W = NSQ * 16                     # 1280
    HS = NSQ // 2                    # sq per half
    WH = W // 2                      # 640

    nc = bass.Bass()
    anch = nc.declare_dram_parameter("anch", [4, A], f16, isOutput=False)
    clsp = nc.declare_dram_parameter("clsp", [C, A], f16, isOutput=False)
    regp = nc.declare_dram_parameter("regp", [4, A], f16, isOutput=False)
    gtaux = nc.declare_dram_parameter("gtaux", [1, 11 * G], f32, isOutput=False)
    out = nc.declare_dram_parameter("out", [1, 4], f32, isOutput=True)

    def plane(t, c):
        return t[c].rearrange("(p w) -> p w", p=P)

    def ts(eng, o, i0, s1, op0, s2=None, op1=None, acc=None):
        kw = {}
        if op1 is not None:
            kw["op1"] = op1
        if acc is not None:
            kw["accum_out"] = acc
        eng.tensor_scalar(out=o, in0=i0, scalar1=s1, scalar2=s2, op0=op0, **kw)

    with tile.TileContext(nc) as tc:
        from contextlib import ExitStack
        with ExitStack() as ctx:
            const = ctx.enter_context(tc.tile_pool(name="const", bufs=1))
            persist = ctx.enter_context(tc.tile_pool(name="persist", bufs=1))

            # ---------- constants ----------
            iotarep = const.tile([P, 512], f16, name="iotarep")
            nc.gpsimd.iota(iotarep[:], pattern=[[0, 16], [1, G]], base=1,
                           channel_multiplier=0,
                           allow_small_or_imprecise_dtypes=True)
            irow = const.tile([P, P], f32, name="irow")
            nc.gpsimd.iota(irow[:], pattern=[[1, P]], base=0,
                           channel_multiplier=0,
                           allow_small_or_imprecise_dtypes=True)
            icol = const.tile([P, 1], f32, name="icol")
            nc.gpsimd.iota(icol[:], pattern=[[0, 1]], base=0,
                           channel_multiplier=1,
                           allow_small_or_imprecise_dtypes=True)
            ident = const.tile([P, P], f16, name="ident")
            ts(nc.vector, ident[:], irow[:], icol[:], AL.is_equal)
            ones1 = const.tile([P, 1], f32, name="ones1")
            nc.gpsimd.memset(ones1[:], 1.0)
            lnb = const.tile([P, 1], f32, name="lnb")
            nc.gpsimd.memset(lnb[:], 1e-7)

            # gt broadcast [P, 320] f32
            gtb = const.tile([P, 11 * G], f32, name="gtb")
            gsrc = gtaux[:]
            gsrc_b = dataclasses.replace(gsrc, ap=[[0, P]] + gsrc.ap[1:])
            nc.sync.dma_start(gtb[:], gsrc_b)

            def gsc(k, j):
                # [P,1] f32 scalar ptr for gt field k, gt j
                return gtb[:, k * G + j:k * G + j + 1]

            # gather table tt16 [P, 20] f16, block-diag:
            # rows 32fs..+32, cols 5fs..+5 = [xg yg lwg lhg labelf]
            NF = 5
            tt16 = const.tile([P, 4 * NF], f16, name="tt16")
            nc.gpsimd.memset(tt16[:], 0.0)
            traw = const.tile([G, 10], f32, name="traw")
            gsrc2 = dataclasses.replace(gsrc, ap=[[1, G], [G, 10]])
            nc.sync.dma_start(traw[:], gsrc2)
            tblk = const.tile([G, NF], f16, name="tblk")
            nc.scalar.copy(tblk[:, 0:NF], traw[:, 5:10])
            for fs in range(4):
                nc.sync.dma_start(tt16[32 * fs:32 * fs + 32,
                                       NF * fs:NF * fs + NF], tblk[:])

            # ---------- anchors + per-anchor prep (all f16) ----------
            pA_stack = ExitStack()
            pA = pA_stack.enter_context(tc.tile_pool(name="pA", bufs=1))
            ax1 = pA.tile([P, COLS], f16, name="ax1")
            ay1 = pA.tile([P, COLS], f16, name="ay1")
            ax2 = pA.tile([P, COLS], f16, name="ax2")
            ay2 = pA.tile([P, COLS], f16, name="ay2")
            for t, c in ((ax1, 0), (ay1, 1), (ax2, 2), (ay2, 3)):
                nc.sync.dma_start(t[:], plane(anch, c))
            # cls/reg planes (f16); DMAs staggered into the j-loop
            xcp = [persist.tile([P, COLS], f16, name=f"xcp{c}") for c in range(C)]
            rpp = [persist.tile([P, COLS], f16, name=f"rpp{k}") for k in range(4)]

            area_a = pA.tile([P, COLS], f16, name="area_a")
            with tc.tile_pool(name="areap", bufs=1) as areap:
                wa0 = areap.tile([P, COLS], f16, name="wa0")
                ha0 = areap.tile([P, COLS], f16, name="ha0")
                nc.vector.tensor_tensor(out=wa0[:], in0=ax2[:], in1=ax1[:],
                                        op=AL.subtract)
                nc.gpsimd.tensor_tensor(out=ha0[:], in0=ay2[:], in1=ay1[:],
                                        op=AL.subtract)
                nc.vector.tensor_tensor(out=area_a[:], in0=wa0[:],
                                        in1=ha0[:], op=AL.mult)
            # xa/ya/iwa/iha/La/Ha are computed after the j-loop (phase B prep)
            xa = persist.tile([P, COLS], f16, name="xa")
            ya = persist.tile([P, COLS], f16, name="ya")
            iwa = persist.tile([P, COLS], f16, name="iwa")
            iha = persist.tile([P, COLS], f16, name="iha")
            La = persist.tile([P, COLS], f16, name="La")
            Ha = persist.tile([P, COLS], f16, name="Ha")

            m2 = [pA.tile([P, COLS], f16, name=f"m{i}") for i in range(2)]
            bestp = pA.tile([P, COLS], f16, name="bestp")
            nc.gpsimd.memset(m2[1][:], -60000.0)
            nc.gpsimd.memset(bestp[:], 0.0)

            # accumulators
            nposA = persist.tile([P, 1], f32, name="nposA")
            sl1A = persist.tile([P, 1], f32, name="sl1A")
            nsumA = persist.tile([P, 1], f32, name="nsumA")
            corrA = persist.tile([P, 1], f32, name="corrA")
            tacc = persist.tile([P, 1], f32, name="tacc")
            for t in (nposA, sl1A, nsumA, corrA):
                nc.vector.memset(t[:], 0.0)

            # focal result planes (retained through phase B)
            Rp = [persist.tile([P, COLS], f16, name=f"Rp{c}") for c in range(C)]

            # ---------- phase A: j-loop + interleaved focal ----------
            with tc.tile_pool(name="jt", bufs=1) as jt:
                t_ltx = [jt.tile([P, COLS], f16, name=f"ltx{i}") for i in range(2)]
                t_mnx = [jt.tile([P, COLS], f16, name=f"mnx{i}") for i in range(2)]
                t_wxr = [jt.tile([P, COLS], f16, name=f"wxr{i}") for i in range(2)]
                t_lty = [jt.tile([P, COLS], f16, name=f"lty{i}") for i in range(2)]
                t_mny = [jt.tile([P, COLS], f16, name=f"mny{i}") for i in range(2)]
                t_wyr = [jt.tile([P, COLS], f16, name=f"wyr{i}") for i in range(2)]
                t_wxp = [jt.tile([P, COLS], f16, name=f"wxp{i}") for i in range(3)]
                wyp_t = [jt.tile([P, COLS], f16, name=f"wyp{i}") for i in range(3)]
                t_li = [jt.tile([P, COLS], f16, name=f"li{i}") for i in range(3)]
                t_den = [jt.tile([P, COLS], f16, name=f"den{i}") for i in range(2)]
                t_int = [jt.tile([P, COLS], f16, name=f"int{i}") for i in range(3)]
                t_t = [jt.tile([P, COLS], f16, name=f"tt{i}") for i in range(3)]
                t_upd = [jt.tile([P, COLS], f16, name=f"upd{i}") for i in range(2)]
                # focal temps
                f_sp = [jt.tile([P, COLS], f16, name=f"fsp{i}") for i in range(2)]
                f_spn = [jt.tile([P, COLS], f16, name=f"fspn{i}") for i in range(2)]
                f_sig = [jt.tile([P, COLS], f16, name=f"fsig{i}") for i in range(2)]
                f_sgn = [jt.tile([P, COLS], f16, name=f"fsgn{i}") for i in range(2)]
                f_s2 = [jt.tile([P, COLS], f16, name=f"fs2{i}") for i in range(2)]
                f_nt = [jt.tile([P, COLS], f16, name=f"fnt{i}") for i in range(2)]
                f_sq = [jt.tile([P, COLS], f16, name=f"fsq{i}") for i in range(2)]
                nacc = [persist.tile([P, 1], f32, name=f"nacc{i}") for i in range(2)]

                def stA(j):
                    v = nc.vector
                    nc.scalar.activation(t_ltx[j % 2][:], ax1[:], AF.Relu,
                                         bias=gsc(10, j))
                    ts(v, t_mnx[j % 2][:], ax2[:], gsc(2, j), AL.min,
                       gsc(0, j), AL.subtract)
                    ts(v, t_lty[j % 2][:], ay1[:], gsc(1, j), AL.max)
                    ts(v, t_mny[j % 2][:], ay2[:], gsc(3, j), AL.min)

                def stB(j):
                    v = nc.vector
                    v.tensor_tensor(out=t_wxr[j % 2][:], in0=t_mnx[j % 2][:],
                                    in1=t_ltx[j % 2][:], op=AL.subtract)
                    v.tensor_tensor(out=t_wyr[j % 2][:], in0=t_mny[j % 2][:],
                                    in1=t_lty[j % 2][:], op=AL.subtract)
                    ts(v, wyp_t[j % 3][:], t_wyr[j % 2][:], 0.0, AL.max)

                def stC(j):
                    nc.scalar.activation(t_wxp[j % 3][:], t_wxr[j % 2][:],
                                         AF.Relu)

                def stD(j):
                    nc.vector.tensor_tensor(out=t_int[j % 3][:],
                                            in0=t_wxp[j % 3][:],
                                            in1=wyp_t[j % 3][:], op=AL.mult)

                def stE(j):
                    nc.scalar.activation(t_li[j % 3][:], t_int[j % 3][:],
                                         AF.Ln, bias=lnb[:])
                    nc.scalar.activation(t_den[j % 2][:], area_a[:], AF.Ln,
                                         bias=gsc(4, j))

                def stF(j):
                    v = nc.vector
                    tj = t_t[j % 3]
                    mprev = m2[(j + 1) % 2]
                    mcur = m2[j % 2]
                    v.tensor_tensor(out=tj[:], in0=t_li[j % 3][:],
                                    in1=t_den[j % 2][:], op=AL.subtract)
                    v.tensor_tensor(out=mcur[:], in0=mprev[:], in1=tj[:],
                                    op=AL.max)
                    nc.gpsimd.tensor_tensor(out=t_upd[j % 2][:], in0=tj[:],
                                            in1=mcur[:], op=AL.subtract)

                def stG(j):
                    v = nc.vector
                    upd = t_upd[j % 2]
                    ts(v, upd[:], upd[:], 0.0, AL.is_ge, float(j + 1), AL.mult)
                    v.tensor_tensor(out=bestp[:], in0=bestp[:], in1=upd[:],
                                    op=AL.max)

                stages = [stA, stB, stC, stD, stE, stF, stG]

                # focal for class c, split into 6 emission slices.
                # exp/ln formulation (single ACT table set):
                #   E = e^-x, u = 1+E, spn = ln(u) = softplus(-x),
                #   sp = x + spn = softplus(x), sgn = E/u = sigmoid(-x),
                #   sig = 1-sgn, N = sig^2*sp, P = sgn^2*spn, Rp = P/3 - N
                def focal_slice(c, s):
                    if c >= C:
                        return
                    v = nc.vector
                    xc = xcp[c]
                    E = f_sig[c % 2]; u = f_sp[c % 2]; spn = f_spn[c % 2]
                    sgn = f_sgn[c % 2]; s2n = f_s2[c % 2]; nt = f_nt[c % 2]
                    sp = u      # overwrites u after spn is computed
                    sig = E     # overwrites E (E dead after u)
                    s2 = f_sq[c % 2]
                    if s == 0:
                        nc.scalar.activation(E[:], xc[:], AF.Exp, scale=-1.0)
                    elif s == 1:
                        ts(nc.gpsimd, u[:], E[:], 1.0, AL.add)
                        nc.scalar.activation(spn[:], u[:], AF.Ln)
                    elif s == 2:
                        nc.gpsimd.tensor_tensor(out=sp[:], in0=xc[:],
                                                in1=spn[:], op=AL.add)
                        # sgn = sigmoid(-x) = exp(-softplus(x))
                        nc.scalar.activation(sgn[:], sp[:], AF.Exp, scale=-1.0)
                    elif s == 3:
                        nc.scalar.activation(s2n[:], sgn[:], AF.Square)
                        ts(v, sig[:], sgn[:], -1.0, AL.mult, 1.0, AL.add)
                    elif s == 4:
                        nc.scalar.activation(s2[:], sig[:], AF.Square)
                        v.scalar_tensor_tensor(
                            out=nt[:], in0=s2[:], scalar=0.0, in1=sp[:],
                            op0=AL.add, op1=AL.mult,
                            accum_out=nacc[c % 2][:])
                        nc.vector.tensor_tensor(out=nsumA[:], in0=nsumA[:],
                                                in1=nacc[c % 2][:], op=AL.add)
                    else:
                        nc.gpsimd.tensor_tensor(out=s2n[:], in0=s2n[:],
                                                in1=spn[:], op=AL.mult)
                        ts(nc.gpsimd, s2n[:], s2n[:], 1.0 / 3.0, AL.mult)
                        nc.gpsimd.tensor_tensor(out=Rp[c][:], in0=s2n[:],
                                                in1=nt[:], op=AL.subtract)

                # class c slices at j = 4c .. 4c+5 (overlap ok: c%2 buffers)
                sched = {}
                for c in range(C):
                    for s in range(6):
                        sched.setdefault(4 * c + s, []).append((c, s))
                NS = len(stages)
                for k in range(G + NS - 1):
                    if k % 4 == 0 and k // 4 < C:
                        nc.sync.dma_start(xcp[k // 4][:], plane(clsp, k // 4))
                    if k >= 24 and k % 2 == 0 and (k - 24) // 2 < 4:
                        k4 = (k - 24) // 2
                        nc.sync.dma_start(rpp[k4][:], plane(regp, k4))
                    for si, st in enumerate(stages):
                        j = k - si
                        if 0 <= j < G:
                            st(j)
                    for (c, s) in sched.get(k, []):
                        focal_slice(c, s)

            # ---------- pos, bpm, deferred anchor prep ----------
            pos = persist.tile([P, COLS], f16, name="pos")
            ts(nc.vector, pos[:], m2[1][:], LN_THIRD, AL.is_ge, None, AL.add,
               acc=tacc[:])
            nc.vector.tensor_tensor(out=nposA[:], in0=nposA[:], in1=tacc[:],
                                    op=AL.add)
            bpm = persist.tile([P, W], f16, name="bpm")
            nc.gpsimd.memset(bpm[:], 0.0)
            nc.vector.tensor_tensor(out=bpm[:, 0:COLS], in0=pos[:],
                                    in1=bestp[:], op=AL.mult)
            # xa/ya/iwa/iha/La/Ha (anchors still alive)
            with tc.tile_pool(name="prepp", bufs=1) as prepp:
                wa = prepp.tile([P, COLS], f16, name="wa")
                ha = prepp.tile([P, COLS], f16, name="ha")
                nc.vector.tensor_tensor(out=wa[:], in0=ax2[:], in1=ax1[:],
                                        op=AL.subtract)
                nc.gpsimd.tensor_tensor(out=ha[:], in0=ay2[:], in1=ay1[:],
                                        op=AL.subtract)
                nc.gpsimd.tensor_tensor(out=xa[:], in0=ax1[:], in1=ax2[:],
                                        op=AL.add)
                ts(nc.gpsimd, xa[:], xa[:], 0.5, AL.mult)
                nc.vector.tensor_tensor(out=ya[:], in0=ay1[:], in1=ay2[:],
                                        op=AL.add)
                ts(nc.vector, ya[:], ya[:], 0.5, AL.mult)
                with nc.allow_low_precision(reason="f16 reg-target recips"):
                    nc.vector.reciprocal(iwa[:], wa[:])
                    nc.vector.reciprocal(iha[:], ha[:])
                nc.scalar.activation(La[:], wa[:], AF.Ln)
                nc.scalar.activation(Ha[:], ha[:], AF.Ln)
            pA_stack.close()

            # ---------- phase B ----------
            with ExitStack() as bctx:
                ohp = bctx.enter_context(tc.tile_pool(name="ohp", bufs=4))
                psum_t = bctx.enter_context(
                    tc.tile_pool(name="psum_t", bufs=3, space="PSUM"))
                psum_g = bctx.enter_context(
                    tc.tile_pool(name="psum_g", bufs=3, space="PSUM"))
                gath_p = bctx.enter_context(tc.tile_pool(name="gath", bufs=2))
                scr = bctx.enter_context(tc.tile_pool(name="scr", bufs=1))

                sc = [scr.tile([P, WH], f16, name=f"sc{i}") for i in range(8)]
                accp = [persist.tile([P, 1], f32, name=f"accp{i}")
                        for i in range(2)]

                pending = []

                def emit_tail(gath, base, rw):
                    tail = []

                    def gplq_f(mm, q0, q1):
                        return gath[:, mm * WH + q0:mm * WH + q1]

                    for q0, q1 in ((0, rw // 2), (rw // 2, rw)):
                        qw = q1 - q0

                        def mk_reg(k, ctr_t, inv_t, lg_t, q0=q0, q1=q1, qw=qw):
                            def go():
                                posh = pos[:, base + q0:base + q1]
                                s1, s2_, s3, s4 = sc[4 * (k % 2):4 * (k % 2) + 4]
                                rt = s1
                                if lg_t is None:
                                    nc.vector.tensor_tensor(
                                        out=s2_[:, :qw],
                                        in0=gplq_f(k, q0, q1),
                                        in1=ctr_t[:, base + q0:base + q1],
                                        op=AL.subtract)
                                    nc.vector.tensor_tensor(
                                        out=rt[:, :qw], in0=s2_[:, :qw],
                                        in1=inv_t[:, base + q0:base + q1],
                                        op=AL.mult)
                                else:
                                    nc.vector.tensor_tensor(
                                        out=rt[:, :qw], in0=gplq_f(k, q0, q1),
                                        in1=lg_t[:, base + q0:base + q1],
                                        op=AL.subtract)
                                e = s2_
                                nc.vector.tensor_tensor(
                                    out=e[:, :qw],
                                    in0=rpp[k][:, base + q0:base + q1],
                                    in1=rt[:, :qw], op=AL.subtract)
                                q = s3
                                nc.scalar.activation(q[:, :qw], e[:, :qw],
                                                     AF.Abs)
                                qm = s4
                                nc.vector.tensor_tensor(out=qm[:, :qw],
                                                        in0=q[:, :qw],
                                                        in1=posh, op=AL.mult)
                                cm = s1
                                ts(nc.vector, cm[:, :qw], qm[:, :qw], BETA,
                                   AL.min)
                                q2 = s3
                                nc.vector.tensor_tensor(
                                    out=q2[:, :qw], in0=qm[:, :qw],
                                    in1=qm[:, :qw], op=AL.add)
                                nc.vector.tensor_tensor(
                                    out=q2[:, :qw], in0=q2[:, :qw],
                                    in1=cm[:, :qw], op=AL.subtract)
                                nc.vector.scalar_tensor_tensor(
                                    out=s4[:, :qw], in0=cm[:, :qw], scalar=0.0,
                                    in1=q2[:, :qw], op0=AL.add, op1=AL.mult,
                                    accum_out=accp[k % 2][:])
                                nc.vector.tensor_tensor(
                                    out=sl1A[:], in0=sl1A[:],
                                    in1=accp[k % 2][:], op=AL.add)
                            return go

                        for k, (ctr_t, inv_t, lg_t) in enumerate(
                                ((xa, iwa, None), (ya, iha, None),
                                 (None, None, La), (None, None, Ha))):
                            tail.append(mk_reg(k, ctr_t, inv_t, lg_t))

                        def mk_corr(c, q0=q0, q1=q1, qw=qw):
                            def go():
                                eqc = sc[4 + (c % 2)]
                                # table holds label+1: background matches none
                                ts(nc.vector, eqc[:, :qw], gplq_f(4, q0, q1),
                                   float(c + 1), AL.is_equal)
                                cc = sc[6 + (c % 2)]
                                nc.vector.scalar_tensor_tensor(
                                    out=cc[:, :qw], in0=eqc[:, :qw],
                                    scalar=0.0,
                                    in1=Rp[c][:, base + q0:base + q1],
                                    op0=AL.add, op1=AL.mult,
                                    accum_out=accp[c % 2][:])
                                nc.vector.tensor_tensor(
                                    out=corrA[:], in0=corrA[:],
                                    in1=accp[c % 2][:], op=AL.add)
                            return go

                        for c in range(C):
                            tail.append(mk_corr(c))
                    return tail

                for half in range(2):
                    base = half * WH
                    rw = min(COLS - base, WH)
                    if rw <= 0:
                        break
                    gath = gath_p.tile([P, NF * WH], f16, name="gath")

                    for s in range(HS):
                        sq = half * HS + s
                        oh = ohp.tile([P, 512], f16, name="oh")
                        srcx = _expand32(bpm[:, 16 * sq:16 * sq + 16])
                        if (sq % 10) < ohd:
                            nc.vector.tensor_tensor(
                                out=oh[:].rearrange("p (f j) -> p f j", j=G),
                                in0=srcx,
                                in1=iotarep[:].rearrange("p (f j) -> p f j",
                                                         j=G),
                                op=AL.is_equal)
                        else:
                            # Pool: e = bpm - iota; DVE: oh = (e == 0)
                            nc.gpsimd.tensor_tensor(
                                out=oh[:].rearrange("p (f j) -> p f j", j=G),
                                in0=srcx,
                                in1=iotarep[:].rearrange("p (f j) -> p f j",
                                                         j=G),
                                op=AL.subtract)
                            ts(nc.vector, oh[:], oh[:], 0.0, AL.is_equal)
                        pt = psum_t.tile([P, 512], f16, name="pt")
                        for t4 in range(4):
                            nc.tensor.transpose(pt[:, 128 * t4:128 * t4 + 128],
                                                oh[:, 128 * t4:128 * t4 + 128],
                                                ident[:])
                        ohT = ohp.tile([P, 512], f16, name="ohT")
                        if s % 4 == 0:
                            nc.vector.tensor_copy(ohT[:], pt[:])
                        else:
                            nc.scalar.copy(ohT[:], pt[:])
                        gp = psum_g.tile([P, 4 * NF * 4], f32, name="gp")
                        for t4 in range(4):
                            nc.tensor.matmul(
                                out=gp[:, 4 * NF * t4:4 * NF * t4 + 4 * NF],
                                lhsT=ohT[:, 128 * t4:128 * t4 + 128],
                                rhs=tt16[:], start=True, stop=True)
                        src_g = gp[:].rearrange("p (t f mm) -> p t f mm",
                                                t=4, f=4)
                        dst = gath[:]
                        dst_ap = dataclasses.replace(
                            dst, offset=dst.offset + 16 * s,
                            ap=[dst.ap[0], [4, 4], [1, 4], [WH, NF]])
                        if s % 8 < 3:
                            nc.vector.tensor_copy(dst_ap, src_g)
                        else:
                            nc.scalar.copy(dst_ap, src_g)
                        # interleave previous half's reg/corr work
                        if pending:
                            pending.pop(0)()

                    tail = emit_tail(gath, base, rw)
                    if half == 0:
                        pending = tail
                    else:
                        for go in pending:
                            go()
                        pending = []
                        for go in tail:
                            go()
                for go in pending:
                    go()

            # ---------- final cross-partition reduce ----------
            acc4 = persist.tile([P, 4], f32, name="acc4")
            nc.scalar.copy(acc4[:, 0:1], nposA[:])
            nc.scalar.copy(acc4[:, 1:2], sl1A[:])
            nc.scalar.copy(acc4[:, 2:3], nsumA[:])
            nc.scalar.copy(acc4[:, 3:4], corrA[:])
            with tc.tile_pool(name="psum_f", bufs=1, space="PSUM") as pf:
                fps = pf.tile([1, 4], f32, name="fps")
                nc.tensor.matmul(out=fps[:], lhsT=ones1[:], rhs=acc4[:],
                                 start=True, stop=True)
                osb = persist.tile([1, 4], f32, name="osb")
                nc.scalar.copy(osb[:], fps[:])
                nc.sync.dma_start(out[:], osb[:])

    return nc


def build_for_timing():
    patch_tile_drain(1)
    nc = build(160000)
    split_sync_waits(nc)
    return nc


# ---------------- host side ----------------

def pack_inputs(cls_preds, reg_preds, anchors, gt_boxes, gt_labels):
    """Full inputs -> list of 8 per-core input maps (planar f16 layouts)."""
    B, A, _ = cls_preds.shape
    anch = np.ascontiguousarray(
        (anchors.astype(np.float32).T * np.float32(CSCALE)).astype(np.float16))
    maps = []
    for b in range(B):
        clsp = np.ascontiguousarray(
            cls_preds[b].astype(np.float32).T.astype(np.float16))
        regp = np.ascontiguousarray(
            reg_preds[b].astype(np.float32).T.astype(np.float16))
        gb = gt_boxes[b].astype(np.float32) * np.float32(CSCALE)
        gx1, gy1, gx2, gy2 = gb[:, 0], gb[:, 1], gb[:, 2], gb[:, 3]
        wg = gx2 - gx1
        hg = gy2 - gy1
        aB = wg * hg
        xg = (gx1 + gx2) * np.float32(0.5)
        yg = (gy1 + gy2) * np.float32(0.5)
        lwg = np.log(wg)
        lhg = np.log(hg)
        lab1 = gt_labels[b].astype(np.float32) + np.float32(1.0)
        gtaux = np.concatenate(
            [gx1, gy1, gx2, gy2, aB, xg, yg, lwg, lhg, lab1, -gx1]
        ).astype(np.float32)[None, :]
        maps.append({"anch": anch, "clsp": clsp, "regp": regp, "gtaux": gtaux})
    return maps


def finish(partials):
    """partials: list of [1,4] arrays per core -> (cls_loss, reg_loss)."""
    f = np.float32
    npos = f(0); sl1 = f(0); nsum = f(0); corr = f(0)
    for p in partials:
        p = p.reshape(4)
        npos += f(p[0]); sl1 += f(p[1]); nsum += f(p[2]); corr += f(p[3])
    denom = max(float(npos), 1.0)
    if npos > 0:
        cls_loss = f(0.75) * (nsum + corr) / f(denom)
        reg_loss = sl1 / f(2 * BETA) / f(denom)
    else:
        cls_loss = f(0.0); reg_loss = f(0.0)
    return np.float32(cls_loss), np.float32(reg_loss)


# ---------------- self-contained kernel entry ----------------

_CACHE = {}


def _get_fn(n_cores=8):
    if "fn" in _CACHE:
        return _CACHE["fn"]
    import jax
    from jax.sharding import Mesh, PartitionSpec, NamedSharding
    from jax.experimental.shard_map import shard_map
    from concourse.bass2jax import (_bass_exec_p, install_neuronx_cc_hook,
                                    partition_id_tensor)
    nc = build_for_timing()
    install_neuronx_cc_hook()
    in_names, out_names, out_avals, zero_shapes = [], [], [], []
    partition_name = (nc.partition_id_tensor.name
                      if nc.partition_id_tensor else None)
    for alloc in nc.m.functions[0].allocations:
        if not isinstance(alloc, mybir.MemoryLocationSet):
            continue
        name = alloc.memorylocations[0].name
        if alloc.kind == "ExternalInput":
            if name != partition_name:
                in_names.append(name)
        elif alloc.kind == "ExternalOutput":
            out_names.append(name)
            shape = tuple(alloc.tensor_shape)
            dtype = mybir.dt.np(alloc.dtype)
            out_avals.append(jax.core.ShapedArray(shape, dtype))
            zero_shapes.append((shape, dtype))
    n_params = len(in_names)
    n_outs = len(out_avals)
    all_in_names = in_names + out_names + ([partition_name]
                                           if partition_name else [])
    donate = tuple(range(n_params, n_params + n_outs))

    def _body(*args):
        operands = list(args)
        if partition_name is not None:
            operands.append(partition_id_tensor())
        outs = _bass_exec_p.bind(
            *operands, out_avals=tuple(out_avals),
            in_names=tuple(all_in_names), out_names=tuple(out_names),
            lowering_input_output_aliases=(),
            sim_require_finite=True, sim_require_nnan=True, nc=nc)
        return tuple(outs)

    devices = jax.devices()[:n_cores]
    mesh = Mesh(np.asarray(devices), ("core",))
    in_specs = (PartitionSpec("core"),) * (n_params + n_outs)
    out_specs = (PartitionSpec("core"),) * len(out_names)
    fn = jax.jit(shard_map(_body, mesh=mesh, in_specs=in_specs,
                           out_specs=out_specs, check_rep=False),
                 donate_argnums=donate, keep_unused=True)
    sh = NamedSharding(mesh, PartitionSpec("core"))
    _CACHE["fn"] = (fn, in_names, out_names, out_avals, zero_shapes, sh,
                    n_cores)
    return _CACHE["fn"]


def kernel(cls_preds, reg_preds, anchors, gt_boxes, gt_labels):
    """Full-input DetectionLoss on 8 NeuronCores (data-parallel over batch).

    Returns (cls_loss, reg_loss) as float32 scalars, matching reference()."""
    import jax
    cls_preds = np.asarray(cls_preds)
    reg_preds = np.asarray(reg_preds)
    anchors = np.asarray(anchors)
    gt_boxes = np.asarray(gt_boxes)
    gt_labels = np.asarray(gt_labels)
    B, A, _ = cls_preds.shape
    assert (B, A) == (8, 160000), (B, A)
    maps = pack_inputs(cls_preds, reg_preds, anchors, gt_boxes, gt_labels)
    fn, in_names, out_names, out_avals, zero_shapes, sh, n_cores = _get_fn()
    concat_in = [jax.device_put(
        np.concatenate([np.asarray(maps[c][nm]) for c in range(n_cores)],
                       axis=0), sh) for nm in in_names]
    zeros = [jax.device_put(
        np.zeros((n_cores * s[0], *s[1:]), d), sh) for s, d in zero_shapes]
    out_arrs = fn(*concat_in, *zeros)
    res = np.asarray(out_arrs[out_names.index("out")]).reshape(n_cores, 1, 4)
    partials = [res[c] for c in range(n_cores)]
    cls_loss, reg_loss = finish(partials)
    return cls_loss, reg_loss


# revision 9
# speedup vs baseline: 1.0597x; 1.0017x over previous
"""DetectionLoss Bass/Tile kernel for TRN2, v2 (one core = one image; SPMD x8).

fp16 data path (coords pre-scaled by 1/64 on host), per-j scalar ops from a
broadcast gt table, DVE 2x/4x perf modes, Pool runs the argmax chain,
ACT runs relu + focal activations, PE does the one-hot gather matmuls.

Per core (image b), layout: anchor a <-> (partition p = a // COLS, col a % COLS).
Inputs (per core, planar, host-packed):
  anch [4, A] f16 (x1,y1,x2,y2 scaled), clsp [8, A] f16, regp [4, A] f16,
  gtaux [1, 320] f32 = gx1 gy1 gx2 gy2 aB xg yg lwg lhg label (each [32])
Output: out [1, 4] f32 = [npos, sl1_sum, nsum, corr] partial sums; host finishes.
"""
import dataclasses
import numpy as np

import concourse.bass as bass
import concourse.mybir as mybir
from concourse import tile

AL = mybir.AluOpType
AF = mybir.ActivationFunctionType
f32 = mybir.dt.float32
f16 = mybir.dt.float16

P = 128
G = 32
C = 8
BETA = 1.0 / 9.0
THIRD = 1.0 / 3.0
LN_THIRD = float(np.log(np.float32(1.0) / np.float32(3.0)))
CSCALE = 1.0 / 64.0


def patch_tile_drain(maxw: int = 1):
    """Split the TileContext exit drain's sem waits across NOPs (walrus
    setupSyncWait rejects >1 wait on a CTRL instruction in this build)."""
    import concourse.tile as tile_mod
    from concourse.vector_clock import ScopedClock

    def _drain_and_barrier(self, tick_clock, wait_clock):
        drain_inst = self.nc.sync.drain()
        wait_clock.add_sem_waits(
            drain_inst.ins, ScopedClock({None: tick_clock.global_clock})
        )
        si = drain_inst.ins.sync_info
        waits = list(si.on_wait)
        if len(waits) > maxw:
            si.on_wait = waits[:maxw]
            rest = waits[maxw:]
            for i in range(0, len(rest), maxw):
                nop = self.nc.sync.nop(nofuse=True, hint="drain_split")
                nop.ins.sync_info = mybir.SyncInfo(
                    on_wait=rest[i:i + maxw], on_update=[]
                )
        self.nc.all_engine_barrier()
        assert self.sems is not None
        popped = self.nc._tile_sem_poison_stack.pop()
        assert popped is self._sem_poison
        self.nc.clear_and_free_semaphores(list(self.sems.allocated().values()))
        self.nc.all_engine_barrier()

    tile_mod.TileContext._drain_and_barrier = _drain_and_barrier


def split_sync_waits(nc, maxw: int = 1):
    """Walrus rejects >2 sem waits on one instruction (and >1 on CTRL-type).
    Hoist excess waits onto same-engine NOPs inserted immediately before."""
    ctr = [0]

    def mknop(engine, waits):
        ctr[0] += 1
        nop = mybir.InstNoOp(name=f"I-wsplit-{ctr[0]}", ins=[], outs=[])
        nop.engine = engine
        nop.sync_info = mybir.SyncInfo(on_wait=waits, on_update=[])
        return nop

    for blk in nc.bb_map.values():
        bb = blk.bb
        il = bb.instructions
        i = 0
        while i < len(il):
            inst = il[i]
            si = inst.sync_info
            mw = 1 if isinstance(inst, mybir.InstTensorScalarPtr) else maxw
            if si is not None and len(si.on_wait) > mw:
                waits = list(si.on_wait)
                si.on_wait = waits[:mw]
                rest = waits[mw:]
                for k in range(0, len(rest), 1):
                    il.insert(i, mknop(inst.engine, rest[k:k + 1]))
                    i += 1
            i += 1


def _expand32(ap):
    """[P, n] AP -> [P, n, 32] with step-0 inner dim (broadcast)."""
    return dataclasses.replace(ap, ap=ap.ap + [[0, G]])


def build(A: int, ohd: int = 7):
    """Emit the per-core program. A must be divisible by 128.
    ohd: number of one-hot superquads handled by DVE (rest on Pool)."""
    assert A % P == 0
    COLS = A // P                    # 1250
    NSQ = (COLS + 15) // 16          # superquads (16 cols each)
    if NSQ % 2:
        NSQ += 1                     # two equal halves
    W = NSQ * 16                     # 1280
    HS = NSQ // 2                    # sq per half
    WH = W // 2                      # 640

    nc = bass.Bass()
    anch = nc.declare_dram_parameter("anch", [4, A], f16, isOutput=False)
    clsp = nc.declare_dram_parameter("clsp", [C, A], f16, isOutput=False)
    regp = nc.declare_dram_parameter("regp", [4, A], f16, isOutput=False)
    gtaux = nc.declare_dram_parameter("gtaux", [1, 11 * G], f32, isOutput=False)
    out = nc.declare_dram_parameter("out", [1, 4], f32, isOutput=True)

    def plane(t, c):
        return t[c].rearrange("(p w) -> p w", p=P)

    def ts(eng, o, i0, s1, op0, s2=None, op1=None, acc=None):
        kw = {}
        if op1 is not None:
            kw["op1"] = op1
        if acc is not None:
            kw["accum_out"] = acc
        eng.tensor_scalar(out=o, in0=i0, scalar1=s1, scalar2=s2, op0=op0, **kw)

    with tile.TileContext(nc) as tc:
        from contextlib import ExitStack
        with ExitStack() as ctx:
            const = ctx.enter_context(tc.tile_pool(name="const", bufs=1))
            persist = ctx.enter_context(tc.tile_pool(name="persist", bufs=1))

            # ---------- constants ----------
            iotarep = const.tile([P, 512], f16, name="iotarep")
            nc.gpsimd.iota(iotarep[:], pattern=[[0, 16], [1, G]], base=1,
                           channel_multiplier=0,
                           allow_small_or_imprecise_dtypes=True)
            irow = const.tile([P, P], f32, name="irow")
            nc.gpsimd.iota(irow[:], pattern=[[1, P]], base=0,
                           channel_multiplier=0,
                           allow_small_or_imprecise_dtypes=True)
            icol = const.tile([P, 1], f32, name="icol")
            nc.gpsimd.iota(icol[:], pattern=[[0, 1]], base=0,
                           channel_multiplier=1,
                           allow_small_or_imprecise_dtypes=True)
            ident = const.tile([P, P], f16, name="ident")
            ts(nc.vector, ident[:], irow[:], icol[:], AL.is_equal)
            ones1 = const.tile([P, 1], f32, name="ones1")
            nc.gpsimd.memset(ones1[:], 1.0)
            lnb = const.tile([P, 1], f32, name="lnb")
            nc.gpsimd.memset(lnb[:], 1e-7)

            # gt broadcast [P, 320] f32
            gtb = const.tile([P, 11 * G], f32, name="gtb")
            gsrc = gtaux[:]
            gsrc_b = dataclasses.replace(gsrc, ap=[[0, P]] + gsrc.ap[1:])
            nc.sync.dma_start(gtb[:], gsrc_b)

            def gsc(k, j):
                # [P,1] f32 scalar ptr for gt field k, gt j
                return gtb[:, k * G + j:k * G + j + 1]

            # gather table tt16 [P, 20] f16, block-diag:
            # rows 32fs..+32, cols 5fs..+5 = [xg yg lwg lhg labelf]
            NF = 5
            tt16 = const.tile([P, 4 * NF], f16, name="tt16")
            nc.gpsimd.memset(tt16[:], 0.0)
            traw = const.tile([G, 10], f32, name="traw")
            gsrc2 = dataclasses.replace(gsrc, ap=[[1, G], [G, 10]])
            nc.sync.dma_start(traw[:], gsrc2)
            tblk = const.tile([G, NF], f16, name="tblk")
            nc.scalar.copy(tblk[:, 0:NF], traw[:, 5:10])
            for fs in range(4):
                nc.sync.dma_start(tt16[32 * fs:32 * fs + 32,
                                       NF * fs:NF * fs + NF], tblk[:])

            # ---------- anchors + per-anchor prep (all f16) ----------
            pA_stack = ExitStack()
            pA = pA_stack.enter_context(tc.tile_pool(name="pA", bufs=1))
            ax1 = pA.tile([P, COLS], f16, name="ax1")
            ay1 = pA.tile([P, COLS], f16, name="ay1")
            ax2 = pA.tile([P, COLS], f16, name="ax2")
            ay2 = pA.tile([P, COLS], f16, name="ay2")
            for t, c in ((ax1, 0), (ay1, 1), (ax2, 2), (ay2, 3)):
                nc.sync.dma_start(t[:], plane(anch, c))
            # cls/reg planes (f16); DMAs staggered into the j-loop
            xcp = [persist.tile([P, COLS], f16, name=f"xcp{c}") for c in range(C)]
            rpp = [persist.tile([P, COLS], f16, name=f"rpp{k}") for k in range(4)]

            area_a = pA.tile([P, COLS], f16, name="area_a")
            with tc.tile_pool(name="areap", bufs=1) as areap:
                wa0 = areap.tile([P, COLS], f16, name="wa0")
                ha0 = areap.tile([P, COLS], f16, name="ha0")
                nc.vector.tensor_tensor(out=wa0[:], in0=ax2[:], in1=ax1[:],
                                        op=AL.subtract)
                nc.gpsimd.tensor_tensor(out=ha0[:], in0=ay2[:], in1=ay1[:],
                                        op=AL.subtract)
                nc.vector.tensor_tensor(out=area_a[:], in0=wa0[:],
                                        in1=ha0[:], op=AL.mult)
            # xa/ya/iwa/iha/La/Ha are computed after the j-loop (phase B prep)
            xa = persist.tile([P, COLS], f16, name="xa")
            ya = persist.tile([P, COLS], f16, name="ya")
            iwa = persist.tile([P, COLS], f16, name="iwa")
            iha = persist.tile([P, COLS], f16, name="iha")
            La = persist.tile([P, COLS], f16, name="La")
            Ha = persist.tile([P, COLS], f16, name="Ha")

            m2 = [pA.tile([P, COLS], f16, name=f"m{i}") for i in range(2)]
            bestp = pA.tile([P, COLS], f16, name="bestp")
            nc.gpsimd.memset(m2[1][:], -60000.0)
            nc.gpsimd.memset(bestp[:], 0.0)

            # accumulators
            nposA = persist.tile([P, 1], f32, name="nposA")
            sl1A = persist.tile([P, 1], f32, name="sl1A")
            nsumA = persist.tile([P, 1], f32, name="nsumA")
            corrA = persist.tile([P, 1], f32, name="corrA")
            tacc = persist.tile([P, 1], f32, name="tacc")
            for t in (nposA, sl1A, nsumA, corrA):
                nc.vector.memset(t[:], 0.0)

            # focal result planes (retained through phase B)
            Rp = [persist.tile([P, COLS], f16, name=f"Rp{c}") for c in range(C)]

            # ---------- phase A: j-loop + interleaved focal ----------
            with tc.tile_pool(name="jt", bufs=1) as jt:
                t_ltx = [jt.tile([P, COLS], f16, name=f"ltx{i}") for i in range(2)]
                t_mnx = [jt.tile([P, COLS], f16, name=f"mnx{i}") for i in range(2)]
                t_wxr = [jt.tile([P, COLS], f16, name=f"wxr{i}") for i in range(2)]
                t_lty = [jt.tile([P, COLS], f16, name=f"lty{i}") for i in range(2)]
                t_mny = [jt.tile([P, COLS], f16, name=f"mny{i}") for i in range(2)]
                t_wyr = [jt.tile([P, COLS], f16, name=f"wyr{i}") for i in range(2)]
                t_wxp = [jt.tile([P, COLS], f16, name=f"wxp{i}") for i in range(3)]
                wyp_t = [jt.tile([P, COLS], f16, name=f"wyp{i}") for i in range(3)]
                t_li = [jt.tile([P, COLS], f16, name=f"li{i}") for i in range(3)]
                t_den = [jt.tile([P, COLS], f16, name=f"den{i}") for i in range(2)]
                t_int = [jt.tile([P, COLS], f16, name=f"int{i}") for i in range(3)]
                t_t = [jt.tile([P, COLS], f16, name=f"tt{i}") for i in range(3)]
                t_upd = [jt.tile([P, COLS], f16, name=f"upd{i}") for i in range(2)]
                # focal temps
                f_sp = [jt.tile([P, COLS], f16, name=f"fsp{i}") for i in range(2)]
                f_spn = [jt.tile([P, COLS], f16, name=f"fspn{i}") for i in range(2)]
                f_sig = [jt.tile([P, COLS], f16, name=f"fsig{i}") for i in range(2)]
                f_sgn = [jt.tile([P, COLS], f16, name=f"fsgn{i}") for i in range(2)]
                f_s2 = [jt.tile([P, COLS], f16, name=f"fs2{i}") for i in range(2)]
                f_nt = [jt.tile([P, COLS], f16, name=f"fnt{i}") for i in range(2)]
                f_sq = [jt.tile([P, COLS], f16, name=f"fsq{i}") for i in range(2)]
                nacc = [persist.tile([P, 1], f32, name=f"nacc{i}") for i in range(2)]

                def stA(j):
                    v = nc.vector
                    nc.scalar.activation(t_ltx[j % 2][:], ax1[:], AF.Relu,
                                         bias=gsc(10, j))
                    ts(v, t_mnx[j % 2][:], ax2[:], gsc(2, j), AL.min,
                       gsc(0, j), AL.subtract)
                    ts(v, t_lty[j % 2][:], ay1[:], gsc(1, j), AL.max)
                    ts(v, t_mny[j % 2][:], ay2[:], gsc(3, j), AL.min)

                def stB(j):
                    v = nc.vector
                    v.tensor_tensor(out=t_wxr[j % 2][:], in0=t_mnx[j % 2][:],
                                    in1=t_ltx[j % 2][:], op=AL.subtract)
                    v.tensor_tensor(out=t_wyr[j % 2][:], in0=t_mny[j % 2][:],
                                    in1=t_lty[j % 2][:], op=AL.subtract)
                    ts(v, wyp_t[j % 3][:], t_wyr[j % 2][:], 0.0, AL.max)

                def stC(j):
                    nc.scalar.activation(t_wxp[j % 3][:], t_wxr[j % 2][:],
                                         AF.Relu)

                def stD(j):
                    nc.vector.tensor_tensor(out=t_int[j % 3][:],
                                            in0=t_wxp[j % 3][:],
                                            in1=wyp_t[j % 3][:], op=AL.mult)

                def stE(j):
                    nc.scalar.activation(t_li[j % 3][:], t_int[j % 3][:],
                                         AF.Ln, bias=lnb[:])
                    nc.scalar.activation(t_den[j % 2][:], area_a[:], AF.Ln,
                                         bias=gsc(4, j))

                def stF(j):
                    v = nc.vector
                    tj = t_t[j % 3]
                    mprev = m2[(j + 1) % 2]
                    mcur = m2[j % 2]
                    v.tensor_tensor(out=tj[:], in0=t_li[j % 3][:],
                                    in1=t_den[j % 2][:], op=AL.subtract)
                    v.tensor_tensor(out=mcur[:], in0=mprev[:], in1=tj[:],
                                    op=AL.max)
                    nc.gpsimd.tensor_tensor(out=t_upd[j % 2][:], in0=tj[:],
                                            in1=mcur[:], op=AL.subtract)

                def stG(j):
                    v = nc.vector
                    upd = t_upd[j % 2]
                    ts(v, upd[:], upd[:], 0.0, AL.is_ge, float(j + 1), AL.mult)
                    v.tensor_tensor(out=bestp[:], in0=bestp[:], in1=upd[:],
                                    op=AL.max)

                stages = [stA, stB, stC, stD, stE, stF, stG]

                # focal for class c, split into 6 emission slices.
                # exp/ln formulation (single ACT table set):
                #   E = e^-x, u = 1+E, spn = ln(u) = softplus(-x),
                #   sp = x + spn = softplus(x), sgn = E/u = sigmoid(-x),
                #   sig = 1-sgn, N = sig^2*sp, P = sgn^2*spn, Rp = P/3 - N
                def focal_slice(c, s):
                    if c >= C:
                        return
                    v = nc.vector
                    xc = xcp[c]
                    E = f_sig[c % 2]; u = f_sp[c % 2]; spn = f_spn[c % 2]
                    sgn = f_sgn[c % 2]; s2n = f_s2[c % 2]; nt = f_nt[c % 2]
                    sp = u      # overwrites u after spn is computed
                    sig = E     # overwrites E (E dead after u)
                    s2 = f_sq[c % 2]
                    if s == 0:
                        nc.scalar.activation(E[:], xc[:], AF.Exp, scale=-1.0)
                    elif s == 1:
                        ts(nc.gpsimd, u[:], E[:], 1.0, AL.add)
                        nc.scalar.activation(spn[:], u[:], AF.Ln)
                    elif s == 2:
                        nc.gpsimd.tensor_tensor(out=sp[:], in0=xc[:],
                                                in1=spn[:], op=AL.add)
                        # sgn = sigmoid(-x) = exp(-softplus(x))
                        nc.scalar.activation(sgn[:], sp[:], AF.Exp, scale=-1.0)
                    elif s == 3:
                        nc.scalar.activation(s2n[:], sgn[:], AF.Square)
                        ts(v, sig[:], sgn[:], -1.0, AL.mult, 1.0, AL.add)
                    elif s == 4:
                        nc.scalar.activation(s2[:], sig[:], AF.Square)
                        v.scalar_tensor_tensor(
                            out=nt[:], in0=s2[:], scalar=0.0, in1=sp[:],
                            op0=AL.add, op1=AL.mult,
                            accum_out=nacc[c % 2][:])
                        nc.vector.tensor_tensor(out=nsumA[:], in0=nsumA[:],
                                                in1=nacc[c % 2][:], op=AL.add)
                    else:
                        nc.gpsimd.tensor_tensor(out=s2n[:], in0=s2n[:],
                                                in1=spn[:], op=AL.mult)
                        ts(nc.gpsimd, s2n[:], s2n[:], 1.0 / 3.0, AL.mult)
                        nc.gpsimd.tensor_tensor(out=Rp[c][:], in0=s2n[:],
                                                in1=nt[:], op=AL.subtract)

                # class c slices at j = 4c .. 4c+5 (overlap ok: c%2 buffers)
                sched = {}
                for c in range(C):
                    for s in range(6):
                        sched.setdefault(4 * c + s, []).append((c, s))
                NS = len(stages)
                for k in range(G + NS - 1):
                    if k % 4 == 0 and k // 4 < C:
                        nc.sync.dma_start(xcp[k // 4][:], plane(clsp, k // 4))
                    if k >= 24 and k % 2 == 0 and (k - 24) // 2 < 4:
                        k4 = (k - 24) // 2
                        nc.sync.dma_start(rpp[k4][:], plane(regp, k4))
                    for si, st in enumerate(stages):
                        j = k - si
                        if 0 <= j < G:
                            st(j)
                    for (c, s) in sched.get(k, []):
                        focal_slice(c, s)

            # ---------- pos, bpm, deferred anchor prep ----------
            pos = persist.tile([P, COLS], f16, name="pos")
            ts(nc.vector, pos[:], m2[1][:], LN_THIRD, AL.is_ge, None, AL.add,
               acc=tacc[:])
            nc.vector.tensor_tensor(out=nposA[:], in0=nposA[:], in1=tacc[:],
                                    op=AL.add)
            bpm = persist.tile([P, W], f16, name="bpm")
            nc.gpsimd.memset(bpm[:], 0.0)
            nc.vector.tensor_tensor(out=bpm[:, 0:COLS], in0=pos[:],
                                    in1=bestp[:], op=AL.mult)
            # xa/ya/iwa/iha/La/Ha (anchors still alive)
            with tc.tile_pool(name="prepp", bufs=1) as prepp:
                wa = prepp.tile([P, COLS], f16, name="wa")
                ha = prepp.tile([P, COLS], f16, name="ha")
                nc.vector.tensor_tensor(out=wa[:], in0=ax2[:], in1=ax1[:],
                                        op=AL.subtract)
                nc.gpsimd.tensor_tensor(out=ha[:], in0=ay2[:], in1=ay1[:],
                                        op=AL.subtract)
                nc.gpsimd.tensor_tensor(out=xa[:], in0=ax1[:], in1=ax2[:],
                                        op=AL.add)
                ts(nc.gpsimd, xa[:], xa[:], 0.5, AL.mult)
                nc.vector.tensor_tensor(out=ya[:], in0=ay1[:], in1=ay2[:],
                                        op=AL.add)
                ts(nc.vector, ya[:], ya[:], 0.5, AL.mult)
                with nc.allow_low_precision(reason="f16 reg-target recips"):
                    nc.vector.reciprocal(iwa[:], wa[:])
                    nc.vector.reciprocal(iha[:], ha[:])
                nc.scalar.activation(La[:], wa[:], AF.Ln)
                nc.scalar.activation(Ha[:], ha[:], AF.Ln)
            pA_stack.close()

            # ---------- phase B ----------
            with ExitStack() as bctx:
                ohp = bctx.enter_context(tc.tile_pool(name="ohp", bufs=4))
                psum_t = bctx.enter_context(
                    tc.tile_pool(name="psum_t", bufs=4, space="PSUM"))
                psum_g = bctx.enter_context(
                    tc.tile_pool(name="psum_g", bufs=4, space="PSUM"))
                gath_p = bctx.enter_context(tc.tile_pool(name="gath", bufs=2))
                scr = bctx.enter_context(tc.tile_pool(name="scr", bufs=1))

                sc = [scr.tile([P, WH], f16, name=f"sc{i}") for i in range(8)]
                accp = [persist.tile([P, 1], f32, name=f"accp{i}")
                        for i in range(2)]

                pending = []

                def emit_tail(gath, base, rw):
                    tail = []
                    tail_q0 = []

                    def gplq_f(mm, q0, q1):
                        return gath[:, mm * WH + q0:mm * WH + q1]

                    for q0, q1 in ((0, rw // 2), (rw // 2, rw)):
                        qw = q1 - q0

                        def mk_reg(k, ctr_t, inv_t, lg_t, q0=q0, q1=q1, qw=qw):
                            def go():
                                posh = pos[:, base + q0:base + q1]
                                s1, s2_, s3, s4 = sc[4 * (k % 2):4 * (k % 2) + 4]
                                rt = s1
                                if lg_t is None:
                                    nc.vector.tensor_tensor(
                                        out=s2_[:, :qw],
                                        in0=gplq_f(k, q0, q1),
                                        in1=ctr_t[:, base + q0:base + q1],
                                        op=AL.subtract)
                                    nc.vector.tensor_tensor(
                                        out=rt[:, :qw], in0=s2_[:, :qw],
                                        in1=inv_t[:, base + q0:base + q1],
                                        op=AL.mult)
                                else:
                                    nc.vector.tensor_tensor(
                                        out=rt[:, :qw], in0=gplq_f(k, q0, q1),
                                        in1=lg_t[:, base + q0:base + q1],
                                        op=AL.subtract)
                                e = s2_
                                nc.vector.tensor_tensor(
                                    out=e[:, :qw],
                                    in0=rpp[k][:, base + q0:base + q1],
                                    in1=rt[:, :qw], op=AL.subtract)
                                q = s3
                                nc.scalar.activation(q[:, :qw], e[:, :qw],
                                                     AF.Abs)
                                qm = s4
                                nc.vector.tensor_tensor(out=qm[:, :qw],
                                                        in0=q[:, :qw],
                                                        in1=posh, op=AL.mult)
                                cm = s1
                                ts(nc.vector, cm[:, :qw], qm[:, :qw], BETA,
                                   AL.min)
                                q2 = s3
                                nc.vector.tensor_tensor(
                                    out=q2[:, :qw], in0=qm[:, :qw],
                                    in1=qm[:, :qw], op=AL.add)
                                nc.vector.tensor_tensor(
                                    out=q2[:, :qw], in0=q2[:, :qw],
                                    in1=cm[:, :qw], op=AL.subtract)
                                nc.vector.scalar_tensor_tensor(
                                    out=s4[:, :qw], in0=cm[:, :qw], scalar=0.0,
                                    in1=q2[:, :qw], op0=AL.add, op1=AL.mult,
                                    accum_out=accp[k % 2][:])
                                nc.vector.tensor_tensor(
                                    out=sl1A[:], in0=sl1A[:],
                                    in1=accp[k % 2][:], op=AL.add)
                            return go

                        for k, (ctr_t, inv_t, lg_t) in enumerate(
                                ((xa, iwa, None), (ya, iha, None),
                                 (None, None, La), (None, None, Ha))):
                            tail.append(mk_reg(k, ctr_t, inv_t, lg_t))

                        def mk_corr(c, q0=q0, q1=q1, qw=qw):
                            def go():
                                eqc = sc[4 + (c % 2)]
                                # table holds label+1: background matches none
                                ts(nc.vector, eqc[:, :qw], gplq_f(4, q0, q1),
                                   float(c + 1), AL.is_equal)
                                cc = sc[6 + (c % 2)]
                                nc.vector.scalar_tensor_tensor(
                                    out=cc[:, :qw], in0=eqc[:, :qw],
                                    scalar=0.0,
                                    in1=Rp[c][:, base + q0:base + q1],
                                    op0=AL.add, op1=AL.mult,
                                    accum_out=accp[c % 2][:])
                                nc.vector.tensor_tensor(
                                    out=corrA[:], in0=corrA[:],
                                    in1=accp[c % 2][:], op=AL.add)
                            return go

                        for c in range(C):
                            tail.append(mk_corr(c))
                        if q0 == 0:
                            tail_q0 = tail
                            tail = []
                    return tail_q0, tail

                for half in range(2):
                    base = half * WH
                    rw = min(COLS - base, WH)
                    if rw <= 0:
                        break
                    gath = gath_p.tile([P, NF * WH], f16, name="gath")
                    tail_q0, tail_q1 = emit_tail(gath, base, rw)

                    for s in range(HS):
                        sq = half * HS + s
                        oh = ohp.tile([P, 512], f16, name="oh")
                        srcx = _expand32(bpm[:, 16 * sq:16 * sq + 16])
                        if (sq % 10) < ohd:
                            nc.vector.tensor_tensor(
                                out=oh[:].rearrange("p (f j) -> p f j", j=G),
                                in0=srcx,
                                in1=iotarep[:].rearrange("p (f j) -> p f j",
                                                         j=G),
                                op=AL.is_equal)
                        else:
                            # Pool: e = bpm - iota; DVE: oh = (e == 0)
                            nc.gpsimd.tensor_tensor(
                                out=oh[:].rearrange("p (f j) -> p f j", j=G),
                                in0=srcx,
                                in1=iotarep[:].rearrange("p (f j) -> p f j",
                                                         j=G),
                                op=AL.subtract)
                            ts(nc.vector, oh[:], oh[:], 0.0, AL.is_equal)
                        pt = psum_t.tile([P, 512], f16, name="pt")
                        for t4 in range(4):
                            nc.tensor.transpose(pt[:, 128 * t4:128 * t4 + 128],
                                                oh[:, 128 * t4:128 * t4 + 128],
                                                ident[:])
                        ohT = ohp.tile([P, 512], f16, name="ohT")
                        if s % 4 == 0:
                            nc.vector.tensor_copy(ohT[:], pt[:])
                        else:
                            nc.scalar.copy(ohT[:], pt[:])
                        gp = psum_g.tile([P, 4 * NF * 4], f32, name="gp")
                        for t4 in range(4):
                            nc.tensor.matmul(
                                out=gp[:, 4 * NF * t4:4 * NF * t4 + 4 * NF],
                                lhsT=ohT[:, 128 * t4:128 * t4 + 128],
                                rhs=tt16[:], start=True, stop=True)
                        src_g = gp[:].rearrange("p (t f mm) -> p t f mm",
                                                t=4, f=4)
                        dst = gath[:]
                        dst_ap = dataclasses.replace(
                            dst, offset=dst.offset + 16 * s,
                            ap=[dst.ap[0], [4, 4], [1, 4], [WH, NF]])
                        if s % 8 < 3:
                            nc.vector.tensor_copy(dst_ap, src_g)
                        else:
                            nc.scalar.copy(dst_ap, src_g)
                        # interleave reg/corr work: earlier halves first,
                        # then this half's first chunk once its columns are
                        # scattered (sqs 0..19 cover chunk q0)
                        if pending:
                            pending.pop(0)()
                        elif s >= 26 and tail_q0:
                            tail_q0.pop(0)()

                    pending = pending + tail_q0 + tail_q1
                for go in pending:
                    go()

            # ---------- final cross-partition reduce ----------
            acc4 = persist.tile([P, 4], f32, name="acc4")
            nc.scalar.copy(acc4[:, 0:1], nposA[:])
            nc.scalar.copy(acc4[:, 1:2], sl1A[:])
            nc.scalar.copy(acc4[:, 2:3], nsumA[:])
            nc.scalar.copy(acc4[:, 3:4], corrA[:])
            with tc.tile_pool(name="psum_f", bufs=1, space="PSUM") as pf:
                fps = pf.tile([1, 4], f32, name="fps")
                nc.tensor.matmul(out=fps[:], lhsT=ones1[:], rhs=acc4[:],
                                 start=True, stop=True)
                osb = persist.tile([1, 4], f32, name="osb")
                nc.scalar.copy(osb[:], fps[:])
                nc.sync.dma_start(out[:], osb[:])

    return nc


def build_for_timing():
    patch_tile_drain(1)
    nc = build(160000)
    split_sync_waits(nc)
    return nc


# ---------------- host side ----------------

def pack_inputs(cls_preds, reg_preds, anchors, gt_boxes, gt_labels):
    """Full inputs -> list of 8 per-core input maps (planar f16 layouts)."""
    B, A, _ = cls_preds.shape
    anch = np.ascontiguousarray(
        (anchors.astype(np.float32).T * np.float32(CSCALE)).astype(np.float16))
    maps = []
    for b in range(B):
        clsp = np.ascontiguousarray(
            cls_preds[b].astype(np.float32).T.astype(np.float16))
        regp = np.ascontiguousarray(
            reg_preds[b].astype(np.float32).T.astype(np.float16))
        gb = gt_boxes[b].astype(np.float32) * np.float32(CSCALE)
        gx1, gy1, gx2, gy2 = gb[:, 0], gb[:, 1], gb[:, 2], gb[:, 3]
        wg = gx2 - gx1
        hg = gy2 - gy1
        aB = wg * hg
        xg = (gx1 + gx2) * np.float32(0.5)
        yg = (gy1 + gy2) * np.float32(0.5)
        lwg = np.log(wg)
        lhg = np.log(hg)
        lab1 = gt_labels[b].astype(np.float32) + np.float32(1.0)
        gtaux = np.concatenate(
            [gx1, gy1, gx2, gy2, aB, xg, yg, lwg, lhg, lab1, -gx1]
        ).astype(np.float32)[None, :]
        maps.append({"anch": anch, "clsp": clsp, "regp": regp, "gtaux": gtaux})
    return maps


def finish(partials):
    """partials: list of [1,4] arrays per core -> (cls_loss, reg_loss)."""
    f = np.float32
    npos = f(0); sl1 = f(0); nsum = f(0); corr = f(0)
    for p in partials:
        p = p.reshape(4)
        npos += f(p[0]); sl1 += f(p[1]); nsum += f(p[2]); corr += f(p[3])
    denom = max(float(npos), 1.0)
    if npos > 0:
        cls_loss = f(0.75) * (nsum + corr) / f(denom)
        reg_loss = sl1 / f(2 * BETA) / f(denom)
    else:
        cls_loss = f(0.0); reg_loss = f(0.0)
    return np.float32(cls_loss), np.float32(reg_loss)


# ---------------- self-contained kernel entry ----------------

_CACHE = {}


def _get_fn(n_cores=8):
    if "fn" in _CACHE:
        return _CACHE["fn"]
    import jax
    from jax.sharding import Mesh, PartitionSpec, NamedSharding
    from jax.experimental.shard_map import shard_map
    from concourse.bass2jax import (_bass_exec_p, install_neuronx_cc_hook,
                                    partition_id_tensor)
    nc = build_for_timing()
    install_neuronx_cc_hook()
    in_names, out_names, out_avals, zero_shapes = [], [], [], []
    partition_name = (nc.partition_id_tensor.name
                      if nc.partition_id_tensor else None)
    for alloc in nc.m.functions[0].allocations:
        if not isinstance(alloc, mybir.MemoryLocationSet):
            continue
        name = alloc.memorylocations[0].name
        if alloc.kind == "ExternalInput":
            if name != partition_name:
                in_names.append(name)
        elif alloc.kind == "ExternalOutput":
            out_names.append(name)
            shape = tuple(alloc.tensor_shape)
            dtype = mybir.dt.np(alloc.dtype)
            out_avals.append(jax.core.ShapedArray(shape, dtype))
            zero_shapes.append((shape, dtype))
    n_params = len(in_names)
    n_outs = len(out_avals)
    all_in_names = in_names + out_names + ([partition_name]
                                           if partition_name else [])
    donate = tuple(range(n_params, n_params + n_outs))

    def _body(*args):
        operands = list(args)
        if partition_name is not None:
            operands.append(partition_id_tensor())
        outs = _bass_exec_p.bind(
            *operands, out_avals=tuple(out_avals),
            in_names=tuple(all_in_names), out_names=tuple(out_names),
            lowering_input_output_aliases=(),
            sim_require_finite=True, sim_require_nnan=True, nc=nc)
        return tuple(outs)

    devices = jax.devices()[:n_cores]
    mesh = Mesh(np.asarray(devices), ("core",))
    in_specs = (PartitionSpec("core"),) * (n_params + n_outs)
    out_specs = (PartitionSpec("core"),) * len(out_names)
    fn = jax.jit(shard_map(_body, mesh=mesh, in_specs=in_specs,
                           out_specs=out_specs, check_rep=False),
                 donate_argnums=donate, keep_unused=True)
    sh = NamedSharding(mesh, PartitionSpec("core"))
    _CACHE["fn"] = (fn, in_names, out_names, out_avals, zero_shapes, sh,
                    n_cores)
    return _CACHE["fn"]


def kernel(cls_preds, reg_preds, anchors, gt_boxes, gt_labels):
    """Full-input DetectionLoss on 8 NeuronCores (data-parallel over batch).

    Returns (cls_loss, reg_loss) as float32 scalars, matching reference()."""
    import jax
    cls_preds = np.asarray(cls_preds)
    reg_preds = np.asarray(reg_preds)
    anchors = np.asarray(anchors)
    gt_boxes = np.asarray(gt_boxes)
    gt_labels = np.asarray(gt_labels)
    B, A, _ = cls_preds.shape
    assert (B, A) == (8, 160000), (B, A)
    maps = pack_inputs(cls_preds, reg_preds, anchors, gt_boxes, gt_labels)
    fn, in_names, out_names, out_avals, zero_shapes, sh, n_cores = _get_fn()
    concat_in = [jax.device_put(
        np.concatenate([np.asarray(maps[c][nm]) for c in range(n_cores)],
                       axis=0), sh) for nm in in_names]
    zeros = [jax.device_put(
        np.zeros((n_cores * s[0], *s[1:]), d), sh) for s, d in zero_shapes]
    out_arrs = fn(*concat_in, *zeros)
    res = np.asarray(out_arrs[out_names.index("out")]).reshape(n_cores, 1, 4)
    partials = [res[c] for c in range(n_cores)]
    cls_loss, reg_loss = finish(partials)
    return cls_loss, reg_loss


# revision 10
# speedup vs baseline: 1.0717x; 1.0113x over previous
"""DetectionLoss Bass/Tile kernel for TRN2, v2 (one core = one image; SPMD x8).

fp16 data path (coords pre-scaled by 1/64 on host), per-j scalar ops from a
broadcast gt table, DVE 2x/4x perf modes, Pool runs the argmax chain,
ACT runs relu + focal activations, PE does the one-hot gather matmuls.

Per core (image b), layout: anchor a <-> (partition p = a // COLS, col a % COLS).
Inputs (per core, planar, host-packed):
  anch [4, A] f16 (x1,y1,x2,y2 scaled), clsp [8, A] f16, regp [4, A] f16,
  gtaux [1, 320] f32 = gx1 gy1 gx2 gy2 aB xg yg lwg lhg label (each [32])
Output: out [1, 4] f32 = [npos, sl1_sum, nsum, corr] partial sums; host finishes.
"""
import dataclasses
import numpy as np

import concourse.bass as bass
import concourse.mybir as mybir
from concourse import tile

AL = mybir.AluOpType
AF = mybir.ActivationFunctionType
f32 = mybir.dt.float32
f16 = mybir.dt.float16

P = 128
G = 32
C = 8
BETA = 1.0 / 9.0
THIRD = 1.0 / 3.0
LN_THIRD = float(np.log(np.float32(1.0) / np.float32(3.0)))
CSCALE = 1.0 / 64.0


def patch_tile_drain(maxw: int = 1):
    """Split the TileContext exit drain's sem waits across NOPs (walrus
    setupSyncWait rejects >1 wait on a CTRL instruction in this build)."""
    import concourse.tile as tile_mod
    from concourse.vector_clock import ScopedClock

    def _drain_and_barrier(self, tick_clock, wait_clock):
        drain_inst = self.nc.sync.drain()
        wait_clock.add_sem_waits(
            drain_inst.ins, ScopedClock({None: tick_clock.global_clock})
        )
        si = drain_inst.ins.sync_info
        waits = list(si.on_wait)
        if len(waits) > maxw:
            si.on_wait = waits[:maxw]
            rest = waits[maxw:]
            for i in range(0, len(rest), maxw):
                nop = self.nc.sync.nop(nofuse=True, hint="drain_split")
                nop.ins.sync_info = mybir.SyncInfo(
                    on_wait=rest[i:i + maxw], on_update=[]
                )
        self.nc.all_engine_barrier()
        assert self.sems is not None
        popped = self.nc._tile_sem_poison_stack.pop()
        assert popped is self._sem_poison
        self.nc.clear_and_free_semaphores(list(self.sems.allocated().values()))
        self.nc.all_engine_barrier()

    tile_mod.TileContext._drain_and_barrier = _drain_and_barrier


def split_sync_waits(nc, maxw: int = 1):
    """Walrus rejects >2 sem waits on one instruction (and >1 on CTRL-type).
    Hoist excess waits onto same-engine NOPs inserted immediately before."""
    ctr = [0]

    def mknop(engine, waits):
        ctr[0] += 1
        nop = mybir.InstNoOp(name=f"I-wsplit-{ctr[0]}", ins=[], outs=[])
        nop.engine = engine
        nop.sync_info = mybir.SyncInfo(on_wait=waits, on_update=[])
        return nop

    for blk in nc.bb_map.values():
        bb = blk.bb
        il = bb.instructions
        i = 0
        while i < len(il):
            inst = il[i]
            si = inst.sync_info
            mw = 1 if isinstance(inst, mybir.InstTensorScalarPtr) else maxw
            if si is not None and len(si.on_wait) > mw:
                waits = list(si.on_wait)
                si.on_wait = waits[:mw]
                rest = waits[mw:]
                for k in range(0, len(rest), 1):
                    il.insert(i, mknop(inst.engine, rest[k:k + 1]))
                    i += 1
            i += 1


def _expand32(ap):
    """[P, n] AP -> [P, n, 32] with step-0 inner dim (broadcast)."""
    return dataclasses.replace(ap, ap=ap.ap + [[0, G]])


def build(A: int, ohd: int = 6):
    """Emit the per-core program. A must be divisible by 128.
    ohd: number of one-hot superquads handled by DVE (rest on Pool)."""
    assert A % P == 0
    COLS = A // P                    # 1250
    NSQ = (COLS + 15) // 16          # superquads (16 cols each)
    if NSQ % 2:
        NSQ += 1                     # two equal halves
    W = NSQ * 16                     # 1280
    HS = NSQ // 2                    # sq per half
    WH = W // 2                      # 640

    nc = bass.Bass()
    anch = nc.declare_dram_parameter("anch", [4, A], f16, isOutput=False)
    clsp = nc.declare_dram_parameter("clsp", [C, A], f16, isOutput=False)
    regp = nc.declare_dram_parameter("regp", [4, A], f16, isOutput=False)
    gtaux = nc.declare_dram_parameter("gtaux", [1, 11 * G], f32, isOutput=False)
    out = nc.declare_dram_parameter("out", [1, 4], f32, isOutput=True)

    def plane(t, c):
        return t[c].rearrange("(p w) -> p w", p=P)

    def ts(eng, o, i0, s1, op0, s2=None, op1=None, acc=None):
        kw = {}
        if op1 is not None:
            kw["op1"] = op1
        if acc is not None:
            kw["accum_out"] = acc
        eng.tensor_scalar(out=o, in0=i0, scalar1=s1, scalar2=s2, op0=op0, **kw)

    with tile.TileContext(nc) as tc:
        from contextlib import ExitStack
        with ExitStack() as ctx:
            const = ctx.enter_context(tc.tile_pool(name="const", bufs=1))
            persist = ctx.enter_context(tc.tile_pool(name="persist", bufs=1))

            # ---------- constants ----------
            iotarep = const.tile([P, 512], f16, name="iotarep")
            nc.gpsimd.iota(iotarep[:], pattern=[[0, 16], [1, G]], base=1,
                           channel_multiplier=0,
                           allow_small_or_imprecise_dtypes=True)
            irow = const.tile([P, P], f32, name="irow")
            nc.gpsimd.iota(irow[:], pattern=[[1, P]], base=0,
                           channel_multiplier=0,
                           allow_small_or_imprecise_dtypes=True)
            icol = const.tile([P, 1], f32, name="icol")
            nc.gpsimd.iota(icol[:], pattern=[[0, 1]], base=0,
                           channel_multiplier=1,
                           allow_small_or_imprecise_dtypes=True)
            ident = const.tile([P, P], f16, name="ident")
            ts(nc.vector, ident[:], irow[:], icol[:], AL.is_equal)
            ones1 = const.tile([P, 1], f32, name="ones1")
            nc.gpsimd.memset(ones1[:], 1.0)
            lnb = const.tile([P, 1], f32, name="lnb")
            nc.gpsimd.memset(lnb[:], 1e-7)

            # gt broadcast [P, 320] f32
            gtb = const.tile([P, 11 * G], f32, name="gtb")
            gsrc = gtaux[:]
            gsrc_b = dataclasses.replace(gsrc, ap=[[0, P]] + gsrc.ap[1:])
            nc.sync.dma_start(gtb[:], gsrc_b)

            def gsc(k, j):
                # [P,1] f32 scalar ptr for gt field k, gt j
                return gtb[:, k * G + j:k * G + j + 1]

            # gather table tt16 [P, 20] f16, block-diag:
            # rows 32fs..+32, cols 5fs..+5 = [xg yg lwg lhg labelf]
            NF = 5
            tt16 = const.tile([P, 4 * NF], f16, name="tt16")
            nc.gpsimd.memset(tt16[:], 0.0)
            traw = const.tile([G, 10], f32, name="traw")
            gsrc2 = dataclasses.replace(gsrc, ap=[[1, G], [G, 10]])
            nc.sync.dma_start(traw[:], gsrc2)
            tblk = const.tile([G, NF], f16, name="tblk")
            nc.scalar.copy(tblk[:, 0:NF], traw[:, 5:10])
            for fs in range(4):
                nc.sync.dma_start(tt16[32 * fs:32 * fs + 32,
                                       NF * fs:NF * fs + NF], tblk[:])

            # ---------- anchors + per-anchor prep (all f16) ----------
            pA_stack = ExitStack()
            pA = pA_stack.enter_context(tc.tile_pool(name="pA", bufs=1))
            ax1 = pA.tile([P, COLS], f16, name="ax1")
            ay1 = pA.tile([P, COLS], f16, name="ay1")
            ax2 = pA.tile([P, COLS], f16, name="ax2")
            ay2 = pA.tile([P, COLS], f16, name="ay2")
            for t, c in ((ax1, 0), (ay1, 1), (ax2, 2), (ay2, 3)):
                nc.sync.dma_start(t[:], plane(anch, c))
            # cls/reg planes (f16); DMAs staggered into the j-loop
            xcp = [persist.tile([P, COLS], f16, name=f"xcp{c}") for c in range(C)]
            rpp = [persist.tile([P, COLS], f16, name=f"rpp{k}") for k in range(4)]

            area_a = pA.tile([P, COLS], f16, name="area_a")
            with tc.tile_pool(name="areap", bufs=1) as areap:
                wa0 = areap.tile([P, COLS], f16, name="wa0")
                ha0 = areap.tile([P, COLS], f16, name="ha0")
                nc.vector.tensor_tensor(out=wa0[:], in0=ax2[:], in1=ax1[:],
                                        op=AL.subtract)
                nc.gpsimd.tensor_tensor(out=ha0[:], in0=ay2[:], in1=ay1[:],
                                        op=AL.subtract)
                nc.vector.tensor_tensor(out=area_a[:], in0=wa0[:],
                                        in1=ha0[:], op=AL.mult)
            # xa/ya/iwa/iha/La/Ha are computed after the j-loop (phase B prep)
            xa = persist.tile([P, COLS], f16, name="xa")
            ya = persist.tile([P, COLS], f16, name="ya")
            iwa = persist.tile([P, COLS], f16, name="iwa")
            iha = persist.tile([P, COLS], f16, name="iha")
            La = persist.tile([P, COLS], f16, name="La")
            Ha = persist.tile([P, COLS], f16, name="Ha")

            m2 = [pA.tile([P, COLS], f16, name=f"m{i}") for i in range(2)]
            bestp = pA.tile([P, COLS], f16, name="bestp")
            nc.gpsimd.memset(m2[1][:], -60000.0)
            nc.gpsimd.memset(bestp[:], 0.0)

            # accumulators
            nposA = persist.tile([P, 1], f32, name="nposA")
            sl1A = persist.tile([P, 1], f32, name="sl1A")
            nsumA = persist.tile([P, 1], f32, name="nsumA")
            corrA = persist.tile([P, 1], f32, name="corrA")
            tacc = persist.tile([P, 1], f32, name="tacc")
            for t in (nposA, sl1A, nsumA, corrA):
                nc.vector.memset(t[:], 0.0)

            # focal result planes (retained through phase B)
            Rp = [persist.tile([P, COLS], f16, name=f"Rp{c}") for c in range(C)]

            # ---------- phase A: j-loop + interleaved focal ----------
            with tc.tile_pool(name="jt", bufs=1) as jt:
                t_ltx = [jt.tile([P, COLS], f16, name=f"ltx{i}") for i in range(2)]
                t_mnx = [jt.tile([P, COLS], f16, name=f"mnx{i}") for i in range(2)]
                t_wxr = [jt.tile([P, COLS], f16, name=f"wxr{i}") for i in range(2)]
                t_lty = [jt.tile([P, COLS], f16, name=f"lty{i}") for i in range(2)]
                t_mny = [jt.tile([P, COLS], f16, name=f"mny{i}") for i in range(2)]
                t_wyr = [jt.tile([P, COLS], f16, name=f"wyr{i}") for i in range(2)]
                t_wxp = [jt.tile([P, COLS], f16, name=f"wxp{i}") for i in range(3)]
                wyp_t = [jt.tile([P, COLS], f16, name=f"wyp{i}") for i in range(3)]
                t_li = [jt.tile([P, COLS], f16, name=f"li{i}") for i in range(3)]
                t_den = [jt.tile([P, COLS], f16, name=f"den{i}") for i in range(2)]
                t_int = [jt.tile([P, COLS], f16, name=f"int{i}") for i in range(3)]
                t_t = [jt.tile([P, COLS], f16, name=f"tt{i}") for i in range(3)]
                t_upd = [jt.tile([P, COLS], f16, name=f"upd{i}") for i in range(2)]
                # focal temps
                f_sp = [jt.tile([P, COLS], f16, name=f"fsp{i}") for i in range(2)]
                f_spn = [jt.tile([P, COLS], f16, name=f"fspn{i}") for i in range(2)]
                f_sig = [jt.tile([P, COLS], f16, name=f"fsig{i}") for i in range(2)]
                f_sgn = [jt.tile([P, COLS], f16, name=f"fsgn{i}") for i in range(2)]
                f_s2 = [jt.tile([P, COLS], f16, name=f"fs2{i}") for i in range(2)]
                f_nt = [jt.tile([P, COLS], f16, name=f"fnt{i}") for i in range(2)]
                f_sq = [jt.tile([P, COLS], f16, name=f"fsq{i}") for i in range(2)]
                nacc = [persist.tile([P, 1], f32, name=f"nacc{i}") for i in range(2)]

                def stA(j):
                    v = nc.vector
                    nc.scalar.activation(t_ltx[j % 2][:], ax1[:], AF.Relu,
                                         bias=gsc(10, j))
                    ts(v, t_mnx[j % 2][:], ax2[:], gsc(2, j), AL.min,
                       gsc(0, j), AL.subtract)
                    ts(v, t_lty[j % 2][:], ay1[:], gsc(1, j), AL.max)
                    ts(v, t_mny[j % 2][:], ay2[:], gsc(3, j), AL.min)

                def stB(j):
                    v = nc.vector
                    v.tensor_tensor(out=t_wxr[j % 2][:], in0=t_mnx[j % 2][:],
                                    in1=t_ltx[j % 2][:], op=AL.subtract)
                    v.tensor_tensor(out=t_wyr[j % 2][:], in0=t_mny[j % 2][:],
                                    in1=t_lty[j % 2][:], op=AL.subtract)
                    ts(v, wyp_t[j % 3][:], t_wyr[j % 2][:], 0.0, AL.max)

                def stC(j):
                    nc.scalar.activation(t_wxp[j % 3][:], t_wxr[j % 2][:],
                                         AF.Relu)

                def stD(j):
                    nc.vector.tensor_tensor(out=t_int[j % 3][:],
                                            in0=t_wxp[j % 3][:],
                                            in1=wyp_t[j % 3][:], op=AL.mult)

                def stE(j):
                    nc.scalar.activation(t_li[j % 3][:], t_int[j % 3][:],
                                         AF.Ln, bias=lnb[:])
                    nc.scalar.activation(t_den[j % 2][:], area_a[:], AF.Ln,
                                         bias=gsc(4, j))

                def stF(j):
                    v = nc.vector
                    tj = t_t[j % 3]
                    mprev = m2[(j + 1) % 2]
                    mcur = m2[j % 2]
                    v.tensor_tensor(out=tj[:], in0=t_li[j % 3][:],
                                    in1=t_den[j % 2][:], op=AL.subtract)
                    v.tensor_tensor(out=mcur[:], in0=mprev[:], in1=tj[:],
                                    op=AL.max)
                    nc.gpsimd.tensor_tensor(out=t_upd[j % 2][:], in0=tj[:],
                                            in1=mcur[:], op=AL.subtract)

                def stG(j):
                    v = nc.vector
                    upd = t_upd[j % 2]
                    ts(v, upd[:], upd[:], 0.0, AL.is_ge, float(j + 1), AL.mult)
                    v.tensor_tensor(out=bestp[:], in0=bestp[:], in1=upd[:],
                                    op=AL.max)

                stages = [stA, stB, stC, stD, stE, stF, stG]

                # focal for class c, split into 6 emission slices.
                # exp/ln formulation (single ACT table set):
                #   E = e^-x, u = 1+E, spn = ln(u) = softplus(-x),
                #   sp = x + spn = softplus(x), sgn = E/u = sigmoid(-x),
                #   sig = 1-sgn, N = sig^2*sp, P = sgn^2*spn, Rp = P/3 - N
                def focal_slice(c, s):
                    if c >= C:
                        return
                    v = nc.vector
                    xc = xcp[c]
                    E = f_sig[c % 2]; u = f_sp[c % 2]; spn = f_spn[c % 2]
                    sgn = f_sgn[c % 2]; s2n = f_s2[c % 2]; nt = f_nt[c % 2]
                    sp = u      # overwrites u after spn is computed
                    sig = E     # overwrites E (E dead after u)
                    s2 = f_sq[c % 2]
                    if s == 0:
                        nc.scalar.activation(E[:], xc[:], AF.Exp, scale=-1.0)
                    elif s == 1:
                        ts(nc.gpsimd, u[:], E[:], 1.0, AL.add)
                        nc.scalar.activation(spn[:], u[:], AF.Ln)
                    elif s == 2:
                        nc.gpsimd.tensor_tensor(out=sp[:], in0=xc[:],
                                                in1=spn[:], op=AL.add)
                        # sgn = sigmoid(-x) = exp(-softplus(x))
                        nc.scalar.activation(sgn[:], sp[:], AF.Exp, scale=-1.0)
                    elif s == 3:
                        nc.scalar.activation(s2n[:], sgn[:], AF.Square)
                        ts(v, sig[:], sgn[:], -1.0, AL.mult, 1.0, AL.add)
                    elif s == 4:
                        nc.scalar.activation(s2[:], sig[:], AF.Square)
                        v.scalar_tensor_tensor(
                            out=nt[:], in0=s2[:], scalar=0.0, in1=sp[:],
                            op0=AL.add, op1=AL.mult,
                            accum_out=nacc[c % 2][:])
                        nc.vector.tensor_tensor(out=nsumA[:], in0=nsumA[:],
                                                in1=nacc[c % 2][:], op=AL.add)
                    else:
                        nc.gpsimd.tensor_tensor(out=s2n[:], in0=s2n[:],
                                                in1=spn[:], op=AL.mult)
                        ts(nc.gpsimd, s2n[:], s2n[:], 1.0 / 3.0, AL.mult)
                        nc.gpsimd.tensor_tensor(out=Rp[c][:], in0=s2n[:],
                                                in1=nt[:], op=AL.subtract)

                # class c slices at j = 4c .. 4c+5 (overlap ok: c%2 buffers)
                sched = {}
                for c in range(C):
                    for s in range(6):
                        sched.setdefault(4 * c + s, []).append((c, s))
                NS = len(stages)
                for k in range(G + NS - 1):
                    if k % 4 == 0 and k // 4 < C:
                        nc.sync.dma_start(xcp[k // 4][:], plane(clsp, k // 4))
                    if k >= 24 and k % 2 == 0 and (k - 24) // 2 < 4:
                        k4 = (k - 24) // 2
                        nc.sync.dma_start(rpp[k4][:], plane(regp, k4))
                    for si, st in enumerate(stages):
                        j = k - si
                        if 0 <= j < G:
                            st(j)
                    for (c, s) in sched.get(k, []):
                        focal_slice(c, s)

            # ---------- pos, bpm, deferred anchor prep ----------
            pos = persist.tile([P, COLS], f16, name="pos")
            ts(nc.vector, pos[:], m2[1][:], LN_THIRD, AL.is_ge, None, AL.add,
               acc=tacc[:])
            nc.vector.tensor_tensor(out=nposA[:], in0=nposA[:], in1=tacc[:],
                                    op=AL.add)
            bpm = persist.tile([P, W], f16, name="bpm")
            nc.gpsimd.memset(bpm[:], 0.0)
            nc.vector.tensor_tensor(out=bpm[:, 0:COLS], in0=pos[:],
                                    in1=bestp[:], op=AL.mult)
            # xa/ya/iwa/iha/La/Ha (anchors still alive)
            with tc.tile_pool(name="prepp", bufs=1) as prepp:
                wa = prepp.tile([P, COLS], f16, name="wa")
                ha = prepp.tile([P, COLS], f16, name="ha")
                nc.vector.tensor_tensor(out=wa[:], in0=ax2[:], in1=ax1[:],
                                        op=AL.subtract)
                nc.gpsimd.tensor_tensor(out=ha[:], in0=ay2[:], in1=ay1[:],
                                        op=AL.subtract)
                nc.gpsimd.tensor_tensor(out=xa[:], in0=ax1[:], in1=ax2[:],
                                        op=AL.add)
                ts(nc.gpsimd, xa[:], xa[:], 0.5, AL.mult)
                nc.vector.tensor_tensor(out=ya[:], in0=ay1[:], in1=ay2[:],
                                        op=AL.add)
                ts(nc.vector, ya[:], ya[:], 0.5, AL.mult)
                with nc.allow_low_precision(reason="f16 reg-target recips"):
                    nc.vector.reciprocal(iwa[:], wa[:])
                    nc.vector.reciprocal(iha[:], ha[:])
                nc.scalar.activation(La[:], wa[:], AF.Ln)
                nc.scalar.activation(Ha[:], ha[:], AF.Ln)
            pA_stack.close()

            # ---------- phase B ----------
            with ExitStack() as bctx:
                ohp = bctx.enter_context(tc.tile_pool(name="ohp", bufs=4))
                psum_t = bctx.enter_context(
                    tc.tile_pool(name="psum_t", bufs=4, space="PSUM"))
                psum_g = bctx.enter_context(
                    tc.tile_pool(name="psum_g", bufs=4, space="PSUM"))
                gath_p = bctx.enter_context(tc.tile_pool(name="gath", bufs=2))
                scr = bctx.enter_context(tc.tile_pool(name="scr", bufs=1))

                sc = [scr.tile([P, WH], f16, name=f"sc{i}") for i in range(8)]
                accp = [persist.tile([P, 1], f32, name=f"accp{i}")
                        for i in range(2)]

                pending = []

                def emit_tail(gath, base, rw):
                    tail = []
                    tail_q0 = []

                    def gplq_f(mm, q0, q1):
                        return gath[:, mm * WH + q0:mm * WH + q1]

                    for q0, q1 in ((0, rw // 2), (rw // 2, rw)):
                        qw = q1 - q0

                        def mk_reg(k, ctr_t, inv_t, lg_t, q0=q0, q1=q1, qw=qw):
                            def go():
                                posh = pos[:, base + q0:base + q1]
                                s1, s2_, s3, s4 = sc[4 * (k % 2):4 * (k % 2) + 4]
                                rt = s1
                                if lg_t is None:
                                    nc.vector.tensor_tensor(
                                        out=s2_[:, :qw],
                                        in0=gplq_f(k, q0, q1),
                                        in1=ctr_t[:, base + q0:base + q1],
                                        op=AL.subtract)
                                    nc.vector.tensor_tensor(
                                        out=rt[:, :qw], in0=s2_[:, :qw],
                                        in1=inv_t[:, base + q0:base + q1],
                                        op=AL.mult)
                                else:
                                    nc.vector.tensor_tensor(
                                        out=rt[:, :qw], in0=gplq_f(k, q0, q1),
                                        in1=lg_t[:, base + q0:base + q1],
                                        op=AL.subtract)
                                e = s2_
                                nc.vector.tensor_tensor(
                                    out=e[:, :qw],
                                    in0=rpp[k][:, base + q0:base + q1],
                                    in1=rt[:, :qw], op=AL.subtract)
                                q = s3
                                nc.scalar.activation(q[:, :qw], e[:, :qw],
                                                     AF.Abs)
                                qm = s4
                                nc.vector.tensor_tensor(out=qm[:, :qw],
                                                        in0=q[:, :qw],
                                                        in1=posh, op=AL.mult)
                                cm = s1
                                ts(nc.vector, cm[:, :qw], qm[:, :qw], BETA,
                                   AL.min)
                                q2 = s3
                                nc.vector.tensor_tensor(
                                    out=q2[:, :qw], in0=qm[:, :qw],
                                    in1=qm[:, :qw], op=AL.add)
                                nc.vector.tensor_tensor(
                                    out=q2[:, :qw], in0=q2[:, :qw],
                                    in1=cm[:, :qw], op=AL.subtract)
                                nc.vector.scalar_tensor_tensor(
                                    out=s4[:, :qw], in0=cm[:, :qw], scalar=0.0,
                                    in1=q2[:, :qw], op0=AL.add, op1=AL.mult,
                                    accum_out=accp[k % 2][:])
                                nc.vector.tensor_tensor(
                                    out=sl1A[:], in0=sl1A[:],
                                    in1=accp[k % 2][:], op=AL.add)
                            return go

                        for k, (ctr_t, inv_t, lg_t) in enumerate(
                                ((xa, iwa, None), (ya, iha, None),
                                 (None, None, La), (None, None, Ha))):
                            tail.append(mk_reg(k, ctr_t, inv_t, lg_t))

                        def mk_corr(c, q0=q0, q1=q1, qw=qw):
                            def go():
                                eqc = sc[4 + (c % 2)]
                                # table holds label+1: background matches none
                                ts(nc.vector, eqc[:, :qw], gplq_f(4, q0, q1),
                                   float(c + 1), AL.is_equal)
                                cc = sc[6 + (c % 2)]
                                nc.vector.scalar_tensor_tensor(
                                    out=cc[:, :qw], in0=eqc[:, :qw],
                                    scalar=0.0,
                                    in1=Rp[c][:, base + q0:base + q1],
                                    op0=AL.add, op1=AL.mult,
                                    accum_out=accp[c % 2][:])
                                nc.vector.tensor_tensor(
                                    out=corrA[:], in0=corrA[:],
                                    in1=accp[c % 2][:], op=AL.add)
                            return go

                        for c in range(C):
                            tail.append(mk_corr(c))
                        if q0 == 0:
                            tail_q0 = tail
                            tail = []
                    return tail_q0, tail

                for half in range(2):
                    base = half * WH
                    rw = min(COLS - base, WH)
                    if rw <= 0:
                        break
                    gath = gath_p.tile([P, NF * WH], f16, name="gath")
                    tail_q0, tail_q1 = emit_tail(gath, base, rw)

                    for s in range(HS):
                        sq = half * HS + s
                        oh = ohp.tile([P, 512], f16, name="oh")
                        srcx = _expand32(bpm[:, 16 * sq:16 * sq + 16])
                        if (sq % 10) < ohd:
                            nc.vector.tensor_tensor(
                                out=oh[:].rearrange("p (f j) -> p f j", j=G),
                                in0=srcx,
                                in1=iotarep[:].rearrange("p (f j) -> p f j",
                                                         j=G),
                                op=AL.is_equal)
                        else:
                            # Pool: e = bpm - iota; DVE: oh = (e == 0)
                            nc.gpsimd.tensor_tensor(
                                out=oh[:].rearrange("p (f j) -> p f j", j=G),
                                in0=srcx,
                                in1=iotarep[:].rearrange("p (f j) -> p f j",
                                                         j=G),
                                op=AL.subtract)
                            ts(nc.vector, oh[:], oh[:], 0.0, AL.is_equal)
                        pt = psum_t.tile([P, 512], f16, name="pt")
                        for t4 in range(4):
                            nc.tensor.transpose(pt[:, 128 * t4:128 * t4 + 128],
                                                oh[:, 128 * t4:128 * t4 + 128],
                                                ident[:])
                        ohT = ohp.tile([P, 512], f16, name="ohT")
                        if s % 4 == 0:
                            nc.vector.tensor_copy(ohT[:], pt[:])
                        else:
                            nc.scalar.copy(ohT[:], pt[:])
                        gp = psum_g.tile([P, 4 * NF * 4], f32, name="gp")
                        for t4 in range(4):
                            nc.tensor.matmul(
                                out=gp[:, 4 * NF * t4:4 * NF * t4 + 4 * NF],
                                lhsT=ohT[:, 128 * t4:128 * t4 + 128],
                                rhs=tt16[:], start=True, stop=True)
                        src_g = gp[:].rearrange("p (t f mm) -> p t f mm",
                                                t=4, f=4)
                        dst = gath[:]
                        dst_ap = dataclasses.replace(
                            dst, offset=dst.offset + 16 * s,
                            ap=[dst.ap[0], [4, 4], [1, 4], [WH, NF]])
                        if s % 8 < 3:
                            nc.vector.tensor_copy(dst_ap, src_g)
                        else:
                            nc.scalar.copy(dst_ap, src_g)
                        # interleave reg/corr work: earlier halves first,
                        # then this half's first chunk once its columns are
                        # scattered (sqs 0..19 cover chunk q0)
                        if pending:
                            pending.pop(0)()
                        elif s >= 26 and tail_q0:
                            tail_q0.pop(0)()

                    pending = pending + tail_q0 + tail_q1
                for go in pending:
                    go()

            # ---------- final cross-partition reduce ----------
            acc4 = persist.tile([P, 4], f32, name="acc4")
            nc.scalar.copy(acc4[:, 0:1], nposA[:])
            nc.scalar.copy(acc4[:, 1:2], sl1A[:])
            nc.scalar.copy(acc4[:, 2:3], nsumA[:])
            nc.scalar.copy(acc4[:, 3:4], corrA[:])
            with tc.tile_pool(name="psum_f", bufs=1, space="PSUM") as pf:
                fps = pf.tile([1, 4], f32, name="fps")
                nc.tensor.matmul(out=fps[:], lhsT=ones1[:], rhs=acc4[:],
                                 start=True, stop=True)
                osb = persist.tile([1, 4], f32, name="osb")
                nc.scalar.copy(osb[:], fps[:])
                nc.sync.dma_start(out[:], osb[:])

    return nc


def build_for_timing():
    patch_tile_drain(1)
    nc = build(160000)
    split_sync_waits(nc)
    return nc


# ---------------- host side ----------------

def pack_inputs(cls_preds, reg_preds, anchors, gt_boxes, gt_labels):
    """Full inputs -> list of 8 per-core input maps (planar f16 layouts)."""
    B, A, _ = cls_preds.shape
    anch = np.ascontiguousarray(
        (anchors.astype(np.float32).T * np.float32(CSCALE)).astype(np.float16))
    maps = []
    for b in range(B):
        clsp = np.ascontiguousarray(
            cls_preds[b].astype(np.float32).T.astype(np.float16))
        regp = np.ascontiguousarray(
            reg_preds[b].astype(np.float32).T.astype(np.float16))
        gb = gt_boxes[b].astype(np.float32) * np.float32(CSCALE)
        gx1, gy1, gx2, gy2 = gb[:, 0], gb[:, 1], gb[:, 2], gb[:, 3]
        wg = gx2 - gx1
        hg = gy2 - gy1
        aB = wg * hg
        xg = (gx1 + gx2) * np.float32(0.5)
        yg = (gy1 + gy2) * np.float32(0.5)
        lwg = np.log(wg)
        lhg = np.log(hg)
        lab1 = gt_labels[b].astype(np.float32) + np.float32(1.0)
        gtaux = np.concatenate(
            [gx1, gy1, gx2, gy2, aB, xg, yg, lwg, lhg, lab1, -gx1]
        ).astype(np.float32)[None, :]
        maps.append({"anch": anch, "clsp": clsp, "regp": regp, "gtaux": gtaux})
    return maps


def finish(partials):
    """partials: list of [1,4] arrays per core -> (cls_loss, reg_loss)."""
    f = np.float32
    npos = f(0); sl1 = f(0); nsum = f(0); corr = f(0)
    for p in partials:
        p = p.reshape(4)
        npos += f(p[0]); sl1 += f(p[1]); nsum += f(p[2]); corr += f(p[3])
    denom = max(float(npos), 1.0)
    if npos > 0:
        cls_loss = f(0.75) * (nsum + corr) / f(denom)
        reg_loss = sl1 / f(2 * BETA) / f(denom)
    else:
        cls_loss = f(0.0); reg_loss = f(0.0)
    return np.float32(cls_loss), np.float32(reg_loss)


# ---------------- self-contained kernel entry ----------------

_CACHE = {}


def _get_fn(n_cores=8):
    if "fn" in _CACHE:
        return _CACHE["fn"]
    import jax
    from jax.sharding import Mesh, PartitionSpec, NamedSharding
    from jax.experimental.shard_map import shard_map
    from concourse.bass2jax import (_bass_exec_p, install_neuronx_cc_hook,
                                    partition_id_tensor)
    nc = build_for_timing()
    install_neuronx_cc_hook()
    in_names, out_names, out_avals, zero_shapes = [], [], [], []
    partition_name = (nc.partition_id_tensor.name
                      if nc.partition_id_tensor else None)
    for alloc in nc.m.functions[0].allocations:
        if not isinstance(alloc, mybir.MemoryLocationSet):
            continue
        name = alloc.memorylocations[0].name
        if alloc.kind == "ExternalInput":
            if name != partition_name:
                in_names.append(name)
        elif alloc.kind == "ExternalOutput":
            out_names.append(name)
            shape = tuple(alloc.tensor_shape)
            dtype = mybir.dt.np(alloc.dtype)
            out_avals.append(jax.core.ShapedArray(shape, dtype))
            zero_shapes.append((shape, dtype))
    n_params = len(in_names)
    n_outs = len(out_avals)
    all_in_names = in_names + out_names + ([partition_name]
                                           if partition_name else [])
    donate = tuple(range(n_params, n_params + n_outs))

    def _body(*args):
        operands = list(args)
        if partition_name is not None:
            operands.append(partition_id_tensor())
        outs = _bass_exec_p.bind(
            *operands, out_avals=tuple(out_avals),
            in_names=tuple(all_in_names), out_names=tuple(out_names),
            lowering_input_output_aliases=(),
            sim_require_finite=True, sim_require_nnan=True, nc=nc)
        return tuple(outs)

    devices = jax.devices()[:n_cores]
    mesh = Mesh(np.asarray(devices), ("core",))
    in_specs = (PartitionSpec("core"),) * (n_params + n_outs)
    out_specs = (PartitionSpec("core"),) * len(out_names)
    fn = jax.jit(shard_map(_body, mesh=mesh, in_specs=in_specs,
                           out_specs=out_specs, check_rep=False),
                 donate_argnums=donate, keep_unused=True)
    sh = NamedSharding(mesh, PartitionSpec("core"))
    _CACHE["fn"] = (fn, in_names, out_names, out_avals, zero_shapes, sh,
                    n_cores)
    return _CACHE["fn"]


def kernel(cls_preds, reg_preds, anchors, gt_boxes, gt_labels):
    """Full-input DetectionLoss on 8 NeuronCores (data-parallel over batch).

    Returns (cls_loss, reg_loss) as float32 scalars, matching reference()."""
    import jax
    cls_preds = np.asarray(cls_preds)
    reg_preds = np.asarray(reg_preds)
    anchors = np.asarray(anchors)
    gt_boxes = np.asarray(gt_boxes)
    gt_labels = np.asarray(gt_labels)
    B, A, _ = cls_preds.shape
    assert (B, A) == (8, 160000), (B, A)
    maps = pack_inputs(cls_preds, reg_preds, anchors, gt_boxes, gt_labels)
    fn, in_names, out_names, out_avals, zero_shapes, sh, n_cores = _get_fn()
    concat_in = [jax.device_put(
        np.concatenate([np.asarray(maps[c][nm]) for c in range(n_cores)],
                       axis=0), sh) for nm in in_names]
    zeros = [jax.device_put(
        np.zeros((n_cores * s[0], *s[1:]), d), sh) for s, d in zero_shapes]
    out_arrs = fn(*concat_in, *zeros)
    res = np.asarray(out_arrs[out_names.index("out")]).reshape(n_cores, 1, 4)
    partials = [res[c] for c in range(n_cores)]
    cls_loss, reg_loss = finish(partials)
    return cls_loss, reg_loss


# revision 11
# speedup vs baseline: 1.0760x; 1.0039x over previous
"""DetectionLoss Bass/Tile kernel for TRN2, v2 (one core = one image; SPMD x8).

fp16 data path (coords pre-scaled by 1/64 on host), per-j scalar ops from a
broadcast gt table, DVE 2x/4x perf modes, Pool runs the argmax chain,
ACT runs relu + focal activations, PE does the one-hot gather matmuls.

Per core (image b), layout: anchor a <-> (partition p = a // COLS, col a % COLS).
Inputs (per core, planar, host-packed):
  anch [4, A] f16 (x1,y1,x2,y2 scaled), clsp [8, A] f16, regp [4, A] f16,
  gtaux [1, 320] f32 = gx1 gy1 gx2 gy2 aB xg yg lwg lhg label (each [32])
Output: out [1, 4] f32 = [npos, sl1_sum, nsum, corr] partial sums; host finishes.
"""
import dataclasses
import numpy as np

import concourse.bass as bass
import concourse.mybir as mybir
from concourse import tile

AL = mybir.AluOpType
AF = mybir.ActivationFunctionType
f32 = mybir.dt.float32
f16 = mybir.dt.float16

P = 128
G = 32
C = 8
BETA = 1.0 / 9.0
THIRD = 1.0 / 3.0
LN_THIRD = float(np.log(np.float32(1.0) / np.float32(3.0)))
CSCALE = 1.0 / 64.0


def patch_tile_drain(maxw: int = 1):
    """Split the TileContext exit drain's sem waits across NOPs (walrus
    setupSyncWait rejects >1 wait on a CTRL instruction in this build)."""
    import concourse.tile as tile_mod
    from concourse.vector_clock import ScopedClock

    def _drain_and_barrier(self, tick_clock, wait_clock):
        drain_inst = self.nc.sync.drain()
        wait_clock.add_sem_waits(
            drain_inst.ins, ScopedClock({None: tick_clock.global_clock})
        )
        si = drain_inst.ins.sync_info
        waits = list(si.on_wait)
        if len(waits) > maxw:
            si.on_wait = waits[:maxw]
            rest = waits[maxw:]
            for i in range(0, len(rest), maxw):
                nop = self.nc.sync.nop(nofuse=True, hint="drain_split")
                nop.ins.sync_info = mybir.SyncInfo(
                    on_wait=rest[i:i + maxw], on_update=[]
                )
        self.nc.all_engine_barrier()
        assert self.sems is not None
        popped = self.nc._tile_sem_poison_stack.pop()
        assert popped is self._sem_poison
        self.nc.clear_and_free_semaphores(list(self.sems.allocated().values()))
        self.nc.all_engine_barrier()

    tile_mod.TileContext._drain_and_barrier = _drain_and_barrier


def split_sync_waits(nc, maxw: int = 1):
    """Walrus rejects >2 sem waits on one instruction (and >1 on CTRL-type).
    Hoist excess waits onto same-engine NOPs inserted immediately before."""
    ctr = [0]

    def mknop(engine, waits):
        ctr[0] += 1
        nop = mybir.InstNoOp(name=f"I-wsplit-{ctr[0]}", ins=[], outs=[])
        nop.engine = engine
        nop.sync_info = mybir.SyncInfo(on_wait=waits, on_update=[])
        return nop

    for blk in nc.bb_map.values():
        bb = blk.bb
        il = bb.instructions
        i = 0
        while i < len(il):
            inst = il[i]
            si = inst.sync_info
            mw = 1 if isinstance(inst, mybir.InstTensorScalarPtr) else maxw
            if si is not None and len(si.on_wait) > mw:
                waits = list(si.on_wait)
                si.on_wait = waits[:mw]
                rest = waits[mw:]
                for k in range(0, len(rest), 1):
                    il.insert(i, mknop(inst.engine, rest[k:k + 1]))
                    i += 1
            i += 1


def _expand32(ap):
    """[P, n] AP -> [P, n, 32] with step-0 inner dim (broadcast)."""
    return dataclasses.replace(ap, ap=ap.ap + [[0, G]])


def build(A: int, ohd: int = 6):
    """Emit the per-core program. A must be divisible by 128.
    ohd: number of one-hot superquads handled by DVE (rest on Pool)."""
    assert A % P == 0
    COLS = A // P                    # 1250
    NSQ = (COLS + 15) // 16          # superquads (16 cols each)
    if NSQ % 2:
        NSQ += 1                     # two equal halves
    W = NSQ * 16                     # 1280
    HS = NSQ // 2                    # sq per half
    WH = W // 2                      # 640

    nc = bass.Bass()
    anch = nc.declare_dram_parameter("anch", [4, A], f16, isOutput=False)
    clsp = nc.declare_dram_parameter("clsp", [C, A], f16, isOutput=False)
    regp = nc.declare_dram_parameter("regp", [4, A], f16, isOutput=False)
    gtaux = nc.declare_dram_parameter("gtaux", [1, 11 * G], f32, isOutput=False)
    out = nc.declare_dram_parameter("out", [1, 4], f32, isOutput=True)

    def plane(t, c):
        return t[c].rearrange("(p w) -> p w", p=P)

    def ts(eng, o, i0, s1, op0, s2=None, op1=None, acc=None):
        kw = {}
        if op1 is not None:
            kw["op1"] = op1
        if acc is not None:
            kw["accum_out"] = acc
        eng.tensor_scalar(out=o, in0=i0, scalar1=s1, scalar2=s2, op0=op0, **kw)

    with tile.TileContext(nc) as tc:
        from contextlib import ExitStack
        with ExitStack() as ctx:
            const = ctx.enter_context(tc.tile_pool(name="const", bufs=1))
            persist = ctx.enter_context(tc.tile_pool(name="persist", bufs=1))

            # ---------- constants ----------
            iotarep = const.tile([P, 512], f16, name="iotarep")
            nc.gpsimd.iota(iotarep[:], pattern=[[0, 16], [1, G]], base=1,
                           channel_multiplier=0,
                           allow_small_or_imprecise_dtypes=True)
            irow = const.tile([P, P], f32, name="irow")
            nc.gpsimd.iota(irow[:], pattern=[[1, P]], base=0,
                           channel_multiplier=0,
                           allow_small_or_imprecise_dtypes=True)
            icol = const.tile([P, 1], f32, name="icol")
            nc.gpsimd.iota(icol[:], pattern=[[0, 1]], base=0,
                           channel_multiplier=1,
                           allow_small_or_imprecise_dtypes=True)
            ident = const.tile([P, P], f16, name="ident")
            ts(nc.vector, ident[:], irow[:], icol[:], AL.is_equal)
            ones1 = const.tile([P, 1], f32, name="ones1")
            nc.gpsimd.memset(ones1[:], 1.0)
            lnb = const.tile([P, 1], f32, name="lnb")
            nc.gpsimd.memset(lnb[:], 1e-7)

            # gt broadcast [P, 320] f32
            gtb = const.tile([P, 11 * G], f32, name="gtb")
            gsrc = gtaux[:]
            gsrc_b = dataclasses.replace(gsrc, ap=[[0, P]] + gsrc.ap[1:])
            nc.sync.dma_start(gtb[:], gsrc_b)

            def gsc(k, j):
                # [P,1] f32 scalar ptr for gt field k, gt j
                return gtb[:, k * G + j:k * G + j + 1]

            # gather table tt16 [P, 20] f16, block-diag:
            # rows 32fs..+32, cols 5fs..+5 = [xg yg lwg lhg labelf]
            NF = 5
            tt16 = const.tile([P, 4 * NF], f16, name="tt16")
            nc.gpsimd.memset(tt16[:], 0.0)
            traw = const.tile([G, 10], f32, name="traw")
            gsrc2 = dataclasses.replace(gsrc, ap=[[1, G], [G, 10]])
            nc.sync.dma_start(traw[:], gsrc2)
            tblk = const.tile([G, NF], f16, name="tblk")
            nc.scalar.copy(tblk[:, 0:NF], traw[:, 5:10])
            for fs in range(4):
                nc.sync.dma_start(tt16[32 * fs:32 * fs + 32,
                                       NF * fs:NF * fs + NF], tblk[:])

            # ---------- anchors + per-anchor prep (all f16) ----------
            pA_stack = ExitStack()
            pA = pA_stack.enter_context(tc.tile_pool(name="pA", bufs=1))
            ax1 = pA.tile([P, COLS], f16, name="ax1")
            ay1 = pA.tile([P, COLS], f16, name="ay1")
            ax2 = pA.tile([P, COLS], f16, name="ax2")
            ay2 = pA.tile([P, COLS], f16, name="ay2")
            for t, c in ((ax1, 0), (ay1, 1), (ax2, 2), (ay2, 3)):
                nc.sync.dma_start(t[:], plane(anch, c))
            # cls/reg planes (f16); DMAs staggered into the j-loop
            xcp = [persist.tile([P, COLS], f16, name=f"xcp{c}") for c in range(C)]
            rpp = [persist.tile([P, COLS], f16, name=f"rpp{k}") for k in range(4)]

            area_a = pA.tile([P, COLS], f16, name="area_a")
            with tc.tile_pool(name="areap", bufs=1) as areap:
                wa0 = areap.tile([P, COLS], f16, name="wa0")
                ha0 = areap.tile([P, COLS], f16, name="ha0")
                nc.vector.tensor_tensor(out=wa0[:], in0=ax2[:], in1=ax1[:],
                                        op=AL.subtract)
                nc.gpsimd.tensor_tensor(out=ha0[:], in0=ay2[:], in1=ay1[:],
                                        op=AL.subtract)
                nc.vector.tensor_tensor(out=area_a[:], in0=wa0[:],
                                        in1=ha0[:], op=AL.mult)
            # xa/ya/iwa/iha/La/Ha are computed after the j-loop (phase B prep)
            xa = persist.tile([P, COLS], f16, name="xa")
            ya = persist.tile([P, COLS], f16, name="ya")
            iwa = persist.tile([P, COLS], f16, name="iwa")
            iha = persist.tile([P, COLS], f16, name="iha")
            La = persist.tile([P, COLS], f16, name="La")
            Ha = persist.tile([P, COLS], f16, name="Ha")

            m2 = [pA.tile([P, COLS], f16, name=f"m{i}") for i in range(2)]
            bestp = pA.tile([P, COLS], f16, name="bestp")
            nc.gpsimd.memset(m2[1][:], -60000.0)
            nc.gpsimd.memset(bestp[:], 0.0)

            # accumulators
            nposA = persist.tile([P, 1], f32, name="nposA")
            sl1A = persist.tile([P, 1], f32, name="sl1A")
            nsumA = persist.tile([P, 1], f32, name="nsumA")
            corrA = persist.tile([P, 1], f32, name="corrA")
            tacc = persist.tile([P, 1], f32, name="tacc")
            for t in (nposA, sl1A, nsumA, corrA):
                nc.vector.memset(t[:], 0.0)

            # focal result planes (retained through phase B)
            Rp = [persist.tile([P, COLS], f16, name=f"Rp{c}") for c in range(C)]

            # ---------- phase A: j-loop + interleaved focal ----------
            with tc.tile_pool(name="jt", bufs=1) as jt:
                t_ltx = [jt.tile([P, COLS], f16, name=f"ltx{i}") for i in range(2)]
                t_mnx = [jt.tile([P, COLS], f16, name=f"mnx{i}") for i in range(2)]
                t_wxr = [jt.tile([P, COLS], f16, name=f"wxr{i}") for i in range(2)]
                t_lty = [jt.tile([P, COLS], f16, name=f"lty{i}") for i in range(2)]
                t_mny = [jt.tile([P, COLS], f16, name=f"mny{i}") for i in range(2)]
                t_wyr = [jt.tile([P, COLS], f16, name=f"wyr{i}") for i in range(2)]
                t_wxp = [jt.tile([P, COLS], f16, name=f"wxp{i}") for i in range(3)]
                wyp_t = [jt.tile([P, COLS], f16, name=f"wyp{i}") for i in range(3)]
                t_li = [jt.tile([P, COLS], f16, name=f"li{i}") for i in range(3)]
                t_den = [jt.tile([P, COLS], f16, name=f"den{i}") for i in range(2)]
                t_int = [jt.tile([P, COLS], f16, name=f"int{i}") for i in range(3)]
                t_t = [jt.tile([P, COLS], f16, name=f"tt{i}") for i in range(3)]
                t_upd = [jt.tile([P, COLS], f16, name=f"upd{i}") for i in range(2)]
                # focal temps
                f_sp = [jt.tile([P, COLS], f16, name=f"fsp{i}") for i in range(2)]
                f_spn = [jt.tile([P, COLS], f16, name=f"fspn{i}") for i in range(2)]
                f_sig = [jt.tile([P, COLS], f16, name=f"fsig{i}") for i in range(2)]
                f_sgn = [jt.tile([P, COLS], f16, name=f"fsgn{i}") for i in range(2)]
                f_s2 = [jt.tile([P, COLS], f16, name=f"fs2{i}") for i in range(2)]
                f_nt = [jt.tile([P, COLS], f16, name=f"fnt{i}") for i in range(2)]
                f_sq = [jt.tile([P, COLS], f16, name=f"fsq{i}") for i in range(2)]
                nacc = [persist.tile([P, 1], f32, name=f"nacc{i}") for i in range(2)]

                def stA(j):
                    v = nc.vector
                    nc.scalar.activation(t_ltx[j % 2][:], ax1[:], AF.Relu,
                                         bias=gsc(10, j))
                    ts(v, t_mnx[j % 2][:], ax2[:], gsc(2, j), AL.min,
                       gsc(0, j), AL.subtract)
                    ts(v, t_lty[j % 2][:], ay1[:], gsc(1, j), AL.max)
                    ts(v, t_mny[j % 2][:], ay2[:], gsc(3, j), AL.min)

                def stB(j):
                    v = nc.vector
                    v.tensor_tensor(out=t_wxr[j % 2][:], in0=t_mnx[j % 2][:],
                                    in1=t_ltx[j % 2][:], op=AL.subtract)
                    v.tensor_tensor(out=t_wyr[j % 2][:], in0=t_mny[j % 2][:],
                                    in1=t_lty[j % 2][:], op=AL.subtract)
                    ts(v, wyp_t[j % 3][:], t_wyr[j % 2][:], 0.0, AL.max)

                def stC(j):
                    nc.scalar.activation(t_wxp[j % 3][:], t_wxr[j % 2][:],
                                         AF.Relu)

                def stD(j):
                    nc.vector.tensor_tensor(out=t_int[j % 3][:],
                                            in0=t_wxp[j % 3][:],
                                            in1=wyp_t[j % 3][:], op=AL.mult)

                def stE(j):
                    nc.scalar.activation(t_li[j % 3][:], t_int[j % 3][:],
                                         AF.Ln, bias=lnb[:])
                    nc.scalar.activation(t_den[j % 2][:], area_a[:], AF.Ln,
                                         bias=gsc(4, j))

                def stF(j):
                    v = nc.vector
                    tj = t_t[j % 3]
                    mprev = m2[(j + 1) % 2]
                    mcur = m2[j % 2]
                    v.tensor_tensor(out=tj[:], in0=t_li[j % 3][:],
                                    in1=t_den[j % 2][:], op=AL.subtract)
                    v.tensor_tensor(out=mcur[:], in0=mprev[:], in1=tj[:],
                                    op=AL.max)
                    nc.gpsimd.tensor_tensor(out=t_upd[j % 2][:], in0=tj[:],
                                            in1=mcur[:], op=AL.subtract)

                def stG(j):
                    v = nc.vector
                    upd = t_upd[j % 2]
                    ts(v, upd[:], upd[:], 0.0, AL.is_ge, float(j + 1), AL.mult)
                    v.tensor_tensor(out=bestp[:], in0=bestp[:], in1=upd[:],
                                    op=AL.max)

                stages = [stA, stB, stC, stD, stE, stF, stG]

                # focal for class c, split into 6 emission slices.
                # exp/ln formulation (single ACT table set):
                #   E = e^-x, u = 1+E, spn = ln(u) = softplus(-x),
                #   sp = x + spn = softplus(x), sgn = E/u = sigmoid(-x),
                #   sig = 1-sgn, N = sig^2*sp, P = sgn^2*spn, Rp = P/3 - N
                def focal_slice(c, s):
                    if c >= C:
                        return
                    v = nc.vector
                    xc = xcp[c]
                    E = f_sig[c % 2]; u = f_sp[c % 2]; spn = f_spn[c % 2]
                    sgn = f_sgn[c % 2]; s2n = f_s2[c % 2]; nt = f_nt[c % 2]
                    sp = u      # overwrites u after spn is computed
                    sig = E     # overwrites E (E dead after u)
                    s2 = f_sq[c % 2]
                    if s == 0:
                        nc.scalar.activation(E[:], xc[:], AF.Exp, scale=-1.0)
                    elif s == 1:
                        ts(nc.gpsimd, u[:], E[:], 1.0, AL.add)
                        nc.scalar.activation(spn[:], u[:], AF.Ln)
                    elif s == 2:
                        nc.gpsimd.tensor_tensor(out=sp[:], in0=xc[:],
                                                in1=spn[:], op=AL.add)
                        # sgn = sigmoid(-x) = exp(-softplus(x))
                        nc.scalar.activation(sgn[:], sp[:], AF.Exp, scale=-1.0)
                    elif s == 3:
                        nc.scalar.activation(s2n[:], sgn[:], AF.Square)
                        ts(v, sig[:], sgn[:], -1.0, AL.mult, 1.0, AL.add)
                    elif s == 4:
                        nc.scalar.activation(s2[:], sig[:], AF.Square)
                        v.scalar_tensor_tensor(
                            out=nt[:], in0=s2[:], scalar=0.0, in1=sp[:],
                            op0=AL.add, op1=AL.mult,
                            accum_out=nacc[c % 2][:])
                        nc.vector.tensor_tensor(out=nsumA[:], in0=nsumA[:],
                                                in1=nacc[c % 2][:], op=AL.add)
                    else:
                        nc.gpsimd.tensor_tensor(out=s2n[:], in0=s2n[:],
                                                in1=spn[:], op=AL.mult)
                        ts(nc.gpsimd, s2n[:], s2n[:], 1.0 / 3.0, AL.mult)
                        nc.gpsimd.tensor_tensor(out=Rp[c][:], in0=s2n[:],
                                                in1=nt[:], op=AL.subtract)

                # class c slices at j = 4c .. 4c+5 (overlap ok: c%2 buffers)
                sched = {}
                for c in range(C):
                    for s in range(6):
                        sched.setdefault(4 * c + s, []).append((c, s))
                NS = len(stages)
                for k in range(G + NS - 1):
                    if k % 4 == 0 and k // 4 < C:
                        nc.sync.dma_start(xcp[k // 4][:], plane(clsp, k // 4))
                    if k >= 24 and k % 2 == 0 and (k - 24) // 2 < 4:
                        k4 = (k - 24) // 2
                        nc.sync.dma_start(rpp[k4][:], plane(regp, k4))
                    for si, st in enumerate(stages):
                        j = k - si
                        if 0 <= j < G:
                            st(j)
                    for (c, s) in sched.get(k, []):
                        focal_slice(c, s)

            # ---------- pos, bpm, deferred anchor prep ----------
            pos = persist.tile([P, COLS], f16, name="pos")
            ts(nc.vector, pos[:], m2[1][:], LN_THIRD, AL.is_ge, None, AL.add,
               acc=tacc[:])
            nc.vector.tensor_tensor(out=nposA[:], in0=nposA[:], in1=tacc[:],
                                    op=AL.add)
            bpm = persist.tile([P, W], f16, name="bpm")
            nc.gpsimd.memset(bpm[:], 0.0)
            nc.vector.tensor_tensor(out=bpm[:, 0:COLS], in0=pos[:],
                                    in1=bestp[:], op=AL.mult)
            # xa/ya/iwa/iha/La/Ha (anchors still alive)
            with tc.tile_pool(name="prepp", bufs=1) as prepp:
                wa = prepp.tile([P, COLS], f16, name="wa")
                ha = prepp.tile([P, COLS], f16, name="ha")
                nc.vector.tensor_tensor(out=wa[:], in0=ax2[:], in1=ax1[:],
                                        op=AL.subtract)
                nc.gpsimd.tensor_tensor(out=ha[:], in0=ay2[:], in1=ay1[:],
                                        op=AL.subtract)
                nc.gpsimd.tensor_tensor(out=xa[:], in0=ax1[:], in1=ax2[:],
                                        op=AL.add)
                ts(nc.gpsimd, xa[:], xa[:], 0.5, AL.mult)
                nc.vector.tensor_tensor(out=ya[:], in0=ay1[:], in1=ay2[:],
                                        op=AL.add)
                ts(nc.vector, ya[:], ya[:], 0.5, AL.mult)
                with nc.allow_low_precision(reason="f16 reg-target recips"):
                    nc.vector.reciprocal(iwa[:], wa[:])
                    nc.vector.reciprocal(iha[:], ha[:])
                nc.scalar.activation(La[:], wa[:], AF.Ln)
                nc.scalar.activation(Ha[:], ha[:], AF.Ln)
            pA_stack.close()

            # ---------- phase B ----------
            with ExitStack() as bctx:
                ohp = bctx.enter_context(tc.tile_pool(name="ohp", bufs=4))
                psum_t = bctx.enter_context(
                    tc.tile_pool(name="psum_t", bufs=4, space="PSUM"))
                psum_g = bctx.enter_context(
                    tc.tile_pool(name="psum_g", bufs=4, space="PSUM"))
                gath_p = bctx.enter_context(tc.tile_pool(name="gath", bufs=2))
                scr = bctx.enter_context(tc.tile_pool(name="scr", bufs=1))

                sc = [scr.tile([P, WH], f16, name=f"sc{i}") for i in range(8)]
                accp = [persist.tile([P, 1], f32, name=f"accp{i}")
                        for i in range(2)]

                pending = []

                def emit_tail(gath, base, rw):
                    tail = []
                    tail_q0 = []

                    def gplq_f(mm, q0, q1):
                        return gath[:, mm * WH + q0:mm * WH + q1]

                    for q0, q1 in ((0, rw // 2), (rw // 2, rw)):
                        qw = q1 - q0

                        def mk_reg(k, ctr_t, inv_t, lg_t, q0=q0, q1=q1, qw=qw):
                            def go():
                                posh = pos[:, base + q0:base + q1]
                                s1, s2_, s3, s4 = sc[4 * (k % 2):4 * (k % 2) + 4]
                                rt = s1
                                if lg_t is None:
                                    nc.vector.tensor_tensor(
                                        out=s2_[:, :qw],
                                        in0=gplq_f(k, q0, q1),
                                        in1=ctr_t[:, base + q0:base + q1],
                                        op=AL.subtract)
                                    nc.vector.tensor_tensor(
                                        out=rt[:, :qw], in0=s2_[:, :qw],
                                        in1=inv_t[:, base + q0:base + q1],
                                        op=AL.mult)
                                else:
                                    nc.vector.tensor_tensor(
                                        out=rt[:, :qw], in0=gplq_f(k, q0, q1),
                                        in1=lg_t[:, base + q0:base + q1],
                                        op=AL.subtract)
                                e = s2_
                                nc.vector.tensor_tensor(
                                    out=e[:, :qw],
                                    in0=rpp[k][:, base + q0:base + q1],
                                    in1=rt[:, :qw], op=AL.subtract)
                                q = s3
                                nc.scalar.activation(q[:, :qw], e[:, :qw],
                                                     AF.Abs)
                                qm = s4
                                nc.vector.tensor_tensor(out=qm[:, :qw],
                                                        in0=q[:, :qw],
                                                        in1=posh, op=AL.mult)
                                cm = s1
                                ts(nc.vector, cm[:, :qw], qm[:, :qw], BETA,
                                   AL.min)
                                q2 = s3
                                nc.vector.tensor_tensor(
                                    out=q2[:, :qw], in0=qm[:, :qw],
                                    in1=qm[:, :qw], op=AL.add)
                                nc.vector.tensor_tensor(
                                    out=q2[:, :qw], in0=q2[:, :qw],
                                    in1=cm[:, :qw], op=AL.subtract)
                                nc.vector.scalar_tensor_tensor(
                                    out=s4[:, :qw], in0=cm[:, :qw], scalar=0.0,
                                    in1=q2[:, :qw], op0=AL.add, op1=AL.mult,
                                    accum_out=accp[k % 2][:])
                                nc.vector.tensor_tensor(
                                    out=sl1A[:], in0=sl1A[:],
                                    in1=accp[k % 2][:], op=AL.add)
                            return go

                        for k, (ctr_t, inv_t, lg_t) in enumerate(
                                ((xa, iwa, None), (ya, iha, None),
                                 (None, None, La), (None, None, Ha))):
                            tail.append(mk_reg(k, ctr_t, inv_t, lg_t))

                        def mk_corr(c, q0=q0, q1=q1, qw=qw):
                            def go():
                                eqc = sc[4 + (c % 2)]
                                # table holds label+1: background matches none
                                ts(nc.vector, eqc[:, :qw], gplq_f(4, q0, q1),
                                   float(c + 1), AL.is_equal)
                                cc = sc[6 + (c % 2)]
                                nc.vector.scalar_tensor_tensor(
                                    out=cc[:, :qw], in0=eqc[:, :qw],
                                    scalar=0.0,
                                    in1=Rp[c][:, base + q0:base + q1],
                                    op0=AL.add, op1=AL.mult,
                                    accum_out=accp[c % 2][:])
                                nc.vector.tensor_tensor(
                                    out=corrA[:], in0=corrA[:],
                                    in1=accp[c % 2][:], op=AL.add)
                            return go

                        for c in range(C):
                            tail.append(mk_corr(c))
                        if q0 == 0:
                            tail_q0 = tail
                            tail = []
                    return tail_q0, tail

                for half in range(2):
                    base = half * WH
                    rw = min(COLS - base, WH)
                    if rw <= 0:
                        break
                    gath = gath_p.tile([P, NF * WH], f16, name="gath")
                    tail_q0, tail_q1 = emit_tail(gath, base, rw)

                    for s in range(HS):
                        sq = half * HS + s
                        oh = ohp.tile([P, 512], f16, name="oh")
                        srcx = _expand32(bpm[:, 16 * sq:16 * sq + 16])
                        if (sq % 10) < ohd:
                            nc.vector.tensor_tensor(
                                out=oh[:].rearrange("p (f j) -> p f j", j=G),
                                in0=srcx,
                                in1=iotarep[:].rearrange("p (f j) -> p f j",
                                                         j=G),
                                op=AL.is_equal)
                        else:
                            # Pool: e = bpm - iota; DVE: oh = (e == 0)
                            nc.gpsimd.tensor_tensor(
                                out=oh[:].rearrange("p (f j) -> p f j", j=G),
                                in0=srcx,
                                in1=iotarep[:].rearrange("p (f j) -> p f j",
                                                         j=G),
                                op=AL.subtract)
                            ts(nc.vector, oh[:], oh[:], 0.0, AL.is_equal)
                        pt = psum_t.tile([P, 512], f16, name="pt")
                        for t4 in range(4):
                            nc.tensor.transpose(pt[:, 128 * t4:128 * t4 + 128],
                                                oh[:, 128 * t4:128 * t4 + 128],
                                                ident[:])
                        ohT = ohp.tile([P, 512], f16, name="ohT")
                        if s % 4 == 0:
                            nc.vector.tensor_copy(ohT[:], pt[:])
                        else:
                            nc.scalar.copy(ohT[:], pt[:])
                        gp = psum_g.tile([P, 4 * NF * 4], f32, name="gp")
                        for t4 in range(4):
                            nc.tensor.matmul(
                                out=gp[:, 4 * NF * t4:4 * NF * t4 + 4 * NF],
                                lhsT=ohT[:, 128 * t4:128 * t4 + 128],
                                rhs=tt16[:], start=True, stop=True)
                        src_g = gp[:].rearrange("p (t f mm) -> p t f mm",
                                                t=4, f=4)
                        dst = gath[:]
                        dst_ap = dataclasses.replace(
                            dst, offset=dst.offset + 16 * s,
                            ap=[dst.ap[0], [4, 4], [1, 4], [WH, NF]])
                        if s % 8 < 2:
                            nc.vector.tensor_copy(dst_ap, src_g)
                        else:
                            nc.scalar.copy(dst_ap, src_g)
                        # interleave reg/corr work: earlier halves first,
                        # then this half's first chunk once its columns are
                        # scattered (sqs 0..19 cover chunk q0)
                        if pending:
                            pending.pop(0)()
                        elif s >= 26 and tail_q0:
                            tail_q0.pop(0)()

                    pending = pending + tail_q0 + tail_q1
                for go in pending:
                    go()

            # ---------- final cross-partition reduce ----------
            acc4 = persist.tile([P, 4], f32, name="acc4")
            nc.scalar.copy(acc4[:, 0:1], nposA[:])
            nc.scalar.copy(acc4[:, 1:2], sl1A[:])
            nc.scalar.copy(acc4[:, 2:3], nsumA[:])
            nc.scalar.copy(acc4[:, 3:4], corrA[:])
            with tc.tile_pool(name="psum_f", bufs=1, space="PSUM") as pf:
                fps = pf.tile([1, 4], f32, name="fps")
                nc.tensor.matmul(out=fps[:], lhsT=ones1[:], rhs=acc4[:],
                                 start=True, stop=True)
                osb = persist.tile([1, 4], f32, name="osb")
                nc.scalar.copy(osb[:], fps[:])
                nc.sync.dma_start(out[:], osb[:])

    return nc


def build_for_timing():
    patch_tile_drain(1)
    nc = build(160000)
    split_sync_waits(nc)
    return nc


# ---------------- host side ----------------

def pack_inputs(cls_preds, reg_preds, anchors, gt_boxes, gt_labels):
    """Full inputs -> list of 8 per-core input maps (planar f16 layouts)."""
    B, A, _ = cls_preds.shape
    anch = np.ascontiguousarray(
        (anchors.astype(np.float32).T * np.float32(CSCALE)).astype(np.float16))
    maps = []
    for b in range(B):
        clsp = np.ascontiguousarray(
            cls_preds[b].astype(np.float32).T.astype(np.float16))
        regp = np.ascontiguousarray(
            reg_preds[b].astype(np.float32).T.astype(np.float16))
        gb = gt_boxes[b].astype(np.float32) * np.float32(CSCALE)
        gx1, gy1, gx2, gy2 = gb[:, 0], gb[:, 1], gb[:, 2], gb[:, 3]
        wg = gx2 - gx1
        hg = gy2 - gy1
        aB = wg * hg
        xg = (gx1 + gx2) * np.float32(0.5)
        yg = (gy1 + gy2) * np.float32(0.5)
        lwg = np.log(wg)
        lhg = np.log(hg)
        lab1 = gt_labels[b].astype(np.float32) + np.float32(1.0)
        gtaux = np.concatenate(
            [gx1, gy1, gx2, gy2, aB, xg, yg, lwg, lhg, lab1, -gx1]
        ).astype(np.float32)[None, :]
        maps.append({"anch": anch, "clsp": clsp, "regp": regp, "gtaux": gtaux})
    return maps


def finish(partials):
    """partials: list of [1,4] arrays per core -> (cls_loss, reg_loss)."""
    f = np.float32
    npos = f(0); sl1 = f(0); nsum = f(0); corr = f(0)
    for p in partials:
        p = p.reshape(4)
        npos += f(p[0]); sl1 += f(p[1]); nsum += f(p[2]); corr += f(p[3])
    denom = max(float(npos), 1.0)
    if npos > 0:
        cls_loss = f(0.75) * (nsum + corr) / f(denom)
        reg_loss = sl1 / f(2 * BETA) / f(denom)
    else:
        cls_loss = f(0.0); reg_loss = f(0.0)
    return np.float32(cls_loss), np.float32(reg_loss)


# ---------------- self-contained kernel entry ----------------

_CACHE = {}


def _get_fn(n_cores=8):
    if "fn" in _CACHE:
        return _CACHE["fn"]
    import jax
    from jax.sharding import Mesh, PartitionSpec, NamedSharding
    from jax.experimental.shard_map import shard_map
    from concourse.bass2jax import (_bass_exec_p, install_neuronx_cc_hook,
                                    partition_id_tensor)
    nc = build_for_timing()
    install_neuronx_cc_hook()
    in_names, out_names, out_avals, zero_shapes = [], [], [], []
    partition_name = (nc.partition_id_tensor.name
                      if nc.partition_id_tensor else None)
    for alloc in nc.m.functions[0].allocations:
        if not isinstance(alloc, mybir.MemoryLocationSet):
            continue
        name = alloc.memorylocations[0].name
        if alloc.kind == "ExternalInput":
            if name != partition_name:
                in_names.append(name)
        elif alloc.kind == "ExternalOutput":
            out_names.append(name)
            shape = tuple(alloc.tensor_shape)
            dtype = mybir.dt.np(alloc.dtype)
            out_avals.append(jax.core.ShapedArray(shape, dtype))
            zero_shapes.append((shape, dtype))
    n_params = len(in_names)
    n_outs = len(out_avals)
    all_in_names = in_names + out_names + ([partition_name]
                                           if partition_name else [])
    donate = tuple(range(n_params, n_params + n_outs))

    def _body(*args):
        operands = list(args)
        if partition_name is not None:
            operands.append(partition_id_tensor())
        outs = _bass_exec_p.bind(
            *operands, out_avals=tuple(out_avals),
            in_names=tuple(all_in_names), out_names=tuple(out_names),
            lowering_input_output_aliases=(),
            sim_require_finite=True, sim_require_nnan=True, nc=nc)
        return tuple(outs)

    devices = jax.devices()[:n_cores]
    mesh = Mesh(np.asarray(devices), ("core",))
    in_specs = (PartitionSpec("core"),) * (n_params + n_outs)
    out_specs = (PartitionSpec("core"),) * len(out_names)
    fn = jax.jit(shard_map(_body, mesh=mesh, in_specs=in_specs,
                           out_specs=out_specs, check_rep=False),
                 donate_argnums=donate, keep_unused=True)
    sh = NamedSharding(mesh, PartitionSpec("core"))
    _CACHE["fn"] = (fn, in_names, out_names, out_avals, zero_shapes, sh,
                    n_cores)
    return _CACHE["fn"]


def kernel(cls_preds, reg_preds, anchors, gt_boxes, gt_labels):
    """Full-input DetectionLoss on 8 NeuronCores (data-parallel over batch).

    Returns (cls_loss, reg_loss) as float32 scalars, matching reference()."""
    import jax
    cls_preds = np.asarray(cls_preds)
    reg_preds = np.asarray(reg_preds)
    anchors = np.asarray(anchors)
    gt_boxes = np.asarray(gt_boxes)
    gt_labels = np.asarray(gt_labels)
    B, A, _ = cls_preds.shape
    assert (B, A) == (8, 160000), (B, A)
    maps = pack_inputs(cls_preds, reg_preds, anchors, gt_boxes, gt_labels)
    fn, in_names, out_names, out_avals, zero_shapes, sh, n_cores = _get_fn()
    concat_in = [jax.device_put(
        np.concatenate([np.asarray(maps[c][nm]) for c in range(n_cores)],
                       axis=0), sh) for nm in in_names]
    zeros = [jax.device_put(
        np.zeros((n_cores * s[0], *s[1:]), d), sh) for s, d in zero_shapes]
    out_arrs = fn(*concat_in, *zeros)
    res = np.asarray(out_arrs[out_names.index("out")]).reshape(n_cores, 1, 4)
    partials = [res[c] for c in range(n_cores)]
    cls_loss, reg_loss = finish(partials)
    return cls_loss, reg_loss


# revision 12
# speedup vs baseline: 1.0828x; 1.0063x over previous
"""DetectionLoss Bass/Tile kernel for TRN2, v2 (one core = one image; SPMD x8).

fp16 data path (coords pre-scaled by 1/64 on host), per-j scalar ops from a
broadcast gt table, DVE 2x/4x perf modes, Pool runs the argmax chain,
ACT runs relu + focal activations, PE does the one-hot gather matmuls.

Per core (image b), layout: anchor a <-> (partition p = a // COLS, col a % COLS).
Inputs (per core, planar, host-packed):
  anch [4, A] f16 (x1,y1,x2,y2 scaled), clsp [8, A] f16, regp [4, A] f16,
  gtaux [1, 320] f32 = gx1 gy1 gx2 gy2 aB xg yg lwg lhg label (each [32])
Output: out [1, 4] f32 = [npos, sl1_sum, nsum, corr] partial sums; host finishes.
"""
import dataclasses
import numpy as np

import concourse.bass as bass
import concourse.mybir as mybir
from concourse import tile

AL = mybir.AluOpType
AF = mybir.ActivationFunctionType
f32 = mybir.dt.float32
f16 = mybir.dt.float16

P = 128
G = 32
C = 8
BETA = 1.0 / 9.0
THIRD = 1.0 / 3.0
LN_THIRD = float(np.log(np.float32(1.0) / np.float32(3.0)))
CSCALE = 1.0 / 64.0


def patch_tile_drain(maxw: int = 1):
    """Split the TileContext exit drain's sem waits across NOPs (walrus
    setupSyncWait rejects >1 wait on a CTRL instruction in this build)."""
    import concourse.tile as tile_mod
    from concourse.vector_clock import ScopedClock

    def _drain_and_barrier(self, tick_clock, wait_clock):
        drain_inst = self.nc.sync.drain()
        wait_clock.add_sem_waits(
            drain_inst.ins, ScopedClock({None: tick_clock.global_clock})
        )
        si = drain_inst.ins.sync_info
        waits = list(si.on_wait)
        if len(waits) > maxw:
            si.on_wait = waits[:maxw]
            rest = waits[maxw:]
            for i in range(0, len(rest), maxw):
                nop = self.nc.sync.nop(nofuse=True, hint="drain_split")
                nop.ins.sync_info = mybir.SyncInfo(
                    on_wait=rest[i:i + maxw], on_update=[]
                )
        self.nc.all_engine_barrier()
        assert self.sems is not None
        popped = self.nc._tile_sem_poison_stack.pop()
        assert popped is self._sem_poison
        self.nc.clear_and_free_semaphores(list(self.sems.allocated().values()))
        self.nc.all_engine_barrier()

    tile_mod.TileContext._drain_and_barrier = _drain_and_barrier


def split_sync_waits(nc, maxw: int = 1):
    """Walrus rejects >2 sem waits on one instruction (and >1 on CTRL-type).
    Hoist excess waits onto same-engine NOPs inserted immediately before."""
    ctr = [0]

    def mknop(engine, waits):
        ctr[0] += 1
        nop = mybir.InstNoOp(name=f"I-wsplit-{ctr[0]}", ins=[], outs=[])
        nop.engine = engine
        nop.sync_info = mybir.SyncInfo(on_wait=waits, on_update=[])
        return nop

    for blk in nc.bb_map.values():
        bb = blk.bb
        il = bb.instructions
        i = 0
        while i < len(il):
            inst = il[i]
            si = inst.sync_info
            mw = 1 if isinstance(inst, mybir.InstTensorScalarPtr) else maxw
            if si is not None and len(si.on_wait) > mw:
                waits = list(si.on_wait)
                si.on_wait = waits[:mw]
                rest = waits[mw:]
                for k in range(0, len(rest), 1):
                    il.insert(i, mknop(inst.engine, rest[k:k + 1]))
                    i += 1
            i += 1


def _expand32(ap):
    """[P, n] AP -> [P, n, 32] with step-0 inner dim (broadcast)."""
    return dataclasses.replace(ap, ap=ap.ap + [[0, G]])


def build(A: int, ohd: int = 6):
    """Emit the per-core program. A must be divisible by 128.
    ohd: number of one-hot superquads handled by DVE (rest on Pool)."""
    assert A % P == 0
    COLS = A // P                    # 1250
    NSQ = (COLS + 15) // 16          # superquads (16 cols each)
    if NSQ % 2:
        NSQ += 1                     # two equal halves
    W = NSQ * 16                     # 1280
    HS = NSQ // 2                    # sq per half
    WH = W // 2                      # 640

    nc = bass.Bass()
    anch = nc.declare_dram_parameter("anch", [4, A], f16, isOutput=False)
    clsp = nc.declare_dram_parameter("clsp", [C, A], f16, isOutput=False)
    regp = nc.declare_dram_parameter("regp", [4, A], f16, isOutput=False)
    gtaux = nc.declare_dram_parameter("gtaux", [1, 11 * G], f32, isOutput=False)
    out = nc.declare_dram_parameter("out", [1, 4], f32, isOutput=True)

    def plane(t, c):
        return t[c].rearrange("(p w) -> p w", p=P)

    def ts(eng, o, i0, s1, op0, s2=None, op1=None, acc=None):
        kw = {}
        if op1 is not None:
            kw["op1"] = op1
        if acc is not None:
            kw["accum_out"] = acc
        eng.tensor_scalar(out=o, in0=i0, scalar1=s1, scalar2=s2, op0=op0, **kw)

    with tile.TileContext(nc) as tc:
        from contextlib import ExitStack
        with ExitStack() as ctx:
            const = ctx.enter_context(tc.tile_pool(name="const", bufs=1))
            persist = ctx.enter_context(tc.tile_pool(name="persist", bufs=1))

            # ---------- constants ----------
            iotarep = const.tile([P, 512], f16, name="iotarep")
            nc.gpsimd.iota(iotarep[:], pattern=[[0, 16], [1, G]], base=1,
                           channel_multiplier=0,
                           allow_small_or_imprecise_dtypes=True)
            irow = const.tile([P, P], f32, name="irow")
            nc.gpsimd.iota(irow[:], pattern=[[1, P]], base=0,
                           channel_multiplier=0,
                           allow_small_or_imprecise_dtypes=True)
            icol = const.tile([P, 1], f32, name="icol")
            nc.gpsimd.iota(icol[:], pattern=[[0, 1]], base=0,
                           channel_multiplier=1,
                           allow_small_or_imprecise_dtypes=True)
            ident = const.tile([P, P], f16, name="ident")
            ts(nc.vector, ident[:], irow[:], icol[:], AL.is_equal)
            ones1 = const.tile([P, 1], f32, name="ones1")
            nc.gpsimd.memset(ones1[:], 1.0)
            lnb = const.tile([P, 1], f32, name="lnb")
            nc.gpsimd.memset(lnb[:], 1e-7)

            # gt broadcast [P, 320] f32
            gtb = const.tile([P, 11 * G], f32, name="gtb")
            gsrc = gtaux[:]
            gsrc_b = dataclasses.replace(gsrc, ap=[[0, P]] + gsrc.ap[1:])
            nc.sync.dma_start(gtb[:], gsrc_b)

            def gsc(k, j):
                # [P,1] f32 scalar ptr for gt field k, gt j
                return gtb[:, k * G + j:k * G + j + 1]

            # gather table tt16 [P, 20] f16, block-diag:
            # rows 32fs..+32, cols 5fs..+5 = [xg yg lwg lhg labelf]
            NF = 5
            tt16 = const.tile([P, 4 * NF], f16, name="tt16")
            nc.gpsimd.memset(tt16[:], 0.0)
            traw = const.tile([G, 10], f32, name="traw")
            gsrc2 = dataclasses.replace(gsrc, ap=[[1, G], [G, 10]])
            nc.sync.dma_start(traw[:], gsrc2)
            tblk = const.tile([G, NF], f16, name="tblk")
            nc.scalar.copy(tblk[:, 0:NF], traw[:, 5:10])
            for fs in range(4):
                nc.sync.dma_start(tt16[32 * fs:32 * fs + 32,
                                       NF * fs:NF * fs + NF], tblk[:])

            # ---------- anchors + per-anchor prep (all f16) ----------
            pA_stack = ExitStack()
            pA = pA_stack.enter_context(tc.tile_pool(name="pA", bufs=1))
            ax1 = pA.tile([P, COLS], f16, name="ax1")
            ay1 = pA.tile([P, COLS], f16, name="ay1")
            ax2 = pA.tile([P, COLS], f16, name="ax2")
            ay2 = pA.tile([P, COLS], f16, name="ay2")
            for t, c in ((ax1, 0), (ay1, 1), (ax2, 2), (ay2, 3)):
                nc.sync.dma_start(t[:], plane(anch, c))
            # cls/reg planes (f16); DMAs staggered into the j-loop
            xcp = [persist.tile([P, COLS], f16, name=f"xcp{c}") for c in range(C)]
            rpp = [persist.tile([P, COLS], f16, name=f"rpp{k}") for k in range(4)]

            area_a = pA.tile([P, COLS], f16, name="area_a")
            with tc.tile_pool(name="areap", bufs=1) as areap:
                wa0 = areap.tile([P, COLS], f16, name="wa0")
                ha0 = areap.tile([P, COLS], f16, name="ha0")
                nc.vector.tensor_tensor(out=wa0[:], in0=ax2[:], in1=ax1[:],
                                        op=AL.subtract)
                nc.gpsimd.tensor_tensor(out=ha0[:], in0=ay2[:], in1=ay1[:],
                                        op=AL.subtract)
                nc.vector.tensor_tensor(out=area_a[:], in0=wa0[:],
                                        in1=ha0[:], op=AL.mult)
            # xa/ya/iwa/iha/La/Ha are computed after the j-loop (phase B prep)
            xa = persist.tile([P, COLS], f16, name="xa")
            ya = persist.tile([P, COLS], f16, name="ya")
            iwa = persist.tile([P, COLS], f16, name="iwa")
            iha = persist.tile([P, COLS], f16, name="iha")
            La = persist.tile([P, COLS], f16, name="La")
            Ha = persist.tile([P, COLS], f16, name="Ha")

            m2 = [pA.tile([P, COLS], f16, name=f"m{i}") for i in range(2)]
            bestp = pA.tile([P, COLS], f16, name="bestp")
            nc.gpsimd.memset(m2[1][:], -60000.0)
            nc.gpsimd.memset(bestp[:], 0.0)

            # accumulators
            nposA = persist.tile([P, 1], f32, name="nposA")
            sl1A = persist.tile([P, 1], f32, name="sl1A")
            nsumA = persist.tile([P, 1], f32, name="nsumA")
            corrA = persist.tile([P, 1], f32, name="corrA")
            tacc = persist.tile([P, 1], f32, name="tacc")
            for t in (nposA, sl1A, nsumA, corrA):
                nc.vector.memset(t[:], 0.0)

            # focal result planes (retained through phase B)
            Rp = [persist.tile([P, COLS], f16, name=f"Rp{c}") for c in range(C)]

            # ---------- phase A: j-loop + interleaved focal ----------
            with tc.tile_pool(name="jt", bufs=1) as jt:
                t_ltx = [jt.tile([P, COLS], f16, name=f"ltx{i}") for i in range(2)]
                t_mnx = [jt.tile([P, COLS], f16, name=f"mnx{i}") for i in range(2)]
                t_wxr = [jt.tile([P, COLS], f16, name=f"wxr{i}") for i in range(2)]
                t_lty = [jt.tile([P, COLS], f16, name=f"lty{i}") for i in range(2)]
                t_mny = [jt.tile([P, COLS], f16, name=f"mny{i}") for i in range(2)]
                t_wyr = [jt.tile([P, COLS], f16, name=f"wyr{i}") for i in range(2)]
                t_wxp = [jt.tile([P, COLS], f16, name=f"wxp{i}") for i in range(3)]
                wyp_t = [jt.tile([P, COLS], f16, name=f"wyp{i}") for i in range(3)]
                t_li = [jt.tile([P, COLS], f16, name=f"li{i}") for i in range(3)]
                t_den = [jt.tile([P, COLS], f16, name=f"den{i}") for i in range(2)]
                t_int = [jt.tile([P, COLS], f16, name=f"int{i}") for i in range(3)]
                t_t = [jt.tile([P, COLS], f16, name=f"tt{i}") for i in range(3)]
                t_upd = [jt.tile([P, COLS], f16, name=f"upd{i}") for i in range(2)]
                # focal temps
                f_sp = [jt.tile([P, COLS], f16, name=f"fsp{i}") for i in range(2)]
                f_spn = [jt.tile([P, COLS], f16, name=f"fspn{i}") for i in range(2)]
                f_sig = [jt.tile([P, COLS], f16, name=f"fsig{i}") for i in range(2)]
                f_sgn = [jt.tile([P, COLS], f16, name=f"fsgn{i}") for i in range(2)]
                f_s2 = [jt.tile([P, COLS], f16, name=f"fs2{i}") for i in range(2)]
                f_nt = [jt.tile([P, COLS], f16, name=f"fnt{i}") for i in range(2)]
                f_sq = [jt.tile([P, COLS], f16, name=f"fsq{i}") for i in range(2)]
                nacc = [persist.tile([P, 1], f32, name=f"nacc{i}") for i in range(2)]

                def stA(j):
                    v = nc.vector
                    nc.scalar.activation(t_ltx[j % 2][:], ax1[:], AF.Relu,
                                         bias=gsc(10, j))
                    ts(v, t_mnx[j % 2][:], ax2[:], gsc(2, j), AL.min,
                       gsc(0, j), AL.subtract)
                    ts(v, t_lty[j % 2][:], ay1[:], gsc(1, j), AL.max)
                    ts(v, t_mny[j % 2][:], ay2[:], gsc(3, j), AL.min)

                def stB(j):
                    v = nc.vector
                    v.tensor_tensor(out=t_wxr[j % 2][:], in0=t_mnx[j % 2][:],
                                    in1=t_ltx[j % 2][:], op=AL.subtract)
                    v.tensor_tensor(out=t_wyr[j % 2][:], in0=t_mny[j % 2][:],
                                    in1=t_lty[j % 2][:], op=AL.subtract)
                    ts(v, wyp_t[j % 3][:], t_wyr[j % 2][:], 0.0, AL.max)

                def stC(j):
                    nc.scalar.activation(t_wxp[j % 3][:], t_wxr[j % 2][:],
                                         AF.Relu)

                def stD(j):
                    nc.vector.tensor_tensor(out=t_int[j % 3][:],
                                            in0=t_wxp[j % 3][:],
                                            in1=wyp_t[j % 3][:], op=AL.mult)

                def stE(j):
                    nc.scalar.activation(t_li[j % 3][:], t_int[j % 3][:],
                                         AF.Ln, bias=lnb[:])
                    nc.scalar.activation(t_den[j % 2][:], area_a[:], AF.Ln,
                                         bias=gsc(4, j))

                def stF(j):
                    v = nc.vector
                    tj = t_t[j % 3]
                    mprev = m2[(j + 1) % 2]
                    mcur = m2[j % 2]
                    v.tensor_tensor(out=tj[:], in0=t_li[j % 3][:],
                                    in1=t_den[j % 2][:], op=AL.subtract)
                    v.tensor_tensor(out=mcur[:], in0=mprev[:], in1=tj[:],
                                    op=AL.max)
                    nc.gpsimd.tensor_tensor(out=t_upd[j % 2][:], in0=tj[:],
                                            in1=mcur[:], op=AL.subtract)

                def stG(j):
                    v = nc.vector
                    upd = t_upd[j % 2]
                    ts(v, upd[:], upd[:], 0.0, AL.is_ge, float(j + 1), AL.mult)
                    v.tensor_tensor(out=bestp[:], in0=bestp[:], in1=upd[:],
                                    op=AL.max)

                stages = [stA, stB, stC, stD, stE, stF, stG]

                # focal for class c, split into 6 emission slices.
                # exp/ln formulation (single ACT table set):
                #   E = e^-x, u = 1+E, spn = ln(u) = softplus(-x),
                #   sp = x + spn = softplus(x), sgn = E/u = sigmoid(-x),
                #   sig = 1-sgn, N = sig^2*sp, P = sgn^2*spn, Rp = P/3 - N
                def focal_slice(c, s):
                    if c >= C:
                        return
                    v = nc.vector
                    xc = xcp[c]
                    E = f_sig[c % 2]; u = f_sp[c % 2]; spn = f_spn[c % 2]
                    sgn = f_sgn[c % 2]; s2n = f_s2[c % 2]; nt = f_nt[c % 2]
                    sp = u      # overwrites u after spn is computed
                    sig = E     # overwrites E (E dead after u)
                    s2 = f_sq[c % 2]
                    if s == 0:
                        nc.scalar.activation(E[:], xc[:], AF.Exp, scale=-1.0)
                    elif s == 1:
                        ts(nc.gpsimd, u[:], E[:], 1.0, AL.add)
                        nc.scalar.activation(spn[:], u[:], AF.Ln)
                    elif s == 2:
                        nc.gpsimd.tensor_tensor(out=sp[:], in0=xc[:],
                                                in1=spn[:], op=AL.add)
                        # sgn = sigmoid(-x) = exp(-softplus(x))
                        nc.scalar.activation(sgn[:], sp[:], AF.Exp, scale=-1.0)
                    elif s == 3:
                        nc.scalar.activation(s2n[:], sgn[:], AF.Square)
                        ts(v, sig[:], sgn[:], -1.0, AL.mult, 1.0, AL.add)
                    elif s == 4:
                        nc.scalar.activation(s2[:], sig[:], AF.Square)
                        v.scalar_tensor_tensor(
                            out=nt[:], in0=s2[:], scalar=0.0, in1=sp[:],
                            op0=AL.add, op1=AL.mult,
                            accum_out=nacc[c % 2][:])
                        nc.vector.tensor_tensor(out=nsumA[:], in0=nsumA[:],
                                                in1=nacc[c % 2][:], op=AL.add)
                    else:
                        nc.gpsimd.tensor_tensor(out=s2n[:], in0=s2n[:],
                                                in1=spn[:], op=AL.mult)
                        ts(nc.gpsimd, s2n[:], s2n[:], 1.0 / 3.0, AL.mult)
                        nc.gpsimd.tensor_tensor(out=Rp[c][:], in0=s2n[:],
                                                in1=nt[:], op=AL.subtract)

                # class c slices at j = 4c .. 4c+5 (overlap ok: c%2 buffers)
                sched = {}
                for c in range(C):
                    for s in range(6):
                        sched.setdefault(4 * c + s, []).append((c, s))
                NS = len(stages)
                for k in range(G + NS - 1):
                    if k % 4 == 0 and k // 4 < C:
                        nc.sync.dma_start(xcp[k // 4][:], plane(clsp, k // 4))
                    if k >= 24 and k % 2 == 0 and (k - 24) // 2 < 4:
                        k4 = (k - 24) // 2
                        nc.sync.dma_start(rpp[k4][:], plane(regp, k4))
                    for si, st in enumerate(stages):
                        j = k - si
                        if 0 <= j < G:
                            st(j)
                    for (c, s) in sched.get(k, []):
                        focal_slice(c, s)

            # ---------- pos, bpm, deferred anchor prep ----------
            pos = persist.tile([P, COLS], f16, name="pos")
            ts(nc.vector, pos[:], m2[1][:], LN_THIRD, AL.is_ge, None, AL.add,
               acc=tacc[:])
            nc.vector.tensor_tensor(out=nposA[:], in0=nposA[:], in1=tacc[:],
                                    op=AL.add)
            bpm = persist.tile([P, W], f16, name="bpm")
            nc.gpsimd.memset(bpm[:], 0.0)
            nc.vector.tensor_tensor(out=bpm[:, 0:COLS], in0=pos[:],
                                    in1=bestp[:], op=AL.mult)
            # xa/ya/iwa/iha/La/Ha (anchors still alive)
            with tc.tile_pool(name="prepp", bufs=1) as prepp:
                wa = prepp.tile([P, COLS], f16, name="wa")
                ha = prepp.tile([P, COLS], f16, name="ha")
                nc.vector.tensor_tensor(out=wa[:], in0=ax2[:], in1=ax1[:],
                                        op=AL.subtract)
                nc.gpsimd.tensor_tensor(out=ha[:], in0=ay2[:], in1=ay1[:],
                                        op=AL.subtract)
                nc.gpsimd.tensor_tensor(out=xa[:], in0=ax1[:], in1=ax2[:],
                                        op=AL.add)
                ts(nc.gpsimd, xa[:], xa[:], 0.5, AL.mult)
                nc.vector.tensor_tensor(out=ya[:], in0=ay1[:], in1=ay2[:],
                                        op=AL.add)
                ts(nc.vector, ya[:], ya[:], 0.5, AL.mult)
                with nc.allow_low_precision(reason="f16 reg-target recips"):
                    nc.vector.reciprocal(iwa[:], wa[:])
                    nc.vector.reciprocal(iha[:], ha[:])
                nc.scalar.activation(La[:], wa[:], AF.Ln)
                nc.scalar.activation(Ha[:], ha[:], AF.Ln)
            pA_stack.close()

            # ---------- phase B ----------
            with ExitStack() as bctx:
                ohp = bctx.enter_context(tc.tile_pool(name="ohp", bufs=4))
                psum_t = bctx.enter_context(
                    tc.tile_pool(name="psum_t", bufs=4, space="PSUM"))
                psum_g = bctx.enter_context(
                    tc.tile_pool(name="psum_g", bufs=4, space="PSUM"))
                gath_p = bctx.enter_context(tc.tile_pool(name="gath", bufs=2))
                scr = bctx.enter_context(tc.tile_pool(name="scr", bufs=1))

                sc = [scr.tile([P, WH], f16, name=f"sc{i}") for i in range(8)]
                accp = [persist.tile([P, 1], f32, name=f"accp{i}")
                        for i in range(2)]

                pending = []

                def emit_tail(gath, base, rw):
                    tail = []
                    tail_q0 = []

                    def gplq_f(mm, q0, q1):
                        return gath[:, mm * WH + q0:mm * WH + q1]

                    for q0, q1 in ((0, rw // 2), (rw // 2, rw)):
                        qw = q1 - q0

                        def mk_reg(k, ctr_t, inv_t, lg_t, q0=q0, q1=q1, qw=qw):
                            def go():
                                posh = pos[:, base + q0:base + q1]
                                s1, s2_, s3, s4 = sc[4 * (k % 2):4 * (k % 2) + 4]
                                rt = s1
                                if lg_t is None:
                                    nc.vector.tensor_tensor(
                                        out=s2_[:, :qw],
                                        in0=gplq_f(k, q0, q1),
                                        in1=ctr_t[:, base + q0:base + q1],
                                        op=AL.subtract)
                                    nc.vector.tensor_tensor(
                                        out=rt[:, :qw], in0=s2_[:, :qw],
                                        in1=inv_t[:, base + q0:base + q1],
                                        op=AL.mult)
                                else:
                                    nc.vector.tensor_tensor(
                                        out=rt[:, :qw], in0=gplq_f(k, q0, q1),
                                        in1=lg_t[:, base + q0:base + q1],
                                        op=AL.subtract)
                                e = s2_
                                nc.vector.tensor_tensor(
                                    out=e[:, :qw],
                                    in0=rpp[k][:, base + q0:base + q1],
                                    in1=rt[:, :qw], op=AL.subtract)
                                q = s3
                                nc.scalar.activation(q[:, :qw], e[:, :qw],
                                                     AF.Abs)
                                qm = s4
                                nc.vector.tensor_tensor(out=qm[:, :qw],
                                                        in0=q[:, :qw],
                                                        in1=posh, op=AL.mult)
                                cm = s1
                                ts(nc.vector, cm[:, :qw], qm[:, :qw], BETA,
                                   AL.min)
                                q2 = s3
                                nc.vector.tensor_tensor(
                                    out=q2[:, :qw], in0=qm[:, :qw],
                                    in1=qm[:, :qw], op=AL.add)
                                nc.vector.tensor_tensor(
                                    out=q2[:, :qw], in0=q2[:, :qw],
                                    in1=cm[:, :qw], op=AL.subtract)
                                nc.vector.scalar_tensor_tensor(
                                    out=s4[:, :qw], in0=cm[:, :qw], scalar=0.0,
                                    in1=q2[:, :qw], op0=AL.add, op1=AL.mult,
                                    accum_out=accp[k % 2][:])
                                nc.vector.tensor_tensor(
                                    out=sl1A[:], in0=sl1A[:],
                                    in1=accp[k % 2][:], op=AL.add)
                            return go

                        for k, (ctr_t, inv_t, lg_t) in enumerate(
                                ((xa, iwa, None), (ya, iha, None),
                                 (None, None, La), (None, None, Ha))):
                            tail.append(mk_reg(k, ctr_t, inv_t, lg_t))

                        def mk_corr(c, q0=q0, q1=q1, qw=qw):
                            def go():
                                eqc = sc[4 + (c % 2)]
                                # table holds label+1: background matches none
                                ts(nc.vector, eqc[:, :qw], gplq_f(4, q0, q1),
                                   float(c + 1), AL.is_equal)
                                cc = sc[6 + (c % 2)]
                                nc.vector.scalar_tensor_tensor(
                                    out=cc[:, :qw], in0=eqc[:, :qw],
                                    scalar=0.0,
                                    in1=Rp[c][:, base + q0:base + q1],
                                    op0=AL.add, op1=AL.mult,
                                    accum_out=accp[c % 2][:])
                                nc.vector.tensor_tensor(
                                    out=corrA[:], in0=corrA[:],
                                    in1=accp[c % 2][:], op=AL.add)
                            return go

                        for c in range(C):
                            tail.append(mk_corr(c))
                        if q0 == 0:
                            tail_q0 = tail
                            tail = []
                    return tail_q0, tail

                for half in range(2):
                    base = half * WH
                    rw = min(COLS - base, WH)
                    if rw <= 0:
                        break
                    gath = gath_p.tile([P, NF * WH], f16, name="gath")
                    tail_q0, tail_q1 = emit_tail(gath, base, rw)

                    for s in range(HS):
                        sq = half * HS + s
                        oh = ohp.tile([P, 512], f16, name="oh")
                        srcx = _expand32(bpm[:, 16 * sq:16 * sq + 16])
                        if (sq % 10) < ohd:
                            nc.vector.tensor_tensor(
                                out=oh[:].rearrange("p (f j) -> p f j", j=G),
                                in0=srcx,
                                in1=iotarep[:].rearrange("p (f j) -> p f j",
                                                         j=G),
                                op=AL.is_equal)
                        else:
                            # Pool: e = bpm - iota; DVE: oh = (e == 0)
                            nc.gpsimd.tensor_tensor(
                                out=oh[:].rearrange("p (f j) -> p f j", j=G),
                                in0=srcx,
                                in1=iotarep[:].rearrange("p (f j) -> p f j",
                                                         j=G),
                                op=AL.subtract)
                            ts(nc.vector, oh[:], oh[:], 0.0, AL.is_equal)
                        pt = psum_t.tile([P, 512], f16, name="pt")
                        for t4 in range(4):
                            nc.tensor.transpose(pt[:, 128 * t4:128 * t4 + 128],
                                                oh[:, 128 * t4:128 * t4 + 128],
                                                ident[:])
                        ohT = ohp.tile([P, 512], f16, name="ohT")
                        nc.scalar.copy(ohT[:], pt[:])
                        gp = psum_g.tile([P, 4 * NF * 4], f32, name="gp")
                        for t4 in range(4):
                            nc.tensor.matmul(
                                out=gp[:, 4 * NF * t4:4 * NF * t4 + 4 * NF],
                                lhsT=ohT[:, 128 * t4:128 * t4 + 128],
                                rhs=tt16[:], start=True, stop=True)
                        src_g = gp[:].rearrange("p (t f mm) -> p t f mm",
                                                t=4, f=4)
                        dst = gath[:]
                        dst_ap = dataclasses.replace(
                            dst, offset=dst.offset + 16 * s,
                            ap=[dst.ap[0], [4, 4], [1, 4], [WH, NF]])
                        if s % 8 < 2:
                            nc.vector.tensor_copy(dst_ap, src_g)
                        else:
                            nc.scalar.copy(dst_ap, src_g)
                        # interleave reg/corr work: earlier halves first,
                        # then this half's first chunk once its columns are
                        # scattered (sqs 0..19 cover chunk q0)
                        if pending:
                            pending.pop(0)()
                        elif s >= 26 and tail_q0:
                            tail_q0.pop(0)()

                    pending = pending + tail_q0 + tail_q1
                for go in pending:
                    go()

            # ---------- final cross-partition reduce ----------
            acc4 = persist.tile([P, 4], f32, name="acc4")
            nc.scalar.copy(acc4[:, 0:1], nposA[:])
            nc.scalar.copy(acc4[:, 1:2], sl1A[:])
            nc.scalar.copy(acc4[:, 2:3], nsumA[:])
            nc.scalar.copy(acc4[:, 3:4], corrA[:])
            with tc.tile_pool(name="psum_f", bufs=1, space="PSUM") as pf:
                fps = pf.tile([1, 4], f32, name="fps")
                nc.tensor.matmul(out=fps[:], lhsT=ones1[:], rhs=acc4[:],
                                 start=True, stop=True)
                osb = persist.tile([1, 4], f32, name="osb")
                nc.scalar.copy(osb[:], fps[:])
                nc.sync.dma_start(out[:], osb[:])

    return nc


def build_for_timing():
    patch_tile_drain(1)
    nc = build(160000)
    split_sync_waits(nc)
    return nc


# ---------------- host side ----------------

def pack_inputs(cls_preds, reg_preds, anchors, gt_boxes, gt_labels):
    """Full inputs -> list of 8 per-core input maps (planar f16 layouts)."""
    B, A, _ = cls_preds.shape
    anch = np.ascontiguousarray(
        (anchors.astype(np.float32).T * np.float32(CSCALE)).astype(np.float16))
    maps = []
    for b in range(B):
        clsp = np.ascontiguousarray(
            cls_preds[b].astype(np.float32).T.astype(np.float16))
        regp = np.ascontiguousarray(
            reg_preds[b].astype(np.float32).T.astype(np.float16))
        gb = gt_boxes[b].astype(np.float32) * np.float32(CSCALE)
        gx1, gy1, gx2, gy2 = gb[:, 0], gb[:, 1], gb[:, 2], gb[:, 3]
        wg = gx2 - gx1
        hg = gy2 - gy1
        aB = wg * hg
        xg = (gx1 + gx2) * np.float32(0.5)
        yg = (gy1 + gy2) * np.float32(0.5)
        lwg = np.log(wg)
        lhg = np.log(hg)
        lab1 = gt_labels[b].astype(np.float32) + np.float32(1.0)
        gtaux = np.concatenate(
            [gx1, gy1, gx2, gy2, aB, xg, yg, lwg, lhg, lab1, -gx1]
        ).astype(np.float32)[None, :]
        maps.append({"anch": anch, "clsp": clsp, "regp": regp, "gtaux": gtaux})
    return maps


def finish(partials):
    """partials: list of [1,4] arrays per core -> (cls_loss, reg_loss)."""
    f = np.float32
    npos = f(0); sl1 = f(0); nsum = f(0); corr = f(0)
    for p in partials:
        p = p.reshape(4)
        npos += f(p[0]); sl1 += f(p[1]); nsum += f(p[2]); corr += f(p[3])
    denom = max(float(npos), 1.0)
    if npos > 0:
        cls_loss = f(0.75) * (nsum + corr) / f(denom)
        reg_loss = sl1 / f(2 * BETA) / f(denom)
    else:
        cls_loss = f(0.0); reg_loss = f(0.0)
    return np.float32(cls_loss), np.float32(reg_loss)


# ---------------- self-contained kernel entry ----------------

_CACHE = {}


def _get_fn(n_cores=8):
    if "fn" in _CACHE:
        return _CACHE["fn"]
    import jax
    from jax.sharding import Mesh, PartitionSpec, NamedSharding
    from jax.experimental.shard_map import shard_map
    from concourse.bass2jax import (_bass_exec_p, install_neuronx_cc_hook,
                                    partition_id_tensor)
    nc = build_for_timing()
    install_neuronx_cc_hook()
    in_names, out_names, out_avals, zero_shapes = [], [], [], []
    partition_name = (nc.partition_id_tensor.name
                      if nc.partition_id_tensor else None)
    for alloc in nc.m.functions[0].allocations:
        if not isinstance(alloc, mybir.MemoryLocationSet):
            continue
        name = alloc.memorylocations[0].name
        if alloc.kind == "ExternalInput":
            if name != partition_name:
                in_names.append(name)
        elif alloc.kind == "ExternalOutput":
            out_names.append(name)
            shape = tuple(alloc.tensor_shape)
            dtype = mybir.dt.np(alloc.dtype)
            out_avals.append(jax.core.ShapedArray(shape, dtype))
            zero_shapes.append((shape, dtype))
    n_params = len(in_names)
    n_outs = len(out_avals)
    all_in_names = in_names + out_names + ([partition_name]
                                           if partition_name else [])
    donate = tuple(range(n_params, n_params + n_outs))

    def _body(*args):
        operands = list(args)
        if partition_name is not None:
            operands.append(partition_id_tensor())
        outs = _bass_exec_p.bind(
            *operands, out_avals=tuple(out_avals),
            in_names=tuple(all_in_names), out_names=tuple(out_names),
            lowering_input_output_aliases=(),
            sim_require_finite=True, sim_require_nnan=True, nc=nc)
        return tuple(outs)

    devices = jax.devices()[:n_cores]
    mesh = Mesh(np.asarray(devices), ("core",))
    in_specs = (PartitionSpec("core"),) * (n_params + n_outs)
    out_specs = (PartitionSpec("core"),) * len(out_names)
    fn = jax.jit(shard_map(_body, mesh=mesh, in_specs=in_specs,
                           out_specs=out_specs, check_rep=False),
                 donate_argnums=donate, keep_unused=True)
    sh = NamedSharding(mesh, PartitionSpec("core"))
    _CACHE["fn"] = (fn, in_names, out_names, out_avals, zero_shapes, sh,
                    n_cores)
    return _CACHE["fn"]


def kernel(cls_preds, reg_preds, anchors, gt_boxes, gt_labels):
    """Full-input DetectionLoss on 8 NeuronCores (data-parallel over batch).

    Returns (cls_loss, reg_loss) as float32 scalars, matching reference()."""
    import jax
    cls_preds = np.asarray(cls_preds)
    reg_preds = np.asarray(reg_preds)
    anchors = np.asarray(anchors)
    gt_boxes = np.asarray(gt_boxes)
    gt_labels = np.asarray(gt_labels)
    B, A, _ = cls_preds.shape
    assert (B, A) == (8, 160000), (B, A)
    maps = pack_inputs(cls_preds, reg_preds, anchors, gt_boxes, gt_labels)
    fn, in_names, out_names, out_avals, zero_shapes, sh, n_cores = _get_fn()
    concat_in = [jax.device_put(
        np.concatenate([np.asarray(maps[c][nm]) for c in range(n_cores)],
                       axis=0), sh) for nm in in_names]
    zeros = [jax.device_put(
        np.zeros((n_cores * s[0], *s[1:]), d), sh) for s, d in zero_shapes]
    out_arrs = fn(*concat_in, *zeros)
    res = np.asarray(out_arrs[out_names.index("out")]).reshape(n_cores, 1, 4)
    partials = [res[c] for c in range(n_cores)]
    cls_loss, reg_loss = finish(partials)
    return cls_loss, reg_loss


# revision 13
# speedup vs baseline: 1.0849x; 1.0020x over previous
"""DetectionLoss Bass/Tile kernel for TRN2, v2 (one core = one image; SPMD x8).

fp16 data path (coords pre-scaled by 1/64 on host), per-j scalar ops from a
broadcast gt table, DVE 2x/4x perf modes, Pool runs the argmax chain,
ACT runs relu + focal activations, PE does the one-hot gather matmuls.

Per core (image b), layout: anchor a <-> (partition p = a // COLS, col a % COLS).
Inputs (per core, planar, host-packed):
  anch [4, A] f16 (x1,y1,x2,y2 scaled), clsp [8, A] f16, regp [4, A] f16,
  gtaux [1, 320] f32 = gx1 gy1 gx2 gy2 aB xg yg lwg lhg label (each [32])
Output: out [1, 4] f32 = [npos, sl1_sum, nsum, corr] partial sums; host finishes.
"""
import dataclasses
import numpy as np

import concourse.bass as bass
import concourse.mybir as mybir
from concourse import tile

AL = mybir.AluOpType
AF = mybir.ActivationFunctionType
f32 = mybir.dt.float32
f16 = mybir.dt.float16

P = 128
G = 32
C = 8
BETA = 1.0 / 9.0
THIRD = 1.0 / 3.0
LN_THIRD = float(np.log(np.float32(1.0) / np.float32(3.0)))
CSCALE = 1.0 / 64.0


def patch_tile_drain(maxw: int = 1):
    """Split the TileContext exit drain's sem waits across NOPs (walrus
    setupSyncWait rejects >1 wait on a CTRL instruction in this build)."""
    import concourse.tile as tile_mod
    from concourse.vector_clock import ScopedClock

    def _drain_and_barrier(self, tick_clock, wait_clock):
        drain_inst = self.nc.sync.drain()
        wait_clock.add_sem_waits(
            drain_inst.ins, ScopedClock({None: tick_clock.global_clock})
        )
        si = drain_inst.ins.sync_info
        waits = list(si.on_wait)
        if len(waits) > maxw:
            si.on_wait = waits[:maxw]
            rest = waits[maxw:]
            for i in range(0, len(rest), maxw):
                nop = self.nc.sync.nop(nofuse=True, hint="drain_split")
                nop.ins.sync_info = mybir.SyncInfo(
                    on_wait=rest[i:i + maxw], on_update=[]
                )
        self.nc.all_engine_barrier()
        assert self.sems is not None
        popped = self.nc._tile_sem_poison_stack.pop()
        assert popped is self._sem_poison
        self.nc.clear_and_free_semaphores(list(self.sems.allocated().values()))
        self.nc.all_engine_barrier()

    tile_mod.TileContext._drain_and_barrier = _drain_and_barrier


def split_sync_waits(nc, maxw: int = 1):
    """Walrus rejects >2 sem waits on one instruction (and >1 on CTRL-type).
    Hoist excess waits onto same-engine NOPs inserted immediately before."""
    ctr = [0]

    def mknop(engine, waits):
        ctr[0] += 1
        nop = mybir.InstNoOp(name=f"I-wsplit-{ctr[0]}", ins=[], outs=[])
        nop.engine = engine
        nop.sync_info = mybir.SyncInfo(on_wait=waits, on_update=[])
        return nop

    for blk in nc.bb_map.values():
        bb = blk.bb
        il = bb.instructions
        i = 0
        while i < len(il):
            inst = il[i]
            si = inst.sync_info
            mw = 1 if isinstance(inst, mybir.InstTensorScalarPtr) else maxw
            if si is not None and len(si.on_wait) > mw:
                waits = list(si.on_wait)
                si.on_wait = waits[:mw]
                rest = waits[mw:]
                for k in range(0, len(rest), 1):
                    il.insert(i, mknop(inst.engine, rest[k:k + 1]))
                    i += 1
            i += 1


def _expand32(ap):
    """[P, n] AP -> [P, n, 32] with step-0 inner dim (broadcast)."""
    return dataclasses.replace(ap, ap=ap.ap + [[0, G]])


def build(A: int, ohd: int = 7):
    """Emit the per-core program. A must be divisible by 128.
    ohd: number of one-hot superquads handled by DVE (rest on Pool)."""
    assert A % P == 0
    COLS = A // P                    # 1250
    NSQ = (COLS + 15) // 16          # superquads (16 cols each)
    if NSQ % 2:
        NSQ += 1                     # two equal halves
    W = NSQ * 16                     # 1280
    HS = NSQ // 2                    # sq per half
    WH = W // 2                      # 640

    nc = bass.Bass()
    anch = nc.declare_dram_parameter("anch", [4, A], f16, isOutput=False)
    clsp = nc.declare_dram_parameter("clsp", [C, A], f16, isOutput=False)
    regp = nc.declare_dram_parameter("regp", [4, A], f16, isOutput=False)
    gtaux = nc.declare_dram_parameter("gtaux", [1, 11 * G], f32, isOutput=False)
    out = nc.declare_dram_parameter("out", [1, 4], f32, isOutput=True)

    def plane(t, c):
        return t[c].rearrange("(p w) -> p w", p=P)

    def ts(eng, o, i0, s1, op0, s2=None, op1=None, acc=None):
        kw = {}
        if op1 is not None:
            kw["op1"] = op1
        if acc is not None:
            kw["accum_out"] = acc
        eng.tensor_scalar(out=o, in0=i0, scalar1=s1, scalar2=s2, op0=op0, **kw)

    with tile.TileContext(nc) as tc:
        from contextlib import ExitStack
        with ExitStack() as ctx:
            const = ctx.enter_context(tc.tile_pool(name="const", bufs=1))
            persist = ctx.enter_context(tc.tile_pool(name="persist", bufs=1))

            # ---------- constants ----------
            iotarep = const.tile([P, 512], f16, name="iotarep")
            nc.gpsimd.iota(iotarep[:], pattern=[[0, 16], [1, G]], base=1,
                           channel_multiplier=0,
                           allow_small_or_imprecise_dtypes=True)
            irow = const.tile([P, P], f32, name="irow")
            nc.gpsimd.iota(irow[:], pattern=[[1, P]], base=0,
                           channel_multiplier=0,
                           allow_small_or_imprecise_dtypes=True)
            icol = const.tile([P, 1], f32, name="icol")
            nc.gpsimd.iota(icol[:], pattern=[[0, 1]], base=0,
                           channel_multiplier=1,
                           allow_small_or_imprecise_dtypes=True)
            ident = const.tile([P, P], f16, name="ident")
            ts(nc.vector, ident[:], irow[:], icol[:], AL.is_equal)
            ones1 = const.tile([P, 1], f32, name="ones1")
            nc.gpsimd.memset(ones1[:], 1.0)
            lnb = const.tile([P, 1], f32, name="lnb")
            nc.gpsimd.memset(lnb[:], 1e-7)

            # gt broadcast [P, 320] f32
            gtb = const.tile([P, 11 * G], f32, name="gtb")
            gsrc = gtaux[:]
            gsrc_b = dataclasses.replace(gsrc, ap=[[0, P]] + gsrc.ap[1:])
            nc.sync.dma_start(gtb[:], gsrc_b)

            def gsc(k, j):
                # [P,1] f32 scalar ptr for gt field k, gt j
                return gtb[:, k * G + j:k * G + j + 1]

            # gather table tt16 [P, 20] f16, block-diag:
            # rows 32fs..+32, cols 5fs..+5 = [xg yg lwg lhg labelf]
            NF = 5
            tt16 = const.tile([P, 4 * NF], f16, name="tt16")
            nc.gpsimd.memset(tt16[:], 0.0)
            traw = const.tile([G, 10], f32, name="traw")
            gsrc2 = dataclasses.replace(gsrc, ap=[[1, G], [G, 10]])
            nc.sync.dma_start(traw[:], gsrc2)
            tblk = const.tile([G, NF], f16, name="tblk")
            nc.scalar.copy(tblk[:, 0:NF], traw[:, 5:10])
            for fs in range(4):
                nc.sync.dma_start(tt16[32 * fs:32 * fs + 32,
                                       NF * fs:NF * fs + NF], tblk[:])

            # ---------- anchors + per-anchor prep (all f16) ----------
            pA_stack = ExitStack()
            pA = pA_stack.enter_context(tc.tile_pool(name="pA", bufs=1))
            ax1 = pA.tile([P, COLS], f16, name="ax1")
            ay1 = pA.tile([P, COLS], f16, name="ay1")
            ax2 = pA.tile([P, COLS], f16, name="ax2")
            ay2 = pA.tile([P, COLS], f16, name="ay2")
            for t, c in ((ax1, 0), (ay1, 1), (ax2, 2), (ay2, 3)):
                nc.sync.dma_start(t[:], plane(anch, c))
            # cls/reg planes (f16); DMAs staggered into the j-loop
            xcp = [persist.tile([P, COLS], f16, name=f"xcp{c}") for c in range(C)]
            rpp = [persist.tile([P, COLS], f16, name=f"rpp{k}") for k in range(4)]

            area_a = pA.tile([P, COLS], f16, name="area_a")
            with tc.tile_pool(name="areap", bufs=1) as areap:
                wa0 = areap.tile([P, COLS], f16, name="wa0")
                ha0 = areap.tile([P, COLS], f16, name="ha0")
                nc.vector.tensor_tensor(out=wa0[:], in0=ax2[:], in1=ax1[:],
                                        op=AL.subtract)
                nc.gpsimd.tensor_tensor(out=ha0[:], in0=ay2[:], in1=ay1[:],
                                        op=AL.subtract)
                nc.vector.tensor_tensor(out=area_a[:], in0=wa0[:],
                                        in1=ha0[:], op=AL.mult)
            # xa/ya/iwa/iha/La/Ha are computed after the j-loop (phase B prep)
            xa = persist.tile([P, COLS], f16, name="xa")
            ya = persist.tile([P, COLS], f16, name="ya")
            iwa = persist.tile([P, COLS], f16, name="iwa")
            iha = persist.tile([P, COLS], f16, name="iha")
            La = persist.tile([P, COLS], f16, name="La")
            Ha = persist.tile([P, COLS], f16, name="Ha")

            m2 = [pA.tile([P, COLS], f16, name=f"m{i}") for i in range(2)]
            bestp = pA.tile([P, COLS], f16, name="bestp")
            nc.gpsimd.memset(m2[1][:], -60000.0)
            nc.gpsimd.memset(bestp[:], 0.0)

            # accumulators
            nposA = persist.tile([P, 1], f32, name="nposA")
            sl1A = persist.tile([P, 1], f32, name="sl1A")
            nsumA = persist.tile([P, 1], f32, name="nsumA")
            corrA = persist.tile([P, 1], f32, name="corrA")
            tacc = persist.tile([P, 1], f32, name="tacc")
            for t in (nposA, sl1A, nsumA, corrA):
                nc.vector.memset(t[:], 0.0)

            # focal result planes (retained through phase B)
            Rp = [persist.tile([P, COLS], f16, name=f"Rp{c}") for c in range(C)]

            # ---------- phase A: j-loop + interleaved focal ----------
            with tc.tile_pool(name="jt", bufs=1) as jt:
                t_ltx = [jt.tile([P, COLS], f16, name=f"ltx{i}") for i in range(2)]
                t_mnx = [jt.tile([P, COLS], f16, name=f"mnx{i}") for i in range(2)]
                t_wxr = [jt.tile([P, COLS], f16, name=f"wxr{i}") for i in range(2)]
                t_lty = [jt.tile([P, COLS], f16, name=f"lty{i}") for i in range(2)]
                t_mny = [jt.tile([P, COLS], f16, name=f"mny{i}") for i in range(2)]
                t_wyr = [jt.tile([P, COLS], f16, name=f"wyr{i}") for i in range(2)]
                t_wxp = [jt.tile([P, COLS], f16, name=f"wxp{i}") for i in range(3)]
                wyp_t = [jt.tile([P, COLS], f16, name=f"wyp{i}") for i in range(3)]
                t_li = [jt.tile([P, COLS], f16, name=f"li{i}") for i in range(3)]
                t_den = [jt.tile([P, COLS], f16, name=f"den{i}") for i in range(2)]
                t_int = [jt.tile([P, COLS], f16, name=f"int{i}") for i in range(3)]
                t_t = [jt.tile([P, COLS], f16, name=f"tt{i}") for i in range(3)]
                t_upd = [jt.tile([P, COLS], f16, name=f"upd{i}") for i in range(2)]
                # focal temps
                f_sp = [jt.tile([P, COLS], f16, name=f"fsp{i}") for i in range(2)]
                f_spn = [jt.tile([P, COLS], f16, name=f"fspn{i}") for i in range(2)]
                f_sig = [jt.tile([P, COLS], f16, name=f"fsig{i}") for i in range(2)]
                f_sgn = [jt.tile([P, COLS], f16, name=f"fsgn{i}") for i in range(2)]
                f_s2 = [jt.tile([P, COLS], f16, name=f"fs2{i}") for i in range(2)]
                f_nt = [jt.tile([P, COLS], f16, name=f"fnt{i}") for i in range(2)]
                f_sq = [jt.tile([P, COLS], f16, name=f"fsq{i}") for i in range(2)]
                nacc = [persist.tile([P, 1], f32, name=f"nacc{i}") for i in range(2)]

                def stA(j):
                    v = nc.vector
                    nc.scalar.activation(t_ltx[j % 2][:], ax1[:], AF.Relu,
                                         bias=gsc(10, j))
                    ts(v, t_mnx[j % 2][:], ax2[:], gsc(2, j), AL.min,
                       gsc(0, j), AL.subtract)
                    ts(v, t_lty[j % 2][:], ay1[:], gsc(1, j), AL.max)
                    ts(v, t_mny[j % 2][:], ay2[:], gsc(3, j), AL.min)

                def stB(j):
                    v = nc.vector
                    v.tensor_tensor(out=t_wxr[j % 2][:], in0=t_mnx[j % 2][:],
                                    in1=t_ltx[j % 2][:], op=AL.subtract)
                    v.tensor_tensor(out=t_wyr[j % 2][:], in0=t_mny[j % 2][:],
                                    in1=t_lty[j % 2][:], op=AL.subtract)
                    ts(v, wyp_t[j % 3][:], t_wyr[j % 2][:], 0.0, AL.max)

                def stC(j):
                    nc.scalar.activation(t_wxp[j % 3][:], t_wxr[j % 2][:],
                                         AF.Relu)

                def stD(j):
                    nc.vector.tensor_tensor(out=t_int[j % 3][:],
                                            in0=t_wxp[j % 3][:],
                                            in1=wyp_t[j % 3][:], op=AL.mult)

                def stE(j):
                    nc.scalar.activation(t_li[j % 3][:], t_int[j % 3][:],
                                         AF.Ln, bias=lnb[:])
                    nc.scalar.activation(t_den[j % 2][:], area_a[:], AF.Ln,
                                         bias=gsc(4, j))

                def stF(j):
                    v = nc.vector
                    tj = t_t[j % 3]
                    mprev = m2[(j + 1) % 2]
                    mcur = m2[j % 2]
                    v.tensor_tensor(out=tj[:], in0=t_li[j % 3][:],
                                    in1=t_den[j % 2][:], op=AL.subtract)
                    v.tensor_tensor(out=mcur[:], in0=mprev[:], in1=tj[:],
                                    op=AL.max)
                    nc.gpsimd.tensor_tensor(out=t_upd[j % 2][:], in0=tj[:],
                                            in1=mcur[:], op=AL.subtract)

                def stG(j):
                    v = nc.vector
                    upd = t_upd[j % 2]
                    ts(v, upd[:], upd[:], 0.0, AL.is_ge, float(j + 1), AL.mult)
                    v.tensor_tensor(out=bestp[:], in0=bestp[:], in1=upd[:],
                                    op=AL.max)

                stages = [stA, stB, stC, stD, stE, stF, stG]

                # focal for class c, split into 6 emission slices.
                # exp/ln formulation (single ACT table set):
                #   E = e^-x, u = 1+E, spn = ln(u) = softplus(-x),
                #   sp = x + spn = softplus(x), sgn = E/u = sigmoid(-x),
                #   sig = 1-sgn, N = sig^2*sp, P = sgn^2*spn, Rp = P/3 - N
                def focal_slice(c, s):
                    if c >= C:
                        return
                    v = nc.vector
                    xc = xcp[c]
                    E = f_sig[c % 2]; u = f_sp[c % 2]; spn = f_spn[c % 2]
                    sgn = f_sgn[c % 2]; s2n = f_s2[c % 2]; nt = f_nt[c % 2]
                    sp = u      # overwrites u after spn is computed
                    sig = E     # overwrites E (E dead after u)
                    s2 = f_sq[c % 2]
                    if s == 0:
                        nc.scalar.activation(E[:], xc[:], AF.Exp, scale=-1.0)
                    elif s == 1:
                        ts(nc.gpsimd, u[:], E[:], 1.0, AL.add)
                        nc.scalar.activation(spn[:], u[:], AF.Ln)
                    elif s == 2:
                        nc.gpsimd.tensor_tensor(out=sp[:], in0=xc[:],
                                                in1=spn[:], op=AL.add)
                        # sgn = sigmoid(-x) = exp(-softplus(x))
                        nc.scalar.activation(sgn[:], sp[:], AF.Exp, scale=-1.0)
                    elif s == 3:
                        nc.scalar.activation(s2n[:], sgn[:], AF.Square)
                        ts(v, sig[:], sgn[:], -1.0, AL.mult, 1.0, AL.add)
                    elif s == 4:
                        nc.scalar.activation(s2[:], sig[:], AF.Square)
                        v.scalar_tensor_tensor(
                            out=nt[:], in0=s2[:], scalar=0.0, in1=sp[:],
                            op0=AL.add, op1=AL.mult,
                            accum_out=nacc[c % 2][:])
                        nc.vector.tensor_tensor(out=nsumA[:], in0=nsumA[:],
                                                in1=nacc[c % 2][:], op=AL.add)
                    else:
                        nc.gpsimd.tensor_tensor(out=s2n[:], in0=s2n[:],
                                                in1=spn[:], op=AL.mult)
                        ts(nc.gpsimd, s2n[:], s2n[:], 1.0 / 3.0, AL.mult)
                        nc.gpsimd.tensor_tensor(out=Rp[c][:], in0=s2n[:],
                                                in1=nt[:], op=AL.subtract)

                # class c slices at j = 4c .. 4c+5 (overlap ok: c%2 buffers)
                sched = {}
                for c in range(C):
                    for s in range(6):
                        sched.setdefault(4 * c + s, []).append((c, s))
                NS = len(stages)
                for k in range(G + NS - 1):
                    if k % 4 == 0 and k // 4 < C:
                        nc.sync.dma_start(xcp[k // 4][:], plane(clsp, k // 4))
                    if k >= 24 and k % 2 == 0 and (k - 24) // 2 < 4:
                        k4 = (k - 24) // 2
                        nc.sync.dma_start(rpp[k4][:], plane(regp, k4))
                    for si, st in enumerate(stages):
                        j = k - si
                        if 0 <= j < G:
                            st(j)
                    for (c, s) in sched.get(k, []):
                        focal_slice(c, s)

            # ---------- pos, bpm, deferred anchor prep ----------
            pos = persist.tile([P, COLS], f16, name="pos")
            ts(nc.vector, pos[:], m2[1][:], LN_THIRD, AL.is_ge, None, AL.add,
               acc=tacc[:])
            nc.vector.tensor_tensor(out=nposA[:], in0=nposA[:], in1=tacc[:],
                                    op=AL.add)
            bpm = persist.tile([P, W], f16, name="bpm")
            nc.gpsimd.memset(bpm[:], 0.0)
            nc.vector.tensor_tensor(out=bpm[:, 0:COLS], in0=pos[:],
                                    in1=bestp[:], op=AL.mult)
            # xa/ya/iwa/iha/La/Ha (anchors still alive)
            with tc.tile_pool(name="prepp", bufs=1) as prepp:
                wa = prepp.tile([P, COLS], f16, name="wa")
                ha = prepp.tile([P, COLS], f16, name="ha")
                nc.vector.tensor_tensor(out=wa[:], in0=ax2[:], in1=ax1[:],
                                        op=AL.subtract)
                nc.gpsimd.tensor_tensor(out=ha[:], in0=ay2[:], in1=ay1[:],
                                        op=AL.subtract)
                nc.gpsimd.tensor_tensor(out=xa[:], in0=ax1[:], in1=ax2[:],
                                        op=AL.add)
                ts(nc.gpsimd, xa[:], xa[:], 0.5, AL.mult)
                nc.vector.tensor_tensor(out=ya[:], in0=ay1[:], in1=ay2[:],
                                        op=AL.add)
                ts(nc.vector, ya[:], ya[:], 0.5, AL.mult)
                with nc.allow_low_precision(reason="f16 reg-target recips"):
                    nc.vector.reciprocal(iwa[:], wa[:])
                    nc.vector.reciprocal(iha[:], ha[:])
                nc.scalar.activation(La[:], wa[:], AF.Ln)
                nc.scalar.activation(Ha[:], ha[:], AF.Ln)
            pA_stack.close()

            # ---------- phase B ----------
            with ExitStack() as bctx:
                ohp = bctx.enter_context(tc.tile_pool(name="ohp", bufs=4))
                psum_t = bctx.enter_context(
                    tc.tile_pool(name="psum_t", bufs=4, space="PSUM"))
                psum_g = bctx.enter_context(
                    tc.tile_pool(name="psum_g", bufs=4, space="PSUM"))
                gath_p = bctx.enter_context(tc.tile_pool(name="gath", bufs=2))
                scr = bctx.enter_context(tc.tile_pool(name="scr", bufs=1))

                sc = [scr.tile([P, WH], f16, name=f"sc{i}") for i in range(8)]
                accp = [persist.tile([P, 1], f32, name=f"accp{i}")
                        for i in range(2)]

                pending = []

                def emit_tail(gath, base, rw):
                    tail = []
                    tail_q0 = []

                    def gplq_f(mm, q0, q1):
                        return gath[:, mm * WH + q0:mm * WH + q1]

                    for q0, q1 in ((0, rw // 2), (rw // 2, rw)):
                        qw = q1 - q0

                        def mk_reg(k, ctr_t, inv_t, lg_t, q0=q0, q1=q1, qw=qw):
                            def go():
                                posh = pos[:, base + q0:base + q1]
                                s1, s2_, s3, s4 = sc[4 * (k % 2):4 * (k % 2) + 4]
                                rt = s1
                                if lg_t is None:
                                    nc.vector.tensor_tensor(
                                        out=s2_[:, :qw],
                                        in0=gplq_f(k, q0, q1),
                                        in1=ctr_t[:, base + q0:base + q1],
                                        op=AL.subtract)
                                    nc.vector.tensor_tensor(
                                        out=rt[:, :qw], in0=s2_[:, :qw],
                                        in1=inv_t[:, base + q0:base + q1],
                                        op=AL.mult)
                                else:
                                    nc.vector.tensor_tensor(
                                        out=rt[:, :qw], in0=gplq_f(k, q0, q1),
                                        in1=lg_t[:, base + q0:base + q1],
                                        op=AL.subtract)
                                e = s2_
                                nc.vector.tensor_tensor(
                                    out=e[:, :qw],
                                    in0=rpp[k][:, base + q0:base + q1],
                                    in1=rt[:, :qw], op=AL.subtract)
                                q = s3
                                nc.scalar.activation(q[:, :qw], e[:, :qw],
                                                     AF.Abs)
                                qm = s4
                                nc.vector.tensor_tensor(out=qm[:, :qw],
                                                        in0=q[:, :qw],
                                                        in1=posh, op=AL.mult)
                                cm = s1
                                ts(nc.vector, cm[:, :qw], qm[:, :qw], BETA,
                                   AL.min)
                                q2 = s3
                                nc.vector.tensor_tensor(
                                    out=q2[:, :qw], in0=qm[:, :qw],
                                    in1=qm[:, :qw], op=AL.add)
                                nc.vector.tensor_tensor(
                                    out=q2[:, :qw], in0=q2[:, :qw],
                                    in1=cm[:, :qw], op=AL.subtract)
                                nc.vector.scalar_tensor_tensor(
                                    out=s4[:, :qw], in0=cm[:, :qw], scalar=0.0,
                                    in1=q2[:, :qw], op0=AL.add, op1=AL.mult,
                                    accum_out=accp[k % 2][:])
                                nc.vector.tensor_tensor(
                                    out=sl1A[:], in0=sl1A[:],
                                    in1=accp[k % 2][:], op=AL.add)
                            return go

                        for k, (ctr_t, inv_t, lg_t) in enumerate(
                                ((xa, iwa, None), (ya, iha, None),
                                 (None, None, La), (None, None, Ha))):
                            tail.append(mk_reg(k, ctr_t, inv_t, lg_t))

                        def mk_corr(c, q0=q0, q1=q1, qw=qw):
                            def go():
                                eqc = sc[4 + (c % 2)]
                                # table holds label+1: background matches none
                                ts(nc.vector, eqc[:, :qw], gplq_f(4, q0, q1),
                                   float(c + 1), AL.is_equal)
                                cc = sc[6 + (c % 2)]
                                nc.vector.scalar_tensor_tensor(
                                    out=cc[:, :qw], in0=eqc[:, :qw],
                                    scalar=0.0,
                                    in1=Rp[c][:, base + q0:base + q1],
                                    op0=AL.add, op1=AL.mult,
                                    accum_out=accp[c % 2][:])
                                nc.vector.tensor_tensor(
                                    out=corrA[:], in0=corrA[:],
                                    in1=accp[c % 2][:], op=AL.add)
                            return go

                        for c in range(C):
                            tail.append(mk_corr(c))
                        if q0 == 0:
                            tail_q0 = tail
                            tail = []
                    return tail_q0, tail

                for half in range(2):
                    base = half * WH
                    rw = min(COLS - base, WH)
                    if rw <= 0:
                        break
                    gath = gath_p.tile([P, NF * WH], f16, name="gath")
                    tail_q0, tail_q1 = emit_tail(gath, base, rw)

                    for s in range(HS):
                        sq = half * HS + s
                        oh = ohp.tile([P, 512], f16, name="oh")
                        srcx = _expand32(bpm[:, 16 * sq:16 * sq + 16])
                        if (sq % 10) < ohd:
                            nc.vector.tensor_tensor(
                                out=oh[:].rearrange("p (f j) -> p f j", j=G),
                                in0=srcx,
                                in1=iotarep[:].rearrange("p (f j) -> p f j",
                                                         j=G),
                                op=AL.is_equal)
                        else:
                            # Pool: e = bpm - iota; DVE: oh = (e == 0)
                            nc.gpsimd.tensor_tensor(
                                out=oh[:].rearrange("p (f j) -> p f j", j=G),
                                in0=srcx,
                                in1=iotarep[:].rearrange("p (f j) -> p f j",
                                                         j=G),
                                op=AL.subtract)
                            ts(nc.vector, oh[:], oh[:], 0.0, AL.is_equal)
                        pt = psum_t.tile([P, 512], f16, name="pt")
                        for t4 in range(4):
                            nc.tensor.transpose(pt[:, 128 * t4:128 * t4 + 128],
                                                oh[:, 128 * t4:128 * t4 + 128],
                                                ident[:])
                        ohT = ohp.tile([P, 512], f16, name="ohT")
                        nc.scalar.copy(ohT[:], pt[:])
                        gp = psum_g.tile([P, 4 * NF * 4], f32, name="gp")
                        for t4 in range(4):
                            nc.tensor.matmul(
                                out=gp[:, 4 * NF * t4:4 * NF * t4 + 4 * NF],
                                lhsT=ohT[:, 128 * t4:128 * t4 + 128],
                                rhs=tt16[:], start=True, stop=True)
                        src_g = gp[:].rearrange("p (t f mm) -> p t f mm",
                                                t=4, f=4)
                        dst = gath[:]
                        dst_ap = dataclasses.replace(
                            dst, offset=dst.offset + 16 * s,
                            ap=[dst.ap[0], [4, 4], [1, 4], [WH, NF]])
                        if s % 8 < 2:
                            nc.vector.tensor_copy(dst_ap, src_g)
                        else:
                            nc.scalar.copy(dst_ap, src_g)
                        # interleave reg/corr work: earlier halves first,
                        # then this half's first chunk once its columns are
                        # scattered (sqs 0..19 cover chunk q0)
                        if pending:
                            pending.pop(0)()
                        elif s >= 26 and tail_q0:
                            tail_q0.pop(0)()

                    pending = pending + tail_q0 + tail_q1
                for go in pending:
                    go()

            # ---------- final cross-partition reduce ----------
            acc4 = persist.tile([P, 4], f32, name="acc4")
            nc.scalar.copy(acc4[:, 0:1], nposA[:])
            nc.scalar.copy(acc4[:, 1:2], sl1A[:])
            nc.scalar.copy(acc4[:, 2:3], nsumA[:])
            nc.scalar.copy(acc4[:, 3:4], corrA[:])
            with tc.tile_pool(name="psum_f", bufs=1, space="PSUM") as pf:
                fps = pf.tile([1, 4], f32, name="fps")
                nc.tensor.matmul(out=fps[:], lhsT=ones1[:], rhs=acc4[:],
                                 start=True, stop=True)
                osb = persist.tile([1, 4], f32, name="osb")
                nc.scalar.copy(osb[:], fps[:])
                nc.sync.dma_start(out[:], osb[:])

    return nc


def build_for_timing():
    patch_tile_drain(1)
    nc = build(160000)
    split_sync_waits(nc)
    return nc


# ---------------- host side ----------------

def pack_inputs(cls_preds, reg_preds, anchors, gt_boxes, gt_labels):
    """Full inputs -> list of 8 per-core input maps (planar f16 layouts)."""
    B, A, _ = cls_preds.shape
    anch = np.ascontiguousarray(
        (anchors.astype(np.float32).T * np.float32(CSCALE)).astype(np.float16))
    maps = []
    for b in range(B):
        clsp = np.ascontiguousarray(
            cls_preds[b].astype(np.float32).T.astype(np.float16))
        regp = np.ascontiguousarray(
            reg_preds[b].astype(np.float32).T.astype(np.float16))
        gb = gt_boxes[b].astype(np.float32) * np.float32(CSCALE)
        gx1, gy1, gx2, gy2 = gb[:, 0], gb[:, 1], gb[:, 2], gb[:, 3]
        wg = gx2 - gx1
        hg = gy2 - gy1
        aB = wg * hg
        xg = (gx1 + gx2) * np.float32(0.5)
        yg = (gy1 + gy2) * np.float32(0.5)
        lwg = np.log(wg)
        lhg = np.log(hg)
        lab1 = gt_labels[b].astype(np.float32) + np.float32(1.0)
        gtaux = np.concatenate(
            [gx1, gy1, gx2, gy2, aB, xg, yg, lwg, lhg, lab1, -gx1]
        ).astype(np.float32)[None, :]
        maps.append({"anch": anch, "clsp": clsp, "regp": regp, "gtaux": gtaux})
    return maps


def finish(partials):
    """partials: list of [1,4] arrays per core -> (cls_loss, reg_loss)."""
    f = np.float32
    npos = f(0); sl1 = f(0); nsum = f(0); corr = f(0)
    for p in partials:
        p = p.reshape(4)
        npos += f(p[0]); sl1 += f(p[1]); nsum += f(p[2]); corr += f(p[3])
    denom = max(float(npos), 1.0)
    if npos > 0:
        cls_loss = f(0.75) * (nsum + corr) / f(denom)
        reg_loss = sl1 / f(2 * BETA) / f(denom)
    else:
        cls_loss = f(0.0); reg_loss = f(0.0)
    return np.float32(cls_loss), np.float32(reg_loss)


# ---------------- self-contained kernel entry ----------------

_CACHE = {}


def _get_fn(n_cores=8):
    if "fn" in _CACHE:
        return _CACHE["fn"]
    import jax
    from jax.sharding import Mesh, PartitionSpec, NamedSharding
    from jax.experimental.shard_map import shard_map
    from concourse.bass2jax import (_bass_exec_p, install_neuronx_cc_hook,
                                    partition_id_tensor)
    nc = build_for_timing()
    install_neuronx_cc_hook()
    in_names, out_names, out_avals, zero_shapes = [], [], [], []
    partition_name = (nc.partition_id_tensor.name
                      if nc.partition_id_tensor else None)
    for alloc in nc.m.functions[0].allocations:
        if not isinstance(alloc, mybir.MemoryLocationSet):
            continue
        name = alloc.memorylocations[0].name
        if alloc.kind == "ExternalInput":
            if name != partition_name:
                in_names.append(name)
        elif alloc.kind == "ExternalOutput":
            out_names.append(name)
            shape = tuple(alloc.tensor_shape)
            dtype = mybir.dt.np(alloc.dtype)
            out_avals.append(jax.core.ShapedArray(shape, dtype))
            zero_shapes.append((shape, dtype))
    n_params = len(in_names)
    n_outs = len(out_avals)
    all_in_names = in_names + out_names + ([partition_name]
                                           if partition_name else [])
    donate = tuple(range(n_params, n_params + n_outs))

    def _body(*args):
        operands = list(args)
        if partition_name is not None:
            operands.append(partition_id_tensor())
        outs = _bass_exec_p.bind(
            *operands, out_avals=tuple(out_avals),
            in_names=tuple(all_in_names), out_names=tuple(out_names),
            lowering_input_output_aliases=(),
            sim_require_finite=True, sim_require_nnan=True, nc=nc)
        return tuple(outs)

    devices = jax.devices()[:n_cores]
    mesh = Mesh(np.asarray(devices), ("core",))
    in_specs = (PartitionSpec("core"),) * (n_params + n_outs)
    out_specs = (PartitionSpec("core"),) * len(out_names)
    fn = jax.jit(shard_map(_body, mesh=mesh, in_specs=in_specs,
                           out_specs=out_specs, check_rep=False),
                 donate_argnums=donate, keep_unused=True)
    sh = NamedSharding(mesh, PartitionSpec("core"))
    _CACHE["fn"] = (fn, in_names, out_names, out_avals, zero_shapes, sh,
                    n_cores)
    return _CACHE["fn"]


def kernel(cls_preds, reg_preds, anchors, gt_boxes, gt_labels):
    """Full-input DetectionLoss on 8 NeuronCores (data-parallel over batch).

    Returns (cls_loss, reg_loss) as float32 scalars, matching reference()."""
    import jax
    cls_preds = np.asarray(cls_preds)
    reg_preds = np.asarray(reg_preds)
    anchors = np.asarray(anchors)
    gt_boxes = np.asarray(gt_boxes)
    gt_labels = np.asarray(gt_labels)
    B, A, _ = cls_preds.shape
    assert (B, A) == (8, 160000), (B, A)
    maps = pack_inputs(cls_preds, reg_preds, anchors, gt_boxes, gt_labels)
    fn, in_names, out_names, out_avals, zero_shapes, sh, n_cores = _get_fn()
    concat_in = [jax.device_put(
        np.concatenate([np.asarray(maps[c][nm]) for c in range(n_cores)],
                       axis=0), sh) for nm in in_names]
    zeros = [jax.device_put(
        np.zeros((n_cores * s[0], *s[1:]), d), sh) for s, d in zero_shapes]
    out_arrs = fn(*concat_in, *zeros)
    res = np.asarray(out_arrs[out_names.index("out")]).reshape(n_cores, 1, 4)
    partials = [res[c] for c in range(n_cores)]
    cls_loss, reg_loss = finish(partials)
    return cls_loss, reg_loss
